# revision 1
# baseline (speedup 1.0000x reference)
"""AttentiveGRU2 Trainium2 Bass kernel.

Model (see reference):
  edge-softmax over incoming edges per dst node, attention-weighted
  gather of projected node features, segment-sum per dst, ELU, GRUCell.

Strategy (8 NeuronCores, SPMD, no collectives):
  * Host sorts edges by dst. Nodes are grouped into 392 windows of 128
    consecutive ids; each core owns 49 windows (6272 node slots).
  * Softmax shift-invariance: a_e = exp(l_e)/sum exp(l_e) without the
    segment max (logits are N(0,1); exp is safe in fp32).
  * The per-edge division by the segment denominator is folded through the
    segment sum:  c_v = W @ (sum_e ex_e nf[src_e]) / (sum_e ex_e) + b.
    Edge phase per 128-edge tile: hardware DMA gather of nf rows
    (InstDMAGatherAnt), one scaled one-hot build on DVE (2 ops), two PE
    matmuls accumulating psum_u += O.T @ G and psum_d += O.T @ 1.
  * dma_gather needs int16 indices but V=50000 > 32767, so the nf table is
    addressed through two overlapping row views: A = rows [0, 32768)
    (src < 32768) and B = rows [17232, 50000) (idx = src - 17232).  Each
    window's edges are grouped A-first/B-second with fixed global slot
    counts (slots_A/slots_B) so the instruction stream is identical on all
    cores; pad slots gather row 0 and are killed by dst_local = -1.
  * Node phase per window: ctx~ = psum_u / max(psum_d, eps) (per-partition),
    one PE transpose, cT = W_proj @ ctx~.T with W stationary, ELU, GRU
    gates with gi+gh fused in PSUM, blend, relu, DMA out.
"""

import numpy as np

V, E, F = 50000, 800000, 128
NC = 8
WPC = 49              # windows per core
NPC = WPC * 128       # 6272 node slots per core
WTOT = NC * WPC       # 392 windows total
WPB = 2               # windows per gather batch
S_SPLIT = 32768       # src < S -> table A
OFF_B = V - 32768     # 17232; table B rows [OFF_B, V)

_compiled = {}


def _build_nc(T_win, sA=None, sB=None, skip_gather=False, skip_onehot=False,
              skip_mm=False, skip_node=False, repeat=1, one_act=False):
    import concourse.bass as bass
    import concourse.bacc as bacc
    import concourse.mybir as mybir
    import concourse.tile as tile

    f32 = mybir.dt.float32
    i16 = mybir.dt.int16
    AF = mybir.ActivationFunctionType
    OP = mybir.AluOpType
    AF_E = AF.Sigmoid if one_act else AF.Exp
    AF_T = AF.Sigmoid if one_act else AF.Tanh
    AF_R = AF.Sigmoid if one_act else AF.Relu

    if sA is None:
        sA, sB = T_win, 0   # legacy path unused
    SW = sA + sB            # slots per window
    T = WPC * SW            # tile-columns per core
    LA = WPC * sA * 128     # A-gather idx count per core
    LB = WPC * sB * 128

    nc = bacc.Bacc("TRN2", target_bir_lowering=False, debug=False,
                   num_devices=NC)

    # ---- DRAM parameters ----
    idxa_d = nc.dram_tensor("idxa", [128, LA // 16], i16,
                            kind="ExternalInput")
    idxb_d = nc.dram_tensor("idxb", [128, LB // 16], i16,
                            kind="ExternalInput")
    dstla_d = nc.dram_tensor("dstla", [128, WPC * sA], f32,
                             kind="ExternalInput")
    dstlb_d = nc.dram_tensor("dstlb", [128, WPC * sB], f32,
                             kind="ExternalInput")
    logita_d = nc.dram_tensor("logita", [128, WPC * sA], f32,
                              kind="ExternalInput")
    logitb_d = nc.dram_tensor("logitb", [128, WPC * sB], f32,
                              kind="ExternalInput")
    table_d = nc.dram_tensor("table", [V, F], f32, kind="ExternalInput")
    nfT_d = nc.dram_tensor("nfT", [128, NPC], f32, kind="ExternalInput")
    wprojT_d = nc.dram_tensor("wprojT", [128, 128], f32, kind="ExternalInput")
    wihT_d = nc.dram_tensor("wihT", [128, 384], f32, kind="ExternalInput")
    whhT_d = nc.dram_tensor("whhT", [128, 384], f32, kind="ExternalInput")
    bproj_d = nc.dram_tensor("bproj", [1, 128], f32, kind="ExternalInput")
    brz_d = nc.dram_tensor("brz", [1, 256], f32, kind="ExternalInput")
    bni_d = nc.dram_tensor("bni", [1, 128], f32, kind="ExternalInput")
    bnh_d = nc.dram_tensor("bnh", [1, 128], f32, kind="ExternalInput")
    iota_d = nc.dram_tensor("iota", [128, 128], f32, kind="ExternalInput")
    ident_d = nc.dram_tensor("ident", [128, 128], f32, kind="ExternalInput")
    onesc_d = nc.dram_tensor("onesc", [128, 1], f32, kind="ExternalInput")
    onesr_d = nc.dram_tensor("onesr", [1, 128], f32, kind="ExternalInput")
    tableb_d = nc.dram_tensor("tableb", [32768, 128], f32,
                              kind="ExternalInput")
    out_d = nc.dram_tensor("out", [NPC, 128], f32, kind="ExternalOutput")

    tabA = table_d[0:32768, :]
    tabB = tableb_d[:]

    with tile.TileContext(nc) as tc:
        with (
            tc.tile_pool(name="const", bufs=1) as cpool,
            tc.tile_pool(name="gat", bufs=2) as gpool,
            tc.tile_pool(name="oh", bufs=2) as opool,
            tc.tile_pool(name="wrk", bufs=2) as wpool,
            tc.tile_pool(name="pedge", bufs=1, space="PSUM") as pe_pool,
            tc.tile_pool(name="pnode", bufs=1, space="PSUM") as pn_pool,
        ):
            def load(pool, name, dram, shape, dtype=f32):
                t = pool.tile(shape, dtype, tag=name)
                nc.sync.dma_start(t[:], dram[:])
                return t

            iota_sb = load(cpool, "iota", iota_d, [128, 128])
            ident_sb = load(cpool, "ident", ident_d, [128, 128])
            onesc_sb = load(cpool, "onesc", onesc_d, [128, 1])
            onesr_sb = load(cpool, "onesr", onesr_d, [1, 128])
            wproj_sb = load(cpool, "wproj", wprojT_d, [128, 128])
            wih_sb = load(cpool, "wih", wihT_d, [128, 384])
            whh_sb = load(cpool, "whh", whhT_d, [128, 384])
            bproj_sb = load(cpool, "bproj", bproj_d, [1, 128])
            brz_sb = load(cpool, "brz", brz_d, [1, 256])
            bni_sb = load(cpool, "bni", bni_d, [1, 128])
            bnh_sb = load(cpool, "bnh", bnh_d, [1, 128])
            idxa_sb = load(cpool, "idxa", idxa_d, [128, LA // 16], i16)
            idxb_sb = load(cpool, "idxb", idxb_d, [128, LB // 16], i16)
            dstla_sb = load(cpool, "dstla", dstla_d, [128, WPC * sA])
            dstlb_sb = load(cpool, "dstlb", dstlb_d, [128, WPC * sB])
            nfT_sb = load(cpool, "nfT", nfT_d, [128, NPC])

            exa_sb = cpool.tile([128, WPC * sA], f32, tag="exa")
            nc.sync.dma_start(exa_sb[:], logita_d[:])
            nc.scalar.activation(exa_sb[:], exa_sb[:], AF.Exp)
            exb_sb = cpool.tile([128, WPC * sB], f32, tag="exb")
            nc.sync.dma_start(exb_sb[:], logitb_d[:])
            nc.scalar.activation(exb_sb[:], exb_sb[:], AF.Exp)

            def apx(base, dims):
                return bass.AP(base.tensor, base.offset,
                               [list(base.ap[0])] + dims)

            n_batches = (WPC + WPB - 1) // WPB
            GA_static = GB_static = None
            if skip_gather:
                GA_static = cpool.tile([128, WPB * sA, 128], f32, tag="GAs")
                nc.gpsimd.memset(GA_static[:], 0.0)
                GB_static = cpool.tile([128, WPB * sB, 128], f32, tag="GBs")
                nc.gpsimd.memset(GB_static[:], 0.0)


            for _rep in range(repeat):
              for b in range(n_batches):
                w0 = b * WPB
                nw = min(WPB, WPC - w0)
                if skip_gather:
                    GA, GB = GA_static, GB_static
                else:
                    GA = gpool.tile([128, WPB * sA, 128], f32, tag="GA")
                    GB = gpool.tile([128, WPB * sB, 128], f32, tag="GB")
                    na = nw * sA * 128
                    nc.gpsimd.dma_gather(
                        out_ap=GA[:, 0:nw * sA, :],
                        in_ap=tabA,
                        idxs_ap=idxa_sb[:, (w0 * sA * 128) // 16:
                                        ((w0 + nw) * sA * 128) // 16],
                        num_idxs=na, num_idxs_reg=na, elem_size=128,
                        single_packet=False,
                    )
                    nb = nw * sB * 128
                    nc.gpsimd.dma_gather(
                        out_ap=GB[:, 0:nw * sB, :],
                        in_ap=tabB,
                        idxs_ap=idxb_sb[:, (w0 * sB * 128) // 16:
                                        ((w0 + nw) * sB * 128) // 16],
                        num_idxs=nb, num_idxs_reg=nb, elem_size=128,
                        single_packet=False,
                    )
                ntA, ntB = nw * sA, nw * sB
                cA0, cB0 = w0 * sA, w0 * sB
                OA = opool.tile([128, WPB * sA, 128], f32, tag="OA")
                OB = opool.tile([128, WPB * sB, 128], f32, tag="OB")
                GsA = gpool.tile([128, WPB * sA, 132], f32, tag="GsA")
                GsB = gpool.tile([128, WPB * sB, 132], f32, tag="GsB")
                if not skip_onehot:
                    for (O, dstl_sb, nt, c0) in (
                            (OA, dstla_sb, ntA, cA0),
                            (OB, dstlb_sb, ntB, cB0)):
                        nc.vector.tensor_tensor(
                            out=O[:, 0:nt, :],
                            in0=apx(iota_sb[:], [[0, nt], [1, 128]]),
                            in1=apx(dstl_sb[:, c0:c0 + nt],
                                    [[1, nt], [0, 128]]),
                            op=OP.is_equal)
                for (G, Gs, ex_sb, nt, c0) in (
                        (GA, GsA, exa_sb, ntA, cA0),
                        (GB, GsB, exb_sb, ntB, cB0)):
                    nc.vector.tensor_tensor(
                        out=Gs[:, 0:nt, 0:128], in0=G[:, 0:nt, :],
                        in1=apx(ex_sb[:, c0:c0 + nt], [[1, nt], [0, 128]]),
                        op=OP.mult)
                    nc.vector.tensor_copy(out=Gs[:, 0:nt, 128:129],
                                          in_=ex_sb[:, c0:c0 + nt])
                for wl in range(nw):
                    w = w0 + wl
                    psum_ud = pe_pool.tile([128, 132], f32, tag="psum_ud",
                                           bufs=2)
                    if not skip_mm:
                        for s_ in range(SW):
                            if s_ < sA:
                                Olh = OA[:, wl * sA + s_, :]
                                Grh = GsA[:, wl * sA + s_, 0:129]
                            else:
                                Olh = OB[:, wl * sB + (s_ - sA), :]
                                Grh = GsB[:, wl * sB + (s_ - sA), 0:129]
                            nc.tensor.matmul(
                                psum_ud[:, 0:129], lhsT=Olh, rhs=Grh,
                                start=(s_ == 0), stop=(s_ == SW - 1),
                            )

                    if skip_node:
                        continue
                    # ---- node phase for window w ----
                    den = wpool.tile([128, 1], f32, tag="den")
                    nc.vector.tensor_scalar(
                        out=den[:], in0=psum_ud[:, 128:129], scalar1=1e-30,
                        scalar2=None, op0=OP.max)
                    rec = wpool.tile([128, 1], f32, tag="rec")
                    nc.vector.reciprocal(rec[:], den[:])
                    ctx_t = wpool.tile([128, 128], f32, tag="ctx_t")
                    nc.vector.tensor_scalar(
                        out=ctx_t[:], in0=psum_ud[:, 0:128],
                        scalar1=rec[:, 0:1],
                        scalar2=None, op0=OP.mult)

                    ptr = pn_pool.tile([128, 128], f32, tag="ptr", bufs=2)
                    nc.tensor.transpose(ptr[:], ctx_t[:], ident_sb[:])
                    ctxT = wpool.tile([128, 128], f32, tag="ctxT")
                    nc.vector.tensor_copy(out=ctxT[:], in_=ptr[:])

                    # cT = W_proj @ ctx~.T + b_proj  (H on partitions)
                    psum_cT = pn_pool.tile([128, 128], f32, tag="psum_cT",
                                           bufs=2)
                    nc.tensor.matmul(psum_cT[:], lhsT=wproj_sb[:],
                                     rhs=ctxT[:], start=True, stop=False)
                    nc.tensor.matmul(psum_cT[:], lhsT=bproj_sb[:],
                                     rhs=onesr_sb[:], start=False, stop=True)

                    # elu(cT) = max(cT,0) + exp(min(cT,0)) - 1
                    cmin = wpool.tile([128, 128], f32, tag="cmin")
                    nc.vector.tensor_scalar(out=cmin[:], in0=psum_cT[:],
                                            scalar1=0.0, scalar2=None,
                                            op0=OP.min)
                    cexp = wpool.tile([128, 128], f32, tag="cexp")
                    nc.scalar.activation(cexp[:], cmin[:], AF_E)
                    crelu = wpool.tile([128, 128], f32, tag="crelu")
                    nc.vector.tensor_scalar(out=crelu[:], in0=psum_cT[:],
                                            scalar1=0.0, scalar2=None,
                                            op0=OP.max)
                    ce1 = wpool.tile([128, 128], f32, tag="ce1")
                    nc.vector.tensor_scalar(out=ce1[:], in0=cexp[:],
                                            scalar1=1.0, scalar2=None,
                                            op0=OP.subtract)
                    ctxT2 = wpool.tile([128, 128], f32, tag="ctxT2")
                    nc.vector.tensor_tensor(out=ctxT2[:], in0=ce1[:],
                                            in1=crelu[:], op=OP.add)

                    nfT_tile = nfT_sb[:, w * 128:(w + 1) * 128]
                    # gates PSUM: [0:256]=r|z (gi+gh), [256:384]=i_n,
                    # [384:512]=h_n
                    psum_g = pn_pool.tile([128, 512], f32, tag="psum_g",
                                          bufs=2)
                    psum_rz = psum_g[:, 0:256]
                    nc.tensor.matmul(psum_rz, lhsT=ctxT2[:],
                                     rhs=wih_sb[:, 0:256],
                                     start=True, stop=False)
                    nc.tensor.matmul(psum_rz, lhsT=nfT_tile,
                                     rhs=whh_sb[:, 0:256],
                                     start=False, stop=False)
                    nc.tensor.matmul(psum_rz, lhsT=onesr_sb[:],
                                     rhs=brz_sb[:], start=False, stop=True)
                    psum_nh = psum_g[:, 256:512]
                    nc.tensor.matmul(psum_nh[:, 0:128], lhsT=ctxT2[:],
                                     rhs=wih_sb[:, 256:384],
                                     start=True, stop=False)
                    nc.tensor.matmul(psum_nh[:, 0:128], lhsT=onesr_sb[:],
                                     rhs=bni_sb[:], start=False, stop=True)
                    nc.tensor.matmul(psum_nh[:, 128:256], lhsT=nfT_tile,
                                     rhs=whh_sb[:, 256:384],
                                     start=True, stop=False)
                    nc.tensor.matmul(psum_nh[:, 128:256], lhsT=onesr_sb[:],
                                     rhs=bnh_sb[:], start=False, stop=True)

                    rzs = wpool.tile([128, 256], f32, tag="rzs")
                    nc.scalar.activation(rzs[:], psum_rz, AF.Sigmoid)
                    nt1 = wpool.tile([128, 128], f32, tag="nt1")
                    nc.vector.tensor_tensor(out=nt1[:], in0=rzs[:, 0:128],
                                            in1=psum_nh[:, 128:256],
                                            op=OP.mult)
                    nt2 = wpool.tile([128, 128], f32, tag="nt2")
                    nc.vector.tensor_tensor(out=nt2[:], in0=nt1[:],
                                            in1=psum_nh[:, 0:128],
                                            op=OP.add)
                    nn = wpool.tile([128, 128], f32, tag="nn")
                    nc.scalar.activation(nn[:], nt2[:], AF_T)

                    pnf = pn_pool.tile([128, 128], f32, tag="ptr", bufs=2)
                    nc.tensor.transpose(pnf[:], nfT_tile, ident_sb[:])
                    df = wpool.tile([128, 128], f32, tag="df")
                    nc.vector.tensor_tensor(out=df[:], in0=pnf[:], in1=nn[:],
                                            op=OP.subtract)
                    dz = wpool.tile([128, 128], f32, tag="dz")
                    nc.vector.tensor_tensor(out=dz[:], in0=df[:],
                                            in1=rzs[:, 128:256], op=OP.mult)
                    hh = wpool.tile([128, 128], f32, tag="hh")
                    nc.vector.tensor_tensor(out=hh[:], in0=dz[:], in1=nn[:],
                                            op=OP.add)
                    outt = wpool.tile([128, 128], f32, tag="outt")
                    nc.scalar.activation(outt[:], hh[:], AF_R)
                    nc.sync.dma_start(out_d[w * 128:(w + 1) * 128, :],
                                      outt[:])

    nc.compile()
    return nc


def _prep(edge_logits, node_feats, W_proj, b_proj, w_ih, w_hh, b_ih, b_hh,
          src, dst):
    """Host-side sharding. Returns (T_win, sA, sB, in_maps)."""
    logits = np.asarray(edge_logits, np.float32).reshape(-1)
    src = np.asarray(src, np.int64)
    dst = np.asarray(dst, np.int64)

    is_b = (src >= S_SPLIT).astype(np.int64)
    win = dst // 128
    key = win * 2 + is_b
    order = np.argsort(key, kind="stable")
    key_s = key[order]
    src_s = src[order]
    dst_s = dst[order]
    log_s = logits[order]

    counts = np.bincount(key_s, minlength=WTOT * 2)
    cA = counts[0::2]
    cB = counts[1::2]
    sA = int((cA.max() + 127) // 128)
    sB = int((cB.max() + 127) // 128)
    T_win = sA + sB

    starts = np.zeros(WTOT * 2, np.int64)
    starts[1:] = np.cumsum(counts)[:-1]
    pos = np.arange(E, dtype=np.int64) - starts[key_s]

    # flat slot index within the core-ordered [WTOT, sA*128 | sB*128] arrays
    winv = key_s // 2
    grp = key_s % 2
    idxA = np.zeros(WTOT * sA * 128, np.int16)
    idxB = np.zeros(WTOT * sB * 128, np.int16)
    dstlA = np.full(WTOT * sA * 128, -1.0, np.float32)
    dstlB = np.full(WTOT * sB * 128, -1.0, np.float32)
    logA = np.zeros(WTOT * sA * 128, np.float32)
    logB = np.zeros(WTOT * sB * 128, np.float32)

    mA = grp == 0
    mB = ~mA
    flatA = winv[mA] * (sA * 128) + pos[mA]
    flatB = winv[mB] * (sB * 128) + pos[mB]
    idxA[flatA] = src_s[mA].astype(np.int16)
    idxB[flatB] = (src_s[mB] - OFF_B).astype(np.int16)
    dstlA[flatA] = (dst_s[mA] - winv[mA] * 128).astype(np.float32)
    dstlB[flatB] = (dst_s[mB] - winv[mB] * 128).astype(np.float32)
    logA[flatA] = log_s[mA]
    logB[flatB] = log_s[mB]

    def core_tiles(a, slots):
        a = a.reshape(WTOT, slots, 128)
        return [np.ascontiguousarray(
            a[k * WPC:(k + 1) * WPC].transpose(2, 0, 1)
            .reshape(128, WPC * slots)) for k in range(NC)]

    dstlA_cores = core_tiles(dstlA, sA)
    dstlB_cores = core_tiles(dstlB, sB)
    logA_cores = core_tiles(logA, sA)
    logB_cores = core_tiles(logB, sB)

    def core_idx(a, slots):
        a = a.reshape(WTOT, slots * 128)
        out = []
        for k in range(NC):
            flat = a[k * WPC:(k + 1) * WPC].reshape(-1)
            blk = flat.reshape(-1, 16).T      # [16, L/16], i -> [i%16,i//16]
            out.append(np.ascontiguousarray(np.tile(blk, (8, 1))))
        return out

    idxA_cores = core_idx(idxA, sA)
    idxB_cores = core_idx(idxB, sB)

    nf = np.asarray(node_feats, np.float32)
    nf_pad = np.zeros((NC * NPC, F), np.float32)
    nf_pad[:V] = nf

    table = np.ascontiguousarray(nf)
    tableb = np.ascontiguousarray(nf[OFF_B:])
    wprojT = np.ascontiguousarray(np.asarray(W_proj, np.float32).T)
    wihT = np.ascontiguousarray(np.asarray(w_ih, np.float32).T)
    whhT = np.ascontiguousarray(np.asarray(w_hh, np.float32).T)
    bproj = np.asarray(b_proj, np.float32).reshape(1, 128)
    bih = np.asarray(b_ih, np.float32).reshape(384)
    bhh = np.asarray(b_hh, np.float32).reshape(384)
    brz = (bih[0:256] + bhh[0:256]).reshape(1, 256)
    bni = bih[256:384].reshape(1, 128)
    bnh = bhh[256:384].reshape(1, 128)
    iota = np.tile(np.arange(128, dtype=np.float32), (128, 1))
    ident = np.eye(128, dtype=np.float32)
    onesc = np.ones((128, 1), np.float32)
    onesr = np.ones((1, 128), np.float32)

    in_maps = []
    for k in range(NC):
        sl = nf_pad[k * NPC:(k + 1) * NPC]
        nfT = np.ascontiguousarray(sl.T)
        in_maps.append({
            "idxa": idxA_cores[k], "idxb": idxB_cores[k],
            "dstla": dstlA_cores[k], "dstlb": dstlB_cores[k],
            "logita": logA_cores[k], "logitb": logB_cores[k],
            "table": table, "tableb": tableb,
            "nfT": nfT,
            "wprojT": wprojT, "wihT": wihT, "whhT": whhT,
            "bproj": bproj, "brz": brz, "bni": bni, "bnh": bnh,
            "iota": iota, "ident": ident,
            "onesc": onesc, "onesr": onesr,
        })
    return T_win, sA, sB, in_maps


def kernel(edge_logits, node_feats, W_proj, b_proj, w_ih, w_hh, b_ih, b_hh,
           src, dst):
    from concourse.bass_utils import run_bass_kernel_spmd

    T_win, sA, sB, in_maps = _prep(edge_logits, node_feats, W_proj, b_proj,
                                   w_ih, w_hh, b_ih, b_hh, src, dst)
    key = (T_win, sA, sB)
    if key not in _compiled:
        _compiled[key] = _build_nc(T_win, sA=sA, sB=sB)
    nc = _compiled[key]

    res = run_bass_kernel_spmd(nc, in_maps, list(range(NC)))
    full = np.concatenate([res.results[k]["out"] for k in range(NC)], axis=0)
    return np.ascontiguousarray(full[:V]).astype(np.float32)



# revision 23
# speedup vs baseline: 1.7071x; 1.7071x over previous
"""AttentiveGRU2 Trainium2 Bass kernel (v2).

Model (see reference):
  edge-softmax over incoming edges per dst node, attention-weighted
  gather of projected node features, segment-sum per dst, ELU, GRUCell.

Strategy (8 NeuronCores, SPMD, no collectives):
  * Host sorts edges by dst window (392 windows of 128 node ids; 49
    windows per core). Softmax folded through the segment sum:
    ctx_v = (sum_e ex_e nf[src_e]) / (sum_e ex_e); proj applied after.
  * Gather: node-feature table in bf16 (256B rows), hardware
    InstDMAGatherAnt across 4 SWDGE queues (the per-queue descriptor
    rate ~8ns/desc is the kernel bottleneck; 4 queues x bf16 measured
    ~4x faster than the fp32 single-queue baseline). int16 idx limit
    handled with two overlapping row views (A: src<32768, B: src-17232).
  * One-hot dst matrices are 0/1 bf16 built on host and streamed in via
    regular DMA (cheap sequential traffic) -- the only on-device
    elementwise edge work is O_s = O01 * ex (split DVE/Pool engines).
  * Edge matmuls per 128-edge slot tile (bf16, 1 cyc/row):
      psum[v,0:128] += O_s^T @ G_raw,  psum[v,128] += O_s^T @ ones.
  * Node phase in [channel, node] layout, weights stationary, batched
    over NB=4 windows: ctx scaled by 1/den -> bf16 -> PE transpose ->
    proj + ELU -> GRU gates. Sigmoid avoided via 0.5*tanh(x/2)+0.5 so
    every activation (Exp/Tanh/Relu) lives in one table: zero 1283ns
    act-table reloads. Biases folded into activation bias APs.
  * Output written [feat, node]; host transposes back.
"""

import numpy as np

V, E, F = 50000, 800000, 128
NC = 8
WPC = 49              # windows per core
NPC = WPC * 128       # 6272 node slots per core
WTOT = NC * WPC       # 392 windows total
WPB = 2               # windows per gather batch
NB = 4                # windows per node-phase group
S_SPLIT = 32768       # src < S -> table A
OFF_B = V - 32768     # 17232; table B rows [OFF_B, V)

_compiled = {}


def _build_nc(T_win, sA=None, sB=None, skip_gather=False, skip_onehot=False,
              skip_mm=False, skip_node=False, repeat=1, one_act=False,
              nq_use=4, den_sep=True, tr_f32=False, den_seq=False,
              dump=None, ud1=False):
    import concourse.bass as bass
    import concourse.bacc as bacc
    import concourse.mybir as mybir
    import concourse.tile as tile

    f32 = mybir.dt.float32
    bf16 = mybir.dt.bfloat16
    i16 = mybir.dt.int16
    AF = mybir.ActivationFunctionType
    OP = mybir.AluOpType

    SW = sA + sB            # slots per window
    LA = WPC * sA * 128     # A-gather idx count per core
    LB = WPC * sB * 128
    CA = WPC * sA           # A slot-tiles per core
    CB = WPC * sB

    nc = bacc.Bacc("TRN2", target_bir_lowering=False, debug=False,
                   num_devices=NC, num_swdge_queues=4)

    # ---- DRAM parameters ----
    idxa_d = nc.dram_tensor("idxa", [128, LA // 16], i16, kind="ExternalInput")
    idxb_d = nc.dram_tensor("idxb", [128, LB // 16], i16, kind="ExternalInput")
    o01a_d = nc.dram_tensor("o01a", [128, CA * 128], bf16,
                            kind="ExternalInput")
    o01b_d = nc.dram_tensor("o01b", [128, CB * 128], bf16,
                            kind="ExternalInput")
    loga_d = nc.dram_tensor("loga", [128, CA], f32, kind="ExternalInput")
    logb_d = nc.dram_tensor("logb", [128, CB], f32, kind="ExternalInput")
    tab16_d = nc.dram_tensor("tab16", [V, F], bf16, kind="ExternalInput")
    tabb16_d = nc.dram_tensor("tabb16", [32768, F], bf16,
                              kind="ExternalInput")
    nfT_d = nc.dram_tensor("nfT", [128, NPC], f32, kind="ExternalInput")
    nfT16_d = nc.dram_tensor("nfT16", [128, NPC], bf16, kind="ExternalInput")
    wproj16_d = nc.dram_tensor("wproj16", [128, 128], bf16,
                               kind="ExternalInput")
    wih16_d = nc.dram_tensor("wih16", [128, 384], bf16, kind="ExternalInput")
    whh16_d = nc.dram_tensor("whh16", [128, 384], bf16, kind="ExternalInput")
    bproj_d = nc.dram_tensor("bprojc", [128, 1], f32, kind="ExternalInput")
    brh_d = nc.dram_tensor("brh", [128, 1], f32, kind="ExternalInput")
    bzh_d = nc.dram_tensor("bzh", [128, 1], f32, kind="ExternalInput")
    bni_d = nc.dram_tensor("bnic", [128, 1], f32, kind="ExternalInput")
    bnhh_d = nc.dram_tensor("bnhh", [128, 1], f32, kind="ExternalInput")
    ident16_d = nc.dram_tensor("ident16", [128, 128], bf16,
                               kind="ExternalInput")
    identf_d = nc.dram_tensor("identf", [128, 128], f32,
                              kind="ExternalInput")
    onesc16_d = nc.dram_tensor("onesc16", [128, 1], bf16,
                               kind="ExternalInput")
    out_d = nc.dram_tensor("out", [128, NPC], f32, kind="ExternalOutput")
    if dump == "g":
        dump_d = nc.dram_tensor("dmp", [128, CA * 128], bf16,
                                kind="ExternalOutput")
    elif dump:
        dump_d = nc.dram_tensor("dmp", [NPC, 128], f32,
                                kind="ExternalOutput")

    tabA = tab16_d[0:32768, :]
    tabB = tabb16_d[:]

    def apx(base, dims):
        return bass.AP(base.tensor, base.offset,
                       [list(base.ap[0])] + dims)

    with tile.TileContext(nc) as tc:
        with (
            tc.tile_pool(name="const", bufs=1) as cpool,
            tc.tile_pool(name="gat", bufs=2) as gpool,
            tc.tile_pool(name="oh", bufs=2) as opool,
            tc.tile_pool(name="wrk", bufs=1) as wpool,
            tc.tile_pool(name="brdg", bufs=2) as bpool,
            tc.tile_pool(name="outp", bufs=2) as qpool,
            tc.tile_pool(name="pedge", bufs=1, space="PSUM") as pe_pool,
            tc.tile_pool(name="ptr", bufs=1, space="PSUM") as ptr_pool,
            tc.tile_pool(name="pnode", bufs=1, space="PSUM") as pn_pool,
        ):
            def load(pool, name, dram, shape, dtype=f32):
                t = pool.tile(shape, dtype, tag=name)
                nc.sync.dma_start(t[:], dram[:])
                return t

            ident_sb = load(cpool, "ident16", ident16_d, [128, 128], bf16)
            identf_sb = load(cpool, "identf", identf_d, [128, 128], f32)
            onesc_sb = load(cpool, "onesc16", onesc16_d, [128, 1], bf16)
            wproj_sb = load(cpool, "wproj16", wproj16_d, [128, 128], bf16)
            wih_sb = load(cpool, "wih16", wih16_d, [128, 384], bf16)
            whh_sb = load(cpool, "whh16", whh16_d, [128, 384], bf16)
            bproj_sb = load(cpool, "bprojc", bproj_d, [128, 1])
            brh_sb = load(cpool, "brh", brh_d, [128, 1])
            bzh_sb = load(cpool, "bzh", bzh_d, [128, 1])
            bni_sb = load(cpool, "bnic", bni_d, [128, 1])
            bnhh_sb = load(cpool, "bnhh", bnhh_d, [128, 1])
            idxa_sb = load(cpool, "idxa", idxa_d, [128, LA // 16], i16)
            idxb_sb = load(cpool, "idxb", idxb_d, [128, LB // 16], i16)
            nfT_sb = load(cpool, "nfT", nfT_d, [128, NPC])
            nfT16_sb = load(cpool, "nfT16", nfT16_d, [128, NPC], bf16)

            # ex = exp(logits), bf16 (softmax shift-invariance: no seg-max;
            # logits are N(0,1) so fp32 exp is safe)
            loga_sb = load(cpool, "loga", loga_d, [128, CA])
            exa_sb = cpool.tile([128, CA], bf16, tag="exa")
            nc.scalar.activation(exa_sb[:], loga_sb[:], AF.Exp)
            logb_sb = load(cpool, "logb", logb_d, [128, CB])
            exb_sb = cpool.tile([128, CB], bf16, tag="exb")
            nc.scalar.activation(exb_sb[:], logb_sb[:], AF.Exp)

            GA_static = GB_static = None
            if skip_gather:
                GA_static = cpool.tile([128, WPB * sA, 128], bf16, tag="GAs")
                nc.gpsimd.memset(GA_static[:], 0.0)
                GB_static = cpool.tile([128, WPB * sB, 128], bf16, tag="GBs")
                nc.gpsimd.memset(GB_static[:], 0.0)
            OA_static = OB_static = None
            if skip_onehot:
                OA_static = cpool.tile([128, WPB * sA, 128], bf16, tag="OAs")
                nc.gpsimd.memset(OA_static[:], 0.0)
                OB_static = cpool.tile([128, WPB * sB, 128], bf16, tag="OBs")
                nc.gpsimd.memset(OB_static[:], 0.0)

            n_batches = (WPC + WPB - 1) // WPB
            qi = 0
            for _rep in range(repeat):
              # node-group state: transpose psum + sbuf ctxT for NB windows
              for b in range(n_batches):
                w0 = b * WPB
                nw = min(WPB, WPC - w0)
                ntA, ntB = nw * sA, nw * sB
                cA0, cB0 = w0 * sA, w0 * sB
                if skip_gather:
                    GA, GB = GA_static, GB_static
                else:
                    GA = gpool.tile([128, WPB * sA, 128], bf16, tag="GA")
                    GB = gpool.tile([128, WPB * sB, 128], bf16, tag="GB")
                    na = ntA * 128
                    nc.gpsimd.dma_gather(
                        out_ap=GA[:, 0:ntA, :], in_ap=tabA,
                        idxs_ap=idxa_sb[:, (cA0 * 128) // 16:
                                        ((cA0 + ntA) * 128) // 16],
                        num_idxs=na, num_idxs_reg=na, elem_size=128,
                        single_packet=False, queue_num=qi % nq_use,
                    )
                    qi += 1
                    nb_ = ntB * 128
                    nc.gpsimd.dma_gather(
                        out_ap=GB[:, 0:ntB, :], in_ap=tabB,
                        idxs_ap=idxb_sb[:, (cB0 * 128) // 16:
                                        ((cB0 + ntB) * 128) // 16],
                        num_idxs=nb_, num_idxs_reg=nb_, elem_size=128,
                        single_packet=False, queue_num=qi % nq_use,
                    )
                    qi += 1
                if dump == "g" and not skip_gather:
                    nc.sync.dma_start(
                        dump_d[:, cA0 * 128:(cA0 + ntA) * 128],
                        GA[:, 0:ntA, :])
                if skip_onehot:
                    OA, OB = OA_static, OB_static
                else:
                    OA = opool.tile([128, WPB * sA, 128], bf16, tag="OA")
                    OB = opool.tile([128, WPB * sB, 128], bf16, tag="OB")
                    nc.sync.dma_start(
                        OA[:, 0:ntA, :],
                        o01a_d[:, cA0 * 128:(cA0 + ntA) * 128])
                    nc.sync.dma_start(
                        OB[:, 0:ntB, :],
                        o01b_d[:, cB0 * 128:(cB0 + ntB) * 128])
                    # O_s = O01 * ex  (walrus rejects ALU ops on Pool, so
                    # both scales run on DVE)
                    nc.vector.tensor_tensor(
                        out=OA[:, 0:ntA, :], in0=OA[:, 0:ntA, :],
                        in1=apx(exa_sb[:, cA0:cA0 + ntA], [[1, ntA],
                                                           [0, 128]]),
                        op=OP.mult)
                    nc.vector.tensor_tensor(
                        out=OB[:, 0:ntB, :], in0=OB[:, 0:ntB, :],
                        in1=apx(exb_sb[:, cB0:cB0 + ntB], [[1, ntB],
                                                           [0, 128]]),
                        op=OP.mult)

                for wl in range(nw):
                    w = w0 + wl
                    psum_ud = pe_pool.tile([128, 132], f32, tag="psum_ud",
                                           bufs=1 if (den_sep or ud1) else 2)
                    if den_sep:
                        psum_dn = pe_pool.tile([128, 4], f32, tag="psum_dn",
                                               bufs=1)
                        den_ap = psum_dn[:, 0:1]
                    else:
                        den_ap = psum_ud[:, 128:129]
                    if not skip_mm:
                        def olh_grh(s_):
                            if s_ < sA:
                                return (OA[:, wl * sA + s_, :],
                                        GA[:, wl * sA + s_, :])
                            return (OB[:, wl * sB + (s_ - sA), :],
                                    GB[:, wl * sB + (s_ - sA), :])
                        if den_seq:
                            for s_ in range(SW):
                                Olh, Grh = olh_grh(s_)
                                nc.tensor.matmul(
                                    psum_ud[:, 0:128], lhsT=Olh, rhs=Grh,
                                    start=(s_ == 0), stop=(s_ == SW - 1))
                            for s_ in range(SW):
                                Olh, _ = olh_grh(s_)
                                nc.tensor.matmul(
                                    den_ap, lhsT=Olh, rhs=onesc_sb[:],
                                    start=(s_ == 0), stop=(s_ == SW - 1))
                        else:
                            for s_ in range(SW):
                                Olh, Grh = olh_grh(s_)
                                nc.tensor.matmul(
                                    psum_ud[:, 0:128], lhsT=Olh, rhs=Grh,
                                    start=(s_ == 0), stop=(s_ == SW - 1))
                                nc.tensor.matmul(
                                    den_ap, lhsT=Olh,
                                    rhs=onesc_sb[:],
                                    start=(s_ == 0), stop=(s_ == SW - 1))

                    if skip_node:
                        continue

                    # ---- bridge: ctx = psum/den -> bf16 -> [feat, node] ----
                    g = w // NB
                    gl = w % NB
                    nwin = min(NB, WPC - g * NB)
                    trdt = f32 if tr_f32 else bf16
                    if gl == 0:
                        psum_tr = ptr_pool.tile([128, NB * 128], trdt,
                                                tag="psum_tr")
                        cur_tr = psum_tr
                    den = bpool.tile([128, 1], f32, tag="den")
                    nc.vector.tensor_scalar(
                        out=den[:], in0=den_ap, scalar1=1e-30,
                        scalar2=None, op0=OP.max)
                    rec = bpool.tile([128, 1], f32, tag="rec")
                    nc.vector.reciprocal(rec[:], den[:])
                    ctx16 = bpool.tile([128, 128], trdt, tag="ctx16")
                    nc.vector.tensor_scalar(
                        out=ctx16[:], in0=psum_ud[:, 0:128],
                        scalar1=rec[:, 0:1], scalar2=None, op0=OP.mult)
                    if dump == "ctx":
                        cdump = bpool.tile([128, 128], f32, tag="cdump")
                        nc.vector.tensor_scalar(
                            out=cdump[:], in0=psum_ud[:, 0:128],
                            scalar1=rec[:, 0:1], scalar2=None, op0=OP.mult)
                        nc.sync.dma_start(
                            dump_d[w * 128:(w + 1) * 128, :], cdump[:])
                    elif dump == "den":
                        cdump = bpool.tile([128, 128], f32, tag="cdump")
                        nc.vector.tensor_scalar(
                            out=cdump[:, 0:1], in0=den_ap,
                            scalar1=1.0, scalar2=None, op0=OP.mult)
                        nc.vector.tensor_copy(out=cdump[:, 1:128],
                                              in_=psum_ud[:, 1:128])
                        nc.sync.dma_start(
                            dump_d[w * 128:(w + 1) * 128, :], cdump[:])
                    nc.tensor.transpose(
                        cur_tr[:, gl * 128:(gl + 1) * 128], ctx16[:],
                        ident_sb[:] if not tr_f32 else identf_sb[:])

                    if gl != nwin - 1:
                        continue

                    # ---- node phase for group g: windows [g*NB, g*NB+nwin)
                    C = nwin * 128
                    c0 = g * NB * 128
                    ctxT = bpool.tile([128, NB * 128], bf16, tag="ctxT")
                    nc.vector.tensor_copy(out=ctxT[:, 0:C],
                                          in_=cur_tr[:, 0:C])

                    # proj + ELU -> ctxT2 (bf16)
                    psum_cT = pn_pool.tile([128, NB * 128], f32,
                                           tag="psum_cT")
                    nc.tensor.matmul(psum_cT[:, 0:C], lhsT=wproj_sb[:],
                                     rhs=ctxT[:, 0:C], start=True, stop=True)
                    eA = wpool.tile([128, NB * 128], f32, tag="eA")
                    nc.vector.tensor_scalar(
                        out=eA[:, 0:C], in0=psum_cT[:, 0:C],
                        scalar1=bproj_sb[:, 0:1], scalar2=0.0,
                        op0=OP.add, op1=OP.min)
                    nc.scalar.activation(eA[:, 0:C], eA[:, 0:C], AF.Exp)
                    eB = wpool.tile([128, NB * 128], f32, tag="eB")
                    nc.vector.tensor_scalar(
                        out=eB[:, 0:C], in0=psum_cT[:, 0:C],
                        scalar1=bproj_sb[:, 0:1], scalar2=0.0,
                        op0=OP.add, op1=OP.max)
                    ctxT2 = wpool.tile([128, NB * 128], bf16, tag="ctxT2")
                    nc.vector.tensor_tensor(out=ctxT2[:, 0:C],
                                            in0=eA[:, 0:C], in1=eB[:, 0:C],
                                            op=OP.add)
                    nc.vector.tensor_scalar(
                        out=ctxT2[:, 0:C], in0=ctxT2[:, 0:C], scalar1=1.0,
                        scalar2=None, op0=OP.subtract)

                    # GRU gates, [gate, node] layout, weights stationary
                    nf16c = nfT16_sb[:, c0:c0 + C]
                    nf32c = nfT_sb[:, c0:c0 + C]
                    psum_r = pn_pool.tile([128, NB * 128], f32, tag="psum_r")
                    nc.tensor.matmul(psum_r[:, 0:C], lhsT=wih_sb[:, 0:128],
                                     rhs=ctxT2[:, 0:C], start=True,
                                     stop=False)
                    nc.tensor.matmul(psum_r[:, 0:C], lhsT=whh_sb[:, 0:128],
                                     rhs=nf16c, start=False, stop=True)
                    psum_z = pn_pool.tile([128, NB * 128], f32, tag="psum_z")
                    nc.tensor.matmul(psum_z[:, 0:C],
                                     lhsT=wih_sb[:, 128:256],
                                     rhs=ctxT2[:, 0:C], start=True,
                                     stop=False)
                    nc.tensor.matmul(psum_z[:, 0:C],
                                     lhsT=whh_sb[:, 128:256],
                                     rhs=nf16c, start=False, stop=True)
                    psum_n = pn_pool.tile([128, NB * 128], f32, tag="psum_n")
                    nc.tensor.matmul(psum_n[:, 0:C],
                                     lhsT=wih_sb[:, 256:384],
                                     rhs=ctxT2[:, 0:C], start=True, stop=True)
                    psum_h = pn_pool.tile([128, NB * 128], f32, tag="psum_h")
                    nc.tensor.matmul(psum_h[:, 0:C],
                                     lhsT=whh_sb[:, 256:384],
                                     rhs=nf16c, start=True, stop=True)

                    # r = sigmoid(s) = 0.5*tanh(0.5 s + 0.5 b_r) + 0.5
                    tr_ = wpool.tile([128, NB * 128], f32, tag="tr_")
                    nc.scalar.activation(tr_[:, 0:C], psum_r[:, 0:C],
                                         AF.Tanh, bias=brh_sb[:, 0:1],
                                         scale=0.5)
                    tz_ = wpool.tile([128, NB * 128], f32, tag="tz_")
                    nc.scalar.activation(tz_[:, 0:C], psum_z[:, 0:C],
                                         AF.Tanh, bias=bzh_sb[:, 0:1],
                                         scale=0.5)
                    # n = tanh(i_n + b_in + r*(h_n + b_hn))
                    #   r*(h_n+b_hn) = (tr+1)*hnb2, hnb2 = 0.5 h_n + 0.5 b_hn
                    hnb2 = wpool.tile([128, NB * 128], f32, tag="hnb2")
                    nc.vector.tensor_scalar(
                        out=hnb2[:, 0:C], in0=psum_h[:, 0:C], scalar1=0.5,
                        scalar2=bnhh_sb[:, 0:1], op0=OP.mult, op1=OP.add)
                    qq = wpool.tile([128, NB * 128], f32, tag="qq")
                    nc.vector.tensor_tensor(out=qq[:, 0:C], in0=tr_[:, 0:C],
                                            in1=hnb2[:, 0:C], op=OP.mult)
                    nc.vector.tensor_tensor(out=qq[:, 0:C], in0=qq[:, 0:C],
                                            in1=hnb2[:, 0:C], op=OP.add)
                    nc.vector.tensor_tensor(out=qq[:, 0:C], in0=qq[:, 0:C],
                                            in1=psum_n[:, 0:C], op=OP.add)
                    nn = wpool.tile([128, NB * 128], f32, tag="nn")
                    nc.scalar.activation(nn[:, 0:C], qq[:, 0:C], AF.Tanh,
                                         bias=bni_sb[:, 0:1])
                    # h = n + z*(nf - n);  z = 0.5 tz + 0.5
                    dd = wpool.tile([128, NB * 128], f32, tag="dd")
                    nc.vector.tensor_tensor(out=dd[:, 0:C], in0=nf32c,
                                            in1=nn[:, 0:C], op=OP.subtract)
                    nc.vector.tensor_scalar(
                        out=tz_[:, 0:C], in0=tz_[:, 0:C], scalar1=0.5,
                        scalar2=0.5, op0=OP.mult, op1=OP.add)
                    nc.vector.tensor_tensor(out=dd[:, 0:C], in0=tz_[:, 0:C],
                                            in1=dd[:, 0:C], op=OP.mult)
                    nc.vector.tensor_tensor(out=dd[:, 0:C], in0=dd[:, 0:C],
                                            in1=nn[:, 0:C], op=OP.add)
                    outt = qpool.tile([128, NB * 128], f32, tag="outt")
                    nc.scalar.activation(outt[:, 0:C], dd[:, 0:C], AF.Relu)
                    nc.sync.dma_start(out_d[:, c0:c0 + C], outt[:, 0:C])

    nc.compile()
    return nc


def _prep(edge_logits, node_feats, W_proj, b_proj, w_ih, w_hh, b_ih, b_hh,
          src, dst):
    """Host-side sharding. Returns (T_win, sA, sB, in_maps)."""
    import ml_dtypes
    BF16 = ml_dtypes.bfloat16

    logits = np.asarray(edge_logits, np.float32).reshape(-1)
    src = np.asarray(src, np.int64)
    dst = np.asarray(dst, np.int64)

    is_b = (src >= S_SPLIT).astype(np.int64)
    win = dst // 128
    key = win * 2 + is_b
    order = np.argsort(key, kind="stable")
    key_s = key[order]
    src_s = src[order]
    dst_s = dst[order]
    log_s = logits[order]

    counts = np.bincount(key_s, minlength=WTOT * 2)
    cA = counts[0::2]
    cB = counts[1::2]
    sA = int((cA.max() + 127) // 128)
    sB = int((cB.max() + 127) // 128)
    T_win = sA + sB

    starts = np.zeros(WTOT * 2, np.int64)
    starts[1:] = np.cumsum(counts)[:-1]
    pos = np.arange(E, dtype=np.int64) - starts[key_s]

    winv = key_s // 2
    grp = key_s % 2
    idxA = np.zeros(WTOT * sA * 128, np.int16)
    idxB = np.zeros(WTOT * sB * 128, np.int16)
    dstlA = np.full(WTOT * sA * 128, -1.0, np.float32)
    dstlB = np.full(WTOT * sB * 128, -1.0, np.float32)
    logA = np.zeros(WTOT * sA * 128, np.float32)
    logB = np.zeros(WTOT * sB * 128, np.float32)

    mA = grp == 0
    mB = ~mA
    flatA = winv[mA] * (sA * 128) + pos[mA]
    flatB = winv[mB] * (sB * 128) + pos[mB]
    idxA[flatA] = src_s[mA].astype(np.int16)
    idxB[flatB] = (src_s[mB] - OFF_B).astype(np.int16)
    dstlA[flatA] = (dst_s[mA] - winv[mA] * 128).astype(np.float32)
    dstlB[flatB] = (dst_s[mB] - winv[mB] * 128).astype(np.float32)
    logA[flatA] = log_s[mA]
    logB[flatB] = log_s[mB]

    def core_tiles(a, slots):
        a = a.reshape(WTOT, slots, 128)
        return [np.ascontiguousarray(
            a[k * WPC:(k + 1) * WPC].transpose(2, 0, 1)
            .reshape(128, WPC * slots)) for k in range(NC)]

    dstlA_cores = core_tiles(dstlA, sA)
    dstlB_cores = core_tiles(dstlB, sB)
    logA_cores = core_tiles(logA, sA)
    logB_cores = core_tiles(logB, sB)

    jj = np.arange(128, dtype=np.float32)

    def onehot_cores(dstl_cores, slots):
        out = []
        for d in dstl_cores:
            o = (d[:, :, None] == jj[None, None, :]).astype(BF16)
            out.append(np.ascontiguousarray(o.reshape(128, WPC * slots * 128)))
        return out

    o01A_cores = onehot_cores(dstlA_cores, sA)
    o01B_cores = onehot_cores(dstlB_cores, sB)

    def core_idx(a, slots):
        a = a.reshape(WTOT, slots * 128)
        out = []
        for k in range(NC):
            flat = a[k * WPC:(k + 1) * WPC].reshape(-1)
            blk = flat.reshape(-1, 16).T      # [16, L/16], i -> [i%16,i//16]
            out.append(np.ascontiguousarray(np.tile(blk, (8, 1))))
        return out

    idxA_cores = core_idx(idxA, sA)
    idxB_cores = core_idx(idxB, sB)

    nf = np.asarray(node_feats, np.float32)
    nf16 = nf.astype(BF16)
    nf_pad = np.zeros((NC * NPC, F), np.float32)
    nf_pad[:V] = nf

    tab16 = np.ascontiguousarray(nf16)
    tabb16 = np.ascontiguousarray(nf16[OFF_B:])
    wproj16 = np.ascontiguousarray(np.asarray(W_proj, np.float32).T
                                   .astype(BF16))
    wih16 = np.ascontiguousarray(np.asarray(w_ih, np.float32).T.astype(BF16))
    whh16 = np.ascontiguousarray(np.asarray(w_hh, np.float32).T.astype(BF16))
    bih = np.asarray(b_ih, np.float32).reshape(384)
    bhh = np.asarray(b_hh, np.float32).reshape(384)
    bprojc = np.asarray(b_proj, np.float32).reshape(128, 1)
    brh = (0.5 * (bih[0:128] + bhh[0:128])).reshape(128, 1)
    bzh = (0.5 * (bih[128:256] + bhh[128:256])).reshape(128, 1)
    bnic = bih[256:384].reshape(128, 1)
    bnhh = (0.5 * bhh[256:384]).reshape(128, 1)
    ident16 = np.eye(128, dtype=BF16)
    identf = np.eye(128, dtype=np.float32)
    onesc16 = np.ones((128, 1), BF16)

    in_maps = []
    for k in range(NC):
        sl = nf_pad[k * NPC:(k + 1) * NPC]
        nfT = np.ascontiguousarray(sl.T)
        nfT16 = np.ascontiguousarray(sl.T.astype(BF16))
        in_maps.append({
            "idxa": idxA_cores[k], "idxb": idxB_cores[k],
            "o01a": o01A_cores[k], "o01b": o01B_cores[k],
            "loga": logA_cores[k], "logb": logB_cores[k],
            "tab16": tab16, "tabb16": tabb16,
            "nfT": nfT, "nfT16": nfT16,
            "wproj16": wproj16, "wih16": wih16, "whh16": whh16,
            "bprojc": bprojc, "brh": brh, "bzh": bzh,
            "bnic": bnic, "bnhh": bnhh,
            "ident16": ident16, "identf": identf, "onesc16": onesc16,
        })
    return T_win, sA, sB, in_maps


def kernel(edge_logits, node_feats, W_proj, b_proj, w_ih, w_hh, b_ih, b_hh,
           src, dst):
    from concourse.bass_utils import run_bass_kernel_spmd

    T_win, sA, sB, in_maps = _prep(edge_logits, node_feats, W_proj, b_proj,
                                   w_ih, w_hh, b_ih, b_hh, src, dst)
    key = (T_win, sA, sB)
    if key not in _compiled:
        _compiled[key] = _build_nc(T_win, sA=sA, sB=sB)
    nc = _compiled[key]

    res = run_bass_kernel_spmd(nc, in_maps, list(range(NC)))
    full = np.concatenate(
        [np.ascontiguousarray(res.results[k]["out"]).T for k in range(NC)],
        axis=0)
    return np.ascontiguousarray(full[:V]).astype(np.float32)


# revision 34
# speedup vs baseline: 2.4177x; 1.4163x over previous
"""AttentiveGRU2 Trainium2 Bass kernel (v2).

Model (see reference):
  edge-softmax over incoming edges per dst node, attention-weighted
  gather of projected node features, segment-sum per dst, ELU, GRUCell.

Strategy (8 NeuronCores, SPMD, no collectives):
  * Host sorts edges by dst window (392 windows of 128 node ids; 49
    windows per core). Softmax folded through the segment sum:
    ctx_v = (sum_e ex_e nf[src_e]) / (sum_e ex_e); proj applied after.
  * Gather: node-feature table in bf16 (256B rows), hardware
    InstDMAGatherAnt across 4 SWDGE queues (the per-queue descriptor
    rate ~8ns/desc is the kernel bottleneck; 4 queues x bf16 measured
    ~4x faster than the fp32 single-queue baseline). int16 idx limit
    handled with two overlapping row views (A: src<32768, B: src-17232).
  * One-hot dst matrices are 0/1 bf16 built on host and streamed in via
    regular DMA (cheap sequential traffic) -- the only on-device
    elementwise edge work is O_s = O01 * ex (split DVE/Pool engines).
  * Edge matmuls per 128-edge slot tile (bf16, 1 cyc/row):
      psum[v,0:128] += O_s^T @ G_raw,  psum[v,128] += O_s^T @ ones.
  * Node phase in [channel, node] layout, weights stationary, batched
    over NB=4 windows: ctx scaled by 1/den -> bf16 -> PE transpose ->
    proj + ELU -> GRU gates. Sigmoid avoided via 0.5*tanh(x/2)+0.5 so
    every activation (Exp/Tanh/Relu) lives in one table: zero 1283ns
    act-table reloads. Biases folded into activation bias APs.
  * Output written [feat, node]; host transposes back.
"""

import numpy as np

V, E, F = 50000, 800000, 128
NC = 8
WPC = 49              # windows per core
NPC = WPC * 128       # 6272 node slots per core
WTOT = NC * WPC       # 392 windows total
WPB = 2               # windows per gather batch
NB = 4                # windows per node-phase group
S_SPLIT = 32768       # src < S -> table A
OFF_B = V - 32768     # 17232; table B rows [OFF_B, V)

_compiled = {}


def _build_nc(T_win, sA=None, sB=None, skip_gather=False, skip_onehot=False,
              skip_mm=False, skip_node=False, repeat=1, one_act=False,
              nq_use=4, den_sep=True, tr_f32=False, den_seq=False,
              dump=None, ud1=False):
    import concourse.bass as bass
    import concourse.bacc as bacc
    import concourse.mybir as mybir
    import concourse.tile as tile

    f32 = mybir.dt.float32
    bf16 = mybir.dt.bfloat16
    i16 = mybir.dt.int16
    AF = mybir.ActivationFunctionType
    OP = mybir.AluOpType

    SW = sA + sB            # slots per window
    LA = WPC * sA * 128     # A-gather idx count per core
    LB = WPC * sB * 128
    CA = WPC * sA           # A slot-tiles per core
    CB = WPC * sB

    nc = bacc.Bacc("TRN2", target_bir_lowering=False, debug=False,
                   num_devices=NC, num_swdge_queues=4)

    # ---- DRAM parameters ----
    idxa_d = nc.dram_tensor("idxa", [128, LA // 16], i16, kind="ExternalInput")
    idxb_d = nc.dram_tensor("idxb", [128, LB // 16], i16, kind="ExternalInput")
    o01a_d = nc.dram_tensor("o01a", [128, CA * 128], bf16,
                            kind="ExternalInput")
    o01b_d = nc.dram_tensor("o01b", [128, CB * 128], bf16,
                            kind="ExternalInput")
    loga_d = nc.dram_tensor("loga", [128, CA], f32, kind="ExternalInput")
    logb_d = nc.dram_tensor("logb", [128, CB], f32, kind="ExternalInput")
    tab16_d = nc.dram_tensor("tab16", [V, F], bf16, kind="ExternalInput")
    tabb16_d = nc.dram_tensor("tabb16", [32768, F], bf16,
                              kind="ExternalInput")
    nfT16_d = nc.dram_tensor("nfT16", [128, NPC], bf16, kind="ExternalInput")
    wproj16_d = nc.dram_tensor("wproj16", [128, 128], bf16,
                               kind="ExternalInput")
    wih16_d = nc.dram_tensor("wih16", [128, 384], bf16, kind="ExternalInput")
    whh16_d = nc.dram_tensor("whh16", [128, 384], bf16, kind="ExternalInput")
    bproj_d = nc.dram_tensor("bprojc", [128, 1], f32, kind="ExternalInput")
    brh_d = nc.dram_tensor("brh", [128, 1], f32, kind="ExternalInput")
    bzh_d = nc.dram_tensor("bzh", [128, 1], f32, kind="ExternalInput")
    bni_d = nc.dram_tensor("bnic", [128, 1], f32, kind="ExternalInput")
    bnhh_d = nc.dram_tensor("bnhh", [128, 1], f32, kind="ExternalInput")
    ident16_d = nc.dram_tensor("ident16", [128, 128], bf16,
                               kind="ExternalInput")
    identf_d = nc.dram_tensor("identf", [128, 128], f32,
                              kind="ExternalInput")
    onesc16_d = nc.dram_tensor("onesc16", [128, 1], bf16,
                               kind="ExternalInput")
    out_d = nc.dram_tensor("out", [128, NPC], f32, kind="ExternalOutput")
    if dump == "g":
        dump_d = nc.dram_tensor("dmp", [128, CA * 128], bf16,
                                kind="ExternalOutput")
    elif dump:
        dump_d = nc.dram_tensor("dmp", [NPC, 128], f32,
                                kind="ExternalOutput")

    tabA = tab16_d[0:32768, :]
    tabB = tabb16_d[:]

    def apx(base, dims):
        return bass.AP(base.tensor, base.offset,
                       [list(base.ap[0])] + dims)

    with tile.TileContext(nc) as tc:
        with (
            tc.tile_pool(name="const", bufs=1) as cpool,
            tc.tile_pool(name="gat", bufs=4) as gpool,
            tc.tile_pool(name="oh", bufs=4) as opool,
            tc.tile_pool(name="wrk", bufs=1) as wpool,
            tc.tile_pool(name="brdg", bufs=2) as bpool,
            tc.tile_pool(name="outp", bufs=2) as qpool,
            tc.tile_pool(name="pedge", bufs=1, space="PSUM") as pe_pool,
            tc.tile_pool(name="ptr", bufs=1, space="PSUM") as ptr_pool,
            tc.tile_pool(name="pnode", bufs=1, space="PSUM") as pn_pool,
        ):
            def load(pool, name, dram, shape, dtype=f32):
                t = pool.tile(shape, dtype, tag=name)
                nc.sync.dma_start(t[:], dram[:])
                return t

            ident_sb = load(cpool, "ident16", ident16_d, [128, 128], bf16)
            identf_sb = load(cpool, "identf", identf_d, [128, 128], f32)
            onesc_sb = load(cpool, "onesc16", onesc16_d, [128, 1], bf16)
            wproj_sb = load(cpool, "wproj16", wproj16_d, [128, 128], bf16)
            wih_sb = load(cpool, "wih16", wih16_d, [128, 384], bf16)
            whh_sb = load(cpool, "whh16", whh16_d, [128, 384], bf16)
            bproj_sb = load(cpool, "bprojc", bproj_d, [128, 1])
            brh_sb = load(cpool, "brh", brh_d, [128, 1])
            bzh_sb = load(cpool, "bzh", bzh_d, [128, 1])
            bni_sb = load(cpool, "bnic", bni_d, [128, 1])
            bnhh_sb = load(cpool, "bnhh", bnhh_d, [128, 1])
            idxa_sb = load(cpool, "idxa", idxa_d, [128, LA // 16], i16)
            idxb_sb = load(cpool, "idxb", idxb_d, [128, LB // 16], i16)
            nfT16_sb = load(cpool, "nfT16", nfT16_d, [128, NPC], bf16)

            # ex = exp(logits), bf16 (softmax shift-invariance: no seg-max;
            # logits are N(0,1) so fp32 exp is safe)
            loga_sb = load(cpool, "loga", loga_d, [128, CA])
            exa_sb = cpool.tile([128, CA], bf16, tag="exa")
            nc.scalar.activation(exa_sb[:], loga_sb[:], AF.Exp)
            logb_sb = load(cpool, "logb", logb_d, [128, CB])
            exb_sb = cpool.tile([128, CB], bf16, tag="exb")
            nc.scalar.activation(exb_sb[:], logb_sb[:], AF.Exp)

            GA_static = GB_static = None
            if skip_gather:
                GA_static = cpool.tile([128, WPB * sA, 128], bf16, tag="GAs")
                nc.gpsimd.memset(GA_static[:], 0.0)
                GB_static = cpool.tile([128, WPB * sB, 128], bf16, tag="GBs")
                nc.gpsimd.memset(GB_static[:], 0.0)
            OA_static = OB_static = None
            if skip_onehot:
                OA_static = cpool.tile([128, WPB * sA, 128], bf16, tag="OAs")
                nc.gpsimd.memset(OA_static[:], 0.0)
                OB_static = cpool.tile([128, WPB * sB, 128], bf16, tag="OBs")
                nc.gpsimd.memset(OB_static[:], 0.0)

            n_batches = (WPC + WPB - 1) // WPB
            qload = [0] * nq_use   # greedy per-queue descriptor balancing
            for _rep in range(repeat):
              # node-group state: transpose psum + sbuf ctxT for NB windows
              for b in range(n_batches):
                w0 = b * WPB
                nw = min(WPB, WPC - w0)
                ntA, ntB = nw * sA, nw * sB
                cA0, cB0 = w0 * sA, w0 * sB
                if skip_gather:
                    GA, GB = GA_static, GB_static
                else:
                    GA = gpool.tile([128, WPB * sA, 128], bf16, tag="GA")
                    GB = gpool.tile([128, WPB * sB, 128], bf16, tag="GB")
                    na = ntA * 128
                    qn = qload.index(min(qload))
                    qload[qn] += na
                    nc.gpsimd.dma_gather(
                        out_ap=GA[:, 0:ntA, :], in_ap=tabA,
                        idxs_ap=idxa_sb[:, (cA0 * 128) // 16:
                                        ((cA0 + ntA) * 128) // 16],
                        num_idxs=na, num_idxs_reg=na, elem_size=128,
                        single_packet=False, queue_num=qn,
                    )
                    nb_ = ntB * 128
                    qn = qload.index(min(qload))
                    qload[qn] += nb_
                    nc.gpsimd.dma_gather(
                        out_ap=GB[:, 0:ntB, :], in_ap=tabB,
                        idxs_ap=idxb_sb[:, (cB0 * 128) // 16:
                                        ((cB0 + ntB) * 128) // 16],
                        num_idxs=nb_, num_idxs_reg=nb_, elem_size=128,
                        single_packet=False, queue_num=qn,
                    )
                if dump == "g" and not skip_gather:
                    nc.sync.dma_start(
                        dump_d[:, cA0 * 128:(cA0 + ntA) * 128],
                        GA[:, 0:ntA, :])
                if skip_onehot:
                    OA, OB = OA_static, OB_static
                else:
                    OA = opool.tile([128, WPB * sA, 128], bf16, tag="OA")
                    OB = opool.tile([128, WPB * sB, 128], bf16, tag="OB")
                    # O01 streams ride the Activation engine's HWDGE queue
                    # so they don't queue behind outputs/consts on SP
                    nc.scalar.dma_start(
                        OA[:, 0:ntA, :],
                        o01a_d[:, cA0 * 128:(cA0 + ntA) * 128])
                    nc.scalar.dma_start(
                        OB[:, 0:ntB, :],
                        o01b_d[:, cB0 * 128:(cB0 + ntB) * 128])
                    # O_s = O01 * ex  (walrus rejects ALU ops on Pool, so
                    # both scales run on DVE)
                    nc.vector.tensor_tensor(
                        out=OA[:, 0:ntA, :], in0=OA[:, 0:ntA, :],
                        in1=apx(exa_sb[:, cA0:cA0 + ntA], [[1, ntA],
                                                           [0, 128]]),
                        op=OP.mult)
                    nc.vector.tensor_tensor(
                        out=OB[:, 0:ntB, :], in0=OB[:, 0:ntB, :],
                        in1=apx(exb_sb[:, cB0:cB0 + ntB], [[1, ntB],
                                                           [0, 128]]),
                        op=OP.mult)

                for wl in range(nw):
                    w = w0 + wl
                    psum_ud = pe_pool.tile([128, 132], f32, tag="psum_ud",
                                           bufs=1 if ud1 else 2)
                    if den_sep:
                        # den accumulates in its own PSUM bank: a second
                        # concurrently-open matmul group in the same 2KB
                        # zero region corrupts the first (hw start_tensor_
                        # calc marks the whole region pending-zero).
                        psum_dn = pe_pool.tile([128, 4], f32, tag="psum_dn",
                                               bufs=2)
                        den_ap = psum_dn[:, 0:1]
                    else:
                        den_ap = psum_ud[:, 128:129]
                    if not skip_mm:
                        def olh_grh(s_):
                            if s_ < sA:
                                return (OA[:, wl * sA + s_, :],
                                        GA[:, wl * sA + s_, :])
                            return (OB[:, wl * sB + (s_ - sA), :],
                                    GB[:, wl * sB + (s_ - sA), :])
                        if den_seq:
                            for s_ in range(SW):
                                Olh, Grh = olh_grh(s_)
                                nc.tensor.matmul(
                                    psum_ud[:, 0:128], lhsT=Olh, rhs=Grh,
                                    start=(s_ == 0), stop=(s_ == SW - 1))
                            for s_ in range(SW):
                                Olh, _ = olh_grh(s_)
                                nc.tensor.matmul(
                                    den_ap, lhsT=Olh, rhs=onesc_sb[:],
                                    start=(s_ == 0), stop=(s_ == SW - 1))
                        else:
                            for s_ in range(SW):
                                Olh, Grh = olh_grh(s_)
                                nc.tensor.matmul(
                                    psum_ud[:, 0:128], lhsT=Olh, rhs=Grh,
                                    start=(s_ == 0), stop=(s_ == SW - 1))
                                nc.tensor.matmul(
                                    den_ap, lhsT=Olh,
                                    rhs=onesc_sb[:],
                                    start=(s_ == 0), stop=(s_ == SW - 1))

                    if skip_node:
                        continue

                    # ---- bridge: ctx = psum/den -> bf16 -> [feat, node] ----
                    g = w // NB
                    gl = w % NB
                    nwin = min(NB, WPC - g * NB)
                    trdt = f32 if tr_f32 else bf16
                    if gl == 0:
                        psum_tr = ptr_pool.tile([128, NB * 128], trdt,
                                                tag="psum_tr")
                        cur_tr = psum_tr
                    den = bpool.tile([128, 1], f32, tag="den")
                    nc.vector.tensor_scalar(
                        out=den[:], in0=den_ap, scalar1=1e-30,
                        scalar2=None, op0=OP.max)
                    rec = bpool.tile([128, 1], f32, tag="rec")
                    nc.vector.reciprocal(rec[:], den[:])
                    ctx16 = bpool.tile([128, 128], trdt, tag="ctx16")
                    nc.vector.tensor_scalar(
                        out=ctx16[:], in0=psum_ud[:, 0:128],
                        scalar1=rec[:, 0:1], scalar2=None, op0=OP.mult)
                    if dump == "ctx":
                        cdump = bpool.tile([128, 128], f32, tag="cdump")
                        nc.vector.tensor_scalar(
                            out=cdump[:], in0=psum_ud[:, 0:128],
                            scalar1=rec[:, 0:1], scalar2=None, op0=OP.mult)
                        nc.sync.dma_start(
                            dump_d[w * 128:(w + 1) * 128, :], cdump[:])
                    elif dump == "den":
                        cdump = bpool.tile([128, 128], f32, tag="cdump")
                        nc.vector.tensor_scalar(
                            out=cdump[:, 0:1], in0=den_ap,
                            scalar1=1.0, scalar2=None, op0=OP.mult)
                        nc.vector.tensor_copy(out=cdump[:, 1:128],
                                              in_=psum_ud[:, 1:128])
                        nc.sync.dma_start(
                            dump_d[w * 128:(w + 1) * 128, :], cdump[:])
                    nc.tensor.transpose(
                        cur_tr[:, gl * 128:(gl + 1) * 128], ctx16[:],
                        ident_sb[:] if not tr_f32 else identf_sb[:])

                    if gl != nwin - 1:
                        continue

                    # ---- node phase for group g: windows [g*NB, g*NB+nwin)
                    C = nwin * 128
                    c0 = g * NB * 128
                    ctxT = bpool.tile([128, NB * 128], bf16, tag="ctxT")
                    nc.vector.tensor_copy(out=ctxT[:, 0:C],
                                          in_=cur_tr[:, 0:C])

                    # proj + ELU -> ctxT2 (bf16)
                    psum_cT = pn_pool.tile([128, NB * 128], f32,
                                           tag="psum_cT")
                    nc.tensor.matmul(psum_cT[:, 0:C], lhsT=wproj_sb[:],
                                     rhs=ctxT[:, 0:C], start=True, stop=True)
                    eA = wpool.tile([128, NB * 128], f32, tag="eA")
                    nc.vector.tensor_scalar(
                        out=eA[:, 0:C], in0=psum_cT[:, 0:C],
                        scalar1=bproj_sb[:, 0:1], scalar2=0.0,
                        op0=OP.add, op1=OP.min)
                    nc.scalar.activation(eA[:, 0:C], eA[:, 0:C], AF.Exp)
                    eB = wpool.tile([128, NB * 128], f32, tag="eB")
                    nc.vector.tensor_scalar(
                        out=eB[:, 0:C], in0=psum_cT[:, 0:C],
                        scalar1=bproj_sb[:, 0:1], scalar2=0.0,
                        op0=OP.add, op1=OP.max)
                    # elu = (eA - 1) + eB, fused on DVE
                    ctxT2 = wpool.tile([128, NB * 128], bf16, tag="ctxT2")
                    nc.vector.scalar_tensor_tensor(
                        out=ctxT2[:, 0:C], in0=eA[:, 0:C], scalar=1.0,
                        in1=eB[:, 0:C], op0=OP.subtract, op1=OP.add)

                    # GRU gates, [gate, node] layout, weights stationary.
                    # r/n share a PSUM bank sequentially, h/z share another
                    # (group lifetimes don't overlap; WAR via tile reuse).
                    nf16c = nfT16_sb[:, c0:c0 + C]
                    psum_rn = pn_pool.tile([128, NB * 128], f32,
                                           tag="psum_rn")
                    nc.tensor.matmul(psum_rn[:, 0:C], lhsT=wih_sb[:, 0:128],
                                     rhs=ctxT2[:, 0:C], start=True,
                                     stop=False)
                    nc.tensor.matmul(psum_rn[:, 0:C], lhsT=whh_sb[:, 0:128],
                                     rhs=nf16c, start=False, stop=True)
                    psum_hz = pn_pool.tile([128, NB * 128], f32,
                                           tag="psum_hz")
                    nc.tensor.matmul(psum_hz[:, 0:C],
                                     lhsT=whh_sb[:, 256:384],
                                     rhs=nf16c, start=True, stop=True)

                    # r = sigmoid(s) = 0.5*tanh(0.5 s + 0.5 b_r) + 0.5
                    tr_ = wpool.tile([128, NB * 128], f32, tag="tr_")
                    nc.scalar.activation(tr_[:, 0:C], psum_rn[:, 0:C],
                                         AF.Tanh, bias=brh_sb[:, 0:1],
                                         scale=0.5)
                    # hnb2 = 0.5 h_n + 0.5 b_hn
                    hnb2 = wpool.tile([128, NB * 128], f32, tag="hnb2")
                    nc.vector.tensor_scalar(
                        out=hnb2[:, 0:C], in0=psum_hz[:, 0:C], scalar1=0.5,
                        scalar2=bnhh_sb[:, 0:1], op0=OP.mult, op1=OP.add)

                    # n-gate reuses the r bank; z reuses the h bank
                    nc.tensor.matmul(psum_rn[:, 0:C],
                                     lhsT=wih_sb[:, 256:384],
                                     rhs=ctxT2[:, 0:C], start=True, stop=True)
                    nc.tensor.matmul(psum_hz[:, 0:C],
                                     lhsT=wih_sb[:, 128:256],
                                     rhs=ctxT2[:, 0:C], start=True,
                                     stop=False)
                    nc.tensor.matmul(psum_hz[:, 0:C],
                                     lhsT=whh_sb[:, 128:256],
                                     rhs=nf16c, start=False, stop=True)

                    # n = tanh(i_n + b_in + (tr+1)*hnb2)
                    qq = wpool.tile([128, NB * 128], f32, tag="qq")
                    nc.vector.scalar_tensor_tensor(
                        out=qq[:, 0:C], in0=tr_[:, 0:C], scalar=1.0,
                        in1=hnb2[:, 0:C], op0=OP.add, op1=OP.mult)
                    nc.vector.tensor_tensor(out=qq[:, 0:C], in0=qq[:, 0:C],
                                            in1=psum_rn[:, 0:C], op=OP.add)
                    nn = wpool.tile([128, NB * 128], f32, tag="nn")
                    nc.scalar.activation(nn[:, 0:C], qq[:, 0:C], AF.Tanh,
                                         bias=bni_sb[:, 0:1])
                    tz_ = wpool.tile([128, NB * 128], f32, tag="tz_")
                    nc.scalar.activation(tz_[:, 0:C], psum_hz[:, 0:C],
                                         AF.Tanh, bias=bzh_sb[:, 0:1],
                                         scale=0.5)
                    # h = n + (0.5 tz + 0.5)*(nf - n)
                    dd = wpool.tile([128, NB * 128], f32, tag="dd")
                    nc.vector.tensor_tensor(out=dd[:, 0:C], in0=nf16c,
                                            in1=nn[:, 0:C], op=OP.subtract)
                    uu = wpool.tile([128, NB * 128], f32, tag="uu")
                    nc.vector.scalar_tensor_tensor(
                        out=uu[:, 0:C], in0=tz_[:, 0:C], scalar=1.0,
                        in1=dd[:, 0:C], op0=OP.add, op1=OP.mult)
                    nc.vector.scalar_tensor_tensor(
                        out=dd[:, 0:C], in0=uu[:, 0:C], scalar=0.5,
                        in1=nn[:, 0:C], op0=OP.mult, op1=OP.add)
                    outt = qpool.tile([128, NB * 128], f32, tag="outt")
                    nc.scalar.activation(outt[:, 0:C], dd[:, 0:C], AF.Relu)
                    nc.sync.dma_start(out_d[:, c0:c0 + C], outt[:, 0:C])

    nc.compile()
    return nc


def _prep(edge_logits, node_feats, W_proj, b_proj, w_ih, w_hh, b_ih, b_hh,
          src, dst):
    """Host-side sharding. Returns (T_win, sA, sB, in_maps)."""
    import ml_dtypes
    BF16 = ml_dtypes.bfloat16

    logits = np.asarray(edge_logits, np.float32).reshape(-1)
    src = np.asarray(src, np.int64)
    dst = np.asarray(dst, np.int64)

    is_b = (src >= S_SPLIT).astype(np.int64)
    win = dst // 128
    key = win * 2 + is_b
    order = np.argsort(key, kind="stable")
    key_s = key[order]
    src_s = src[order]
    dst_s = dst[order]
    log_s = logits[order]

    counts = np.bincount(key_s, minlength=WTOT * 2)
    cA = counts[0::2]
    cB = counts[1::2]
    sA = int((cA.max() + 127) // 128)
    sB = int((cB.max() + 127) // 128)
    T_win = sA + sB

    starts = np.zeros(WTOT * 2, np.int64)
    starts[1:] = np.cumsum(counts)[:-1]
    pos = np.arange(E, dtype=np.int64) - starts[key_s]

    winv = key_s // 2
    grp = key_s % 2
    idxA = np.zeros(WTOT * sA * 128, np.int16)
    idxB = np.zeros(WTOT * sB * 128, np.int16)
    dstlA = np.full(WTOT * sA * 128, -1.0, np.float32)
    dstlB = np.full(WTOT * sB * 128, -1.0, np.float32)
    logA = np.zeros(WTOT * sA * 128, np.float32)
    logB = np.zeros(WTOT * sB * 128, np.float32)

    mA = grp == 0
    mB = ~mA
    flatA = winv[mA] * (sA * 128) + pos[mA]
    flatB = winv[mB] * (sB * 128) + pos[mB]
    idxA[flatA] = src_s[mA].astype(np.int16)
    idxB[flatB] = (src_s[mB] - OFF_B).astype(np.int16)
    dstlA[flatA] = (dst_s[mA] - winv[mA] * 128).astype(np.float32)
    dstlB[flatB] = (dst_s[mB] - winv[mB] * 128).astype(np.float32)
    logA[flatA] = log_s[mA]
    logB[flatB] = log_s[mB]

    def core_tiles(a, slots):
        a = a.reshape(WTOT, slots, 128)
        return [np.ascontiguousarray(
            a[k * WPC:(k + 1) * WPC].transpose(2, 0, 1)
            .reshape(128, WPC * slots)) for k in range(NC)]

    dstlA_cores = core_tiles(dstlA, sA)
    dstlB_cores = core_tiles(dstlB, sB)
    logA_cores = core_tiles(logA, sA)
    logB_cores = core_tiles(logB, sB)

    jj = np.arange(128, dtype=np.float32)

    def onehot_cores(dstl_cores, slots):
        out = []
        for d in dstl_cores:
            o = (d[:, :, None] == jj[None, None, :]).astype(BF16)
            out.append(np.ascontiguousarray(o.reshape(128, WPC * slots * 128)))
        return out

    o01A_cores = onehot_cores(dstlA_cores, sA)
    o01B_cores = onehot_cores(dstlB_cores, sB)

    def core_idx(a, slots):
        a = a.reshape(WTOT, slots * 128)
        out = []
        for k in range(NC):
            flat = a[k * WPC:(k + 1) * WPC].reshape(-1)
            blk = flat.reshape(-1, 16).T      # [16, L/16], i -> [i%16,i//16]
            out.append(np.ascontiguousarray(np.tile(blk, (8, 1))))
        return out

    idxA_cores = core_idx(idxA, sA)
    idxB_cores = core_idx(idxB, sB)

    nf = np.asarray(node_feats, np.float32)
    nf16 = nf.astype(BF16)
    nf_pad = np.zeros((NC * NPC, F), np.float32)
    nf_pad[:V] = nf

    tab16 = np.ascontiguousarray(nf16)
    tabb16 = np.ascontiguousarray(nf16[OFF_B:])
    wproj16 = np.ascontiguousarray(np.asarray(W_proj, np.float32).T
                                   .astype(BF16))
    wih16 = np.ascontiguousarray(np.asarray(w_ih, np.float32).T.astype(BF16))
    whh16 = np.ascontiguousarray(np.asarray(w_hh, np.float32).T.astype(BF16))
    bih = np.asarray(b_ih, np.float32).reshape(384)
    bhh = np.asarray(b_hh, np.float32).reshape(384)
    bprojc = np.asarray(b_proj, np.float32).reshape(128, 1)
    brh = (0.5 * (bih[0:128] + bhh[0:128])).reshape(128, 1)
    bzh = (0.5 * (bih[128:256] + bhh[128:256])).reshape(128, 1)
    bnic = bih[256:384].reshape(128, 1)
    bnhh = (0.5 * bhh[256:384]).reshape(128, 1)
    ident16 = np.eye(128, dtype=BF16)
    identf = np.eye(128, dtype=np.float32)
    onesc16 = np.ones((128, 1), BF16)

    in_maps = []
    for k in range(NC):
        sl = nf_pad[k * NPC:(k + 1) * NPC]
        nfT = np.ascontiguousarray(sl.T)
        nfT16 = np.ascontiguousarray(sl.T.astype(BF16))
        in_maps.append({
            "idxa": idxA_cores[k], "idxb": idxB_cores[k],
            "o01a": o01A_cores[k], "o01b": o01B_cores[k],
            "loga": logA_cores[k], "logb": logB_cores[k],
            "tab16": tab16, "tabb16": tabb16,
            "nfT16": nfT16,
            "wproj16": wproj16, "wih16": wih16, "whh16": whh16,
            "bprojc": bprojc, "brh": brh, "bzh": bzh,
            "bnic": bnic, "bnhh": bnhh,
            "ident16": ident16, "identf": identf, "onesc16": onesc16,
        })
    return T_win, sA, sB, in_maps


def kernel(edge_logits, node_feats, W_proj, b_proj, w_ih, w_hh, b_ih, b_hh,
           src, dst):
    from concourse.bass_utils import run_bass_kernel_spmd

    T_win, sA, sB, in_maps = _prep(edge_logits, node_feats, W_proj, b_proj,
                                   w_ih, w_hh, b_ih, b_hh, src, dst)
    key = (T_win, sA, sB)
    if key not in _compiled:
        _compiled[key] = _build_nc(T_win, sA=sA, sB=sB)
    nc = _compiled[key]

    res = run_bass_kernel_spmd(nc, in_maps, list(range(NC)))
    full = np.concatenate(
        [np.ascontiguousarray(res.results[k]["out"]).T for k in range(NC)],
        axis=0)
    return np.ascontiguousarray(full[:V]).astype(np.float32)


# revision 39
# speedup vs baseline: 2.7017x; 1.1175x over previous
"""AttentiveGRU2 Trainium2 Bass kernel (v2).

Model (see reference):
  edge-softmax over incoming edges per dst node, attention-weighted
  gather of projected node features, segment-sum per dst, ELU, GRUCell.

Strategy (8 NeuronCores, SPMD, no collectives):
  * Host sorts edges by dst window (392 windows of 128 node ids; 49
    windows per core). Softmax folded through the segment sum:
    ctx_v = (sum_e ex_e nf[src_e]) / (sum_e ex_e); proj applied after.
  * Gather: node-feature table in bf16 (256B rows), hardware
    InstDMAGatherAnt across 4 SWDGE queues (the per-queue descriptor
    rate ~8ns/desc is the kernel bottleneck; 4 queues x bf16 measured
    ~4x faster than the fp32 single-queue baseline). int16 idx limit
    handled with two overlapping row views (A: src<32768, B: src-17232).
  * One-hot dst matrices are 0/1 bf16 built on host and streamed in via
    regular DMA (cheap sequential traffic) -- the only on-device
    elementwise edge work is O_s = O01 * ex (split DVE/Pool engines).
  * Edge matmuls per 128-edge slot tile (bf16, 1 cyc/row):
      psum[v,0:128] += O_s^T @ G_raw,  psum[v,128] += O_s^T @ ones.
  * Node phase in [channel, node] layout, weights stationary, batched
    over NB=4 windows: ctx scaled by 1/den -> bf16 -> PE transpose ->
    proj + ELU -> GRU gates. Sigmoid avoided via 0.5*tanh(x/2)+0.5 so
    every activation (Exp/Tanh/Relu) lives in one table: zero 1283ns
    act-table reloads. Biases folded into activation bias APs.
  * Output written [feat, node]; host transposes back.
"""

import numpy as np

V, E, F = 50000, 800000, 128
NC = 8
WPC = 49              # windows per core
NPC = WPC * 128       # 6272 node slots per core
WTOT = NC * WPC       # 392 windows total
WPB = 2               # windows per gather batch
NB = 4                # windows per node-phase group
S_SPLIT = 32768       # src < S -> table A
OFF_B = V - 32768     # 17232; table B rows [OFF_B, V)

_compiled = {}


def _build_nc(T_win, sA=None, sB=None, skip_gather=False, skip_onehot=False,
              skip_mm=False, skip_node=False, repeat=1, one_act=False,
              nq_use=4, den_sep=True, tr_f32=False, den_seq=False,
              dump=None, ud1=False):
    import concourse.bass as bass
    import concourse.bacc as bacc
    import concourse.mybir as mybir
    import concourse.tile as tile

    f32 = mybir.dt.float32
    bf16 = mybir.dt.bfloat16
    i16 = mybir.dt.int16
    AF = mybir.ActivationFunctionType
    OP = mybir.AluOpType

    # sA/sB: per-position slot-count lists (scalars = uniform legacy)
    sAp = list(sA) if not isinstance(sA, int) else [sA] * WPC
    sBp = list(sB) if not isinstance(sB, int) else [sB] * WPC
    ofsA = np.concatenate([[0], np.cumsum(sAp)]).astype(int)
    ofsB = np.concatenate([[0], np.cumsum(sBp)]).astype(int)
    CA = int(ofsA[-1])      # A slot-tiles per core
    CB = int(ofsB[-1])
    LA = CA * 128           # A-gather idx count per core
    LB = CB * 128
    n_batches = (WPC + WPB - 1) // WPB
    mxA = max(int(ofsA[min(b * WPB + WPB, WPC)] - ofsA[b * WPB])
              for b in range(n_batches))
    mxB = max(int(ofsB[min(b * WPB + WPB, WPC)] - ofsB[b * WPB])
              for b in range(n_batches))

    nc = bacc.Bacc("TRN2", target_bir_lowering=False, debug=False,
                   num_devices=NC, num_swdge_queues=4)

    # ---- DRAM parameters ----
    idxa_d = nc.dram_tensor("idxa", [128, LA // 16], i16, kind="ExternalInput")
    idxb_d = nc.dram_tensor("idxb", [128, LB // 16], i16, kind="ExternalInput")
    o01a_d = nc.dram_tensor("o01a", [128, CA * 128], bf16,
                            kind="ExternalInput")
    o01b_d = nc.dram_tensor("o01b", [128, CB * 128], bf16,
                            kind="ExternalInput")
    loga_d = nc.dram_tensor("loga", [128, CA], f32, kind="ExternalInput")
    logb_d = nc.dram_tensor("logb", [128, CB], f32, kind="ExternalInput")
    tab16_d = nc.dram_tensor("tab16", [V, F], bf16, kind="ExternalInput")
    tabb16_d = nc.dram_tensor("tabb16", [32768, F], bf16,
                              kind="ExternalInput")
    nfT16_d = nc.dram_tensor("nfT16", [128, NPC], bf16, kind="ExternalInput")
    wproj16_d = nc.dram_tensor("wproj16", [128, 128], bf16,
                               kind="ExternalInput")
    wih16_d = nc.dram_tensor("wih16", [128, 384], bf16, kind="ExternalInput")
    whh16_d = nc.dram_tensor("whh16", [128, 384], bf16, kind="ExternalInput")
    bproj_d = nc.dram_tensor("bprojc", [128, 1], f32, kind="ExternalInput")
    brh_d = nc.dram_tensor("brh", [128, 1], f32, kind="ExternalInput")
    bzh_d = nc.dram_tensor("bzh", [128, 1], f32, kind="ExternalInput")
    bni_d = nc.dram_tensor("bnic", [128, 1], f32, kind="ExternalInput")
    bnhh_d = nc.dram_tensor("bnhh", [128, 1], f32, kind="ExternalInput")
    ident16_d = nc.dram_tensor("ident16", [128, 128], bf16,
                               kind="ExternalInput")
    identf_d = nc.dram_tensor("identf", [128, 128], f32,
                              kind="ExternalInput")
    onesc16_d = nc.dram_tensor("onesc16", [128, 1], bf16,
                               kind="ExternalInput")
    out_d = nc.dram_tensor("out", [128, NPC], f32, kind="ExternalOutput")
    if dump == "g":
        dump_d = nc.dram_tensor("dmp", [128, CA * 128], bf16,
                                kind="ExternalOutput")
    elif dump:
        dump_d = nc.dram_tensor("dmp", [NPC, 128], f32,
                                kind="ExternalOutput")

    tabA = tab16_d[0:32768, :]
    tabB = tabb16_d[:]

    def apx(base, dims):
        return bass.AP(base.tensor, base.offset,
                       [list(base.ap[0])] + dims)

    with tile.TileContext(nc) as tc:
        with (
            tc.tile_pool(name="const", bufs=1) as cpool,
            tc.tile_pool(name="gat", bufs=4) as gpool,
            tc.tile_pool(name="oh", bufs=4) as opool,
            tc.tile_pool(name="wrk", bufs=1) as wpool,
            tc.tile_pool(name="brdg", bufs=2) as bpool,
            tc.tile_pool(name="outp", bufs=2) as qpool,
            tc.tile_pool(name="pedge", bufs=1, space="PSUM") as pe_pool,
            tc.tile_pool(name="ptr", bufs=1, space="PSUM") as ptr_pool,
            tc.tile_pool(name="pnode", bufs=1, space="PSUM") as pn_pool,
        ):
            def load(pool, name, dram, shape, dtype=f32):
                t = pool.tile(shape, dtype, tag=name)
                nc.sync.dma_start(t[:], dram[:])
                return t

            ident_sb = load(cpool, "ident16", ident16_d, [128, 128], bf16)
            identf_sb = load(cpool, "identf", identf_d, [128, 128], f32)
            onesc_sb = load(cpool, "onesc16", onesc16_d, [128, 1], bf16)
            wproj_sb = load(cpool, "wproj16", wproj16_d, [128, 128], bf16)
            wih_sb = load(cpool, "wih16", wih16_d, [128, 384], bf16)
            whh_sb = load(cpool, "whh16", whh16_d, [128, 384], bf16)
            bproj_sb = load(cpool, "bprojc", bproj_d, [128, 1])
            brh_sb = load(cpool, "brh", brh_d, [128, 1])
            bzh_sb = load(cpool, "bzh", bzh_d, [128, 1])
            bni_sb = load(cpool, "bnic", bni_d, [128, 1])
            bnhh_sb = load(cpool, "bnhh", bnhh_d, [128, 1])
            idxa_sb = load(cpool, "idxa", idxa_d, [128, LA // 16], i16)
            idxb_sb = load(cpool, "idxb", idxb_d, [128, LB // 16], i16)
            nfT16_sb = load(cpool, "nfT16", nfT16_d, [128, NPC], bf16)

            # ex = exp(logits), bf16 (softmax shift-invariance: no seg-max;
            # logits are N(0,1) so fp32 exp is safe)
            loga_sb = load(cpool, "loga", loga_d, [128, CA])
            exa_sb = cpool.tile([128, CA], bf16, tag="exa")
            nc.scalar.activation(exa_sb[:], loga_sb[:], AF.Exp)
            logb_sb = load(cpool, "logb", logb_d, [128, CB])
            exb_sb = cpool.tile([128, CB], bf16, tag="exb")
            nc.scalar.activation(exb_sb[:], logb_sb[:], AF.Exp)

            GA_static = GB_static = None
            if skip_gather:
                GA_static = cpool.tile([128, mxA, 128], bf16, tag="GAs")
                nc.gpsimd.memset(GA_static[:], 0.0)
                GB_static = cpool.tile([128, mxB, 128], bf16, tag="GBs")
                nc.gpsimd.memset(GB_static[:], 0.0)
            OA_static = OB_static = None
            if skip_onehot:
                OA_static = cpool.tile([128, mxA, 128], bf16, tag="OAs")
                nc.gpsimd.memset(OA_static[:], 0.0)
                OB_static = cpool.tile([128, mxB, 128], bf16, tag="OBs")
                nc.gpsimd.memset(OB_static[:], 0.0)

            qload = [0] * nq_use   # greedy per-queue descriptor balancing
            for _rep in range(repeat):
              # node-group state: transpose psum + sbuf ctxT for NB windows
              for b in range(n_batches):
                w0 = b * WPB
                nw = min(WPB, WPC - w0)
                cA0, cB0 = int(ofsA[w0]), int(ofsB[w0])
                ntA = int(ofsA[w0 + nw]) - cA0
                ntB = int(ofsB[w0 + nw]) - cB0
                if skip_gather:
                    GA, GB = GA_static, GB_static
                else:
                    GA = gpool.tile([128, mxA, 128], bf16, tag="GA")
                    GB = gpool.tile([128, mxB, 128], bf16, tag="GB")
                    na = ntA * 128
                    qn = qload.index(min(qload))
                    qload[qn] += na
                    nc.gpsimd.dma_gather(
                        out_ap=GA[:, 0:ntA, :], in_ap=tabA,
                        idxs_ap=idxa_sb[:, (cA0 * 128) // 16:
                                        ((cA0 + ntA) * 128) // 16],
                        num_idxs=na, num_idxs_reg=na, elem_size=128,
                        single_packet=False, queue_num=qn,
                    )
                    nb_ = ntB * 128
                    qn = qload.index(min(qload))
                    qload[qn] += nb_
                    nc.gpsimd.dma_gather(
                        out_ap=GB[:, 0:ntB, :], in_ap=tabB,
                        idxs_ap=idxb_sb[:, (cB0 * 128) // 16:
                                        ((cB0 + ntB) * 128) // 16],
                        num_idxs=nb_, num_idxs_reg=nb_, elem_size=128,
                        single_packet=False, queue_num=qn,
                    )
                if dump == "g" and not skip_gather:
                    nc.sync.dma_start(
                        dump_d[:, cA0 * 128:(cA0 + ntA) * 128],
                        GA[:, 0:ntA, :])
                if skip_onehot:
                    OA, OB = OA_static, OB_static
                else:
                    OA = opool.tile([128, mxA, 128], bf16, tag="OA")
                    OB = opool.tile([128, mxB, 128], bf16, tag="OB")
                    # O01 streams ride the Activation engine's HWDGE queue
                    # so they don't queue behind outputs/consts on SP
                    nc.scalar.dma_start(
                        OA[:, 0:ntA, :],
                        o01a_d[:, cA0 * 128:(cA0 + ntA) * 128])
                    nc.scalar.dma_start(
                        OB[:, 0:ntB, :],
                        o01b_d[:, cB0 * 128:(cB0 + ntB) * 128])
                    # O_s = O01 * ex  (walrus rejects ALU ops on Pool, so
                    # both scales run on DVE)
                    nc.vector.tensor_tensor(
                        out=OA[:, 0:ntA, :], in0=OA[:, 0:ntA, :],
                        in1=apx(exa_sb[:, cA0:cA0 + ntA], [[1, ntA],
                                                           [0, 128]]),
                        op=OP.mult)
                    nc.vector.tensor_tensor(
                        out=OB[:, 0:ntB, :], in0=OB[:, 0:ntB, :],
                        in1=apx(exb_sb[:, cB0:cB0 + ntB], [[1, ntB],
                                                           [0, 128]]),
                        op=OP.mult)

                for wl in range(nw):
                    w = w0 + wl
                    sAw, sBw = sAp[w], sBp[w]
                    SW = sAw + sBw
                    tA0 = int(ofsA[w]) - cA0    # window tile base in batch
                    tB0 = int(ofsB[w]) - cB0
                    psum_ud = pe_pool.tile([128, 132], f32, tag="psum_ud",
                                           bufs=1 if ud1 else 2)
                    if den_sep:
                        # den accumulates in its own PSUM bank: a second
                        # concurrently-open matmul group in the same 2KB
                        # zero region corrupts the first (hw start_tensor_
                        # calc marks the whole region pending-zero).
                        psum_dn = pe_pool.tile([128, 4], f32, tag="psum_dn",
                                               bufs=2)
                        den_ap = psum_dn[:, 0:1]
                    else:
                        den_ap = psum_ud[:, 128:129]
                    if not skip_mm:
                        def olh_grh(s_):
                            if s_ < sAw:
                                return (OA[:, tA0 + s_, :],
                                        GA[:, tA0 + s_, :])
                            return (OB[:, tB0 + (s_ - sAw), :],
                                    GB[:, tB0 + (s_ - sAw), :])
                        if den_seq:
                            for s_ in range(SW):
                                Olh, Grh = olh_grh(s_)
                                nc.tensor.matmul(
                                    psum_ud[:, 0:128], lhsT=Olh, rhs=Grh,
                                    start=(s_ == 0), stop=(s_ == SW - 1))
                            for s_ in range(SW):
                                Olh, _ = olh_grh(s_)
                                nc.tensor.matmul(
                                    den_ap, lhsT=Olh, rhs=onesc_sb[:],
                                    start=(s_ == 0), stop=(s_ == SW - 1))
                        else:
                            for s_ in range(SW):
                                Olh, Grh = olh_grh(s_)
                                nc.tensor.matmul(
                                    psum_ud[:, 0:128], lhsT=Olh, rhs=Grh,
                                    start=(s_ == 0), stop=(s_ == SW - 1))
                                nc.tensor.matmul(
                                    den_ap, lhsT=Olh,
                                    rhs=onesc_sb[:],
                                    start=(s_ == 0), stop=(s_ == SW - 1))

                    if skip_node:
                        continue

                    # ---- bridge: ctx = psum/den -> bf16 -> [feat, node] ----
                    g = w // NB
                    gl = w % NB
                    nwin = min(NB, WPC - g * NB)
                    trdt = f32 if tr_f32 else bf16
                    if gl == 0:
                        psum_tr = ptr_pool.tile([128, NB * 128], trdt,
                                                tag="psum_tr")
                        cur_tr = psum_tr
                    den = bpool.tile([128, 1], f32, tag="den")
                    nc.vector.tensor_scalar(
                        out=den[:], in0=den_ap, scalar1=1e-30,
                        scalar2=None, op0=OP.max)
                    rec = bpool.tile([128, 1], f32, tag="rec")
                    nc.vector.reciprocal(rec[:], den[:])
                    ctx16 = bpool.tile([128, 128], trdt, tag="ctx16")
                    nc.vector.tensor_scalar(
                        out=ctx16[:], in0=psum_ud[:, 0:128],
                        scalar1=rec[:, 0:1], scalar2=None, op0=OP.mult)
                    if dump == "ctx":
                        cdump = bpool.tile([128, 128], f32, tag="cdump")
                        nc.vector.tensor_scalar(
                            out=cdump[:], in0=psum_ud[:, 0:128],
                            scalar1=rec[:, 0:1], scalar2=None, op0=OP.mult)
                        nc.sync.dma_start(
                            dump_d[w * 128:(w + 1) * 128, :], cdump[:])
                    elif dump == "den":
                        cdump = bpool.tile([128, 128], f32, tag="cdump")
                        nc.vector.tensor_scalar(
                            out=cdump[:, 0:1], in0=den_ap,
                            scalar1=1.0, scalar2=None, op0=OP.mult)
                        nc.vector.tensor_copy(out=cdump[:, 1:128],
                                              in_=psum_ud[:, 1:128])
                        nc.sync.dma_start(
                            dump_d[w * 128:(w + 1) * 128, :], cdump[:])
                    nc.tensor.transpose(
                        cur_tr[:, gl * 128:(gl + 1) * 128], ctx16[:],
                        ident_sb[:] if not tr_f32 else identf_sb[:])

                    if gl != nwin - 1:
                        continue

                    # ---- node phase for group g: windows [g*NB, g*NB+nwin)
                    C = nwin * 128
                    c0 = g * NB * 128
                    ctxT = bpool.tile([128, NB * 128], bf16, tag="ctxT")
                    nc.vector.tensor_copy(out=ctxT[:, 0:C],
                                          in_=cur_tr[:, 0:C])

                    # proj + ELU -> ctxT2 (bf16)
                    psum_cT = pn_pool.tile([128, NB * 128], f32,
                                           tag="psum_cT")
                    nc.tensor.matmul(psum_cT[:, 0:C], lhsT=wproj_sb[:],
                                     rhs=ctxT[:, 0:C], start=True, stop=True)
                    eA = wpool.tile([128, NB * 128], f32, tag="eA")
                    nc.vector.tensor_scalar(
                        out=eA[:, 0:C], in0=psum_cT[:, 0:C],
                        scalar1=bproj_sb[:, 0:1], scalar2=0.0,
                        op0=OP.add, op1=OP.min)
                    nc.scalar.activation(eA[:, 0:C], eA[:, 0:C], AF.Exp)
                    eB = wpool.tile([128, NB * 128], f32, tag="eB")
                    nc.vector.tensor_scalar(
                        out=eB[:, 0:C], in0=psum_cT[:, 0:C],
                        scalar1=bproj_sb[:, 0:1], scalar2=0.0,
                        op0=OP.add, op1=OP.max)
                    # elu = (eA - 1) + eB, fused on DVE
                    ctxT2 = wpool.tile([128, NB * 128], bf16, tag="ctxT2")
                    nc.vector.scalar_tensor_tensor(
                        out=ctxT2[:, 0:C], in0=eA[:, 0:C], scalar=1.0,
                        in1=eB[:, 0:C], op0=OP.subtract, op1=OP.add)

                    # GRU gates, [gate, node] layout, weights stationary.
                    # r/n share a PSUM bank sequentially, h/z share another
                    # (group lifetimes don't overlap; WAR via tile reuse).
                    nf16c = nfT16_sb[:, c0:c0 + C]
                    psum_rn = pn_pool.tile([128, NB * 128], f32,
                                           tag="psum_rn")
                    nc.tensor.matmul(psum_rn[:, 0:C], lhsT=wih_sb[:, 0:128],
                                     rhs=ctxT2[:, 0:C], start=True,
                                     stop=False)
                    nc.tensor.matmul(psum_rn[:, 0:C], lhsT=whh_sb[:, 0:128],
                                     rhs=nf16c, start=False, stop=True)
                    psum_hz = pn_pool.tile([128, NB * 128], f32,
                                           tag="psum_hz")
                    nc.tensor.matmul(psum_hz[:, 0:C],
                                     lhsT=whh_sb[:, 256:384],
                                     rhs=nf16c, start=True, stop=True)

                    # r = sigmoid(s) = 0.5*tanh(0.5 s + 0.5 b_r) + 0.5
                    tr_ = wpool.tile([128, NB * 128], f32, tag="tr_")
                    nc.scalar.activation(tr_[:, 0:C], psum_rn[:, 0:C],
                                         AF.Tanh, bias=brh_sb[:, 0:1],
                                         scale=0.5)
                    # hnb2 = 0.5 h_n + 0.5 b_hn
                    hnb2 = wpool.tile([128, NB * 128], f32, tag="hnb2")
                    nc.vector.tensor_scalar(
                        out=hnb2[:, 0:C], in0=psum_hz[:, 0:C], scalar1=0.5,
                        scalar2=bnhh_sb[:, 0:1], op0=OP.mult, op1=OP.add)

                    # n-gate reuses the r bank; z reuses the h bank
                    nc.tensor.matmul(psum_rn[:, 0:C],
                                     lhsT=wih_sb[:, 256:384],
                                     rhs=ctxT2[:, 0:C], start=True, stop=True)
                    nc.tensor.matmul(psum_hz[:, 0:C],
                                     lhsT=wih_sb[:, 128:256],
                                     rhs=ctxT2[:, 0:C], start=True,
                                     stop=False)
                    nc.tensor.matmul(psum_hz[:, 0:C],
                                     lhsT=whh_sb[:, 128:256],
                                     rhs=nf16c, start=False, stop=True)

                    # n = tanh(i_n + b_in + (tr+1)*hnb2)
                    qq = wpool.tile([128, NB * 128], f32, tag="qq")
                    nc.vector.scalar_tensor_tensor(
                        out=qq[:, 0:C], in0=tr_[:, 0:C], scalar=1.0,
                        in1=hnb2[:, 0:C], op0=OP.add, op1=OP.mult)
                    nc.vector.tensor_tensor(out=qq[:, 0:C], in0=qq[:, 0:C],
                                            in1=psum_rn[:, 0:C], op=OP.add)
                    nn = wpool.tile([128, NB * 128], f32, tag="nn")
                    nc.scalar.activation(nn[:, 0:C], qq[:, 0:C], AF.Tanh,
                                         bias=bni_sb[:, 0:1])
                    tz_ = wpool.tile([128, NB * 128], f32, tag="tz_")
                    nc.scalar.activation(tz_[:, 0:C], psum_hz[:, 0:C],
                                         AF.Tanh, bias=bzh_sb[:, 0:1],
                                         scale=0.5)
                    # h = n + (0.5 tz + 0.5)*(nf - n)
                    dd = wpool.tile([128, NB * 128], f32, tag="dd")
                    nc.vector.tensor_tensor(out=dd[:, 0:C], in0=nf16c,
                                            in1=nn[:, 0:C], op=OP.subtract)
                    uu = wpool.tile([128, NB * 128], f32, tag="uu")
                    nc.vector.scalar_tensor_tensor(
                        out=uu[:, 0:C], in0=tz_[:, 0:C], scalar=1.0,
                        in1=dd[:, 0:C], op0=OP.add, op1=OP.mult)
                    nc.vector.scalar_tensor_tensor(
                        out=dd[:, 0:C], in0=uu[:, 0:C], scalar=0.5,
                        in1=nn[:, 0:C], op0=OP.mult, op1=OP.add)
                    outt = qpool.tile([128, NB * 128], f32, tag="outt")
                    nc.scalar.activation(outt[:, 0:C], dd[:, 0:C], AF.Relu)
                    nc.sync.dma_start(out_d[:, c0:c0 + C], outt[:, 0:C])

    nc.compile()
    return nc


def _prep(edge_logits, node_feats, W_proj, b_proj, w_ih, w_hh, b_ih, b_hh,
          src, dst):
    """Host-side sharding. Returns (T_win, sAp, sBp, in_maps).

    Windows are sorted by edge count and dealt round-robin to (core,
    position) so the 8 windows sharing a position have similar counts;
    slot counts are per-position (max over cores) instead of one global
    max -- cuts gather padding from ~19% to a few %.  _prep stashes the
    window assignment in module global _wassign for kernel() to invert.
    """
    global _wassign
    import ml_dtypes
    BF16 = ml_dtypes.bfloat16

    logits = np.asarray(edge_logits, np.float32).reshape(-1)
    src = np.asarray(src, np.int64)
    dst = np.asarray(dst, np.int64)

    is_b = (src >= S_SPLIT).astype(np.int64)
    win = dst // 128
    key = win * 2 + is_b
    order = np.argsort(key, kind="stable")
    key_s = key[order]
    src_s = src[order]
    dst_s = dst[order]
    log_s = logits[order]

    counts = np.bincount(key_s, minlength=WTOT * 2)
    cA = counts[0::2]
    cB = counts[1::2]

    # sorted round-robin window assignment: rank p -> (core p%NC, pos p//NC)
    worder = np.argsort(-(cA + cB), kind="stable")
    wassign = worder.reshape(WPC, NC).T          # [NC, WPC] window ids
    _wassign = wassign
    core_of = np.zeros(WTOT, np.int64)
    pos_of = np.zeros(WTOT, np.int64)
    core_of[worder] = np.arange(WTOT) % NC
    pos_of[worder] = np.arange(WTOT) // NC

    # per-position slot counts (max over the 8 cores at that position)
    cA_kp = cA[wassign]                          # [NC, WPC]
    cB_kp = cB[wassign]
    sAp = ((cA_kp.max(axis=0) + 127) // 128).astype(np.int64)
    sBp = ((cB_kp.max(axis=0) + 127) // 128).astype(np.int64)
    ofsA = np.concatenate([[0], np.cumsum(sAp)])
    ofsB = np.concatenate([[0], np.cumsum(sBp)])
    CAc = int(ofsA[-1])
    CBc = int(ofsB[-1])
    T_win = int(sAp.sum() + sBp.sum())
    sA_ret = tuple(int(x) for x in sAp)
    sB_ret = tuple(int(x) for x in sBp)

    starts = np.zeros(WTOT * 2, np.int64)
    starts[1:] = np.cumsum(counts)[:-1]
    pos = np.arange(E, dtype=np.int64) - starts[key_s]

    winv = key_s // 2
    grp = key_s % 2
    idxA = np.zeros(NC * CAc * 128, np.int16)
    idxB = np.zeros(NC * CBc * 128, np.int16)
    dstlA = np.full(NC * CAc * 128, -1.0, np.float32)
    dstlB = np.full(NC * CBc * 128, -1.0, np.float32)
    logA = np.zeros(NC * CAc * 128, np.float32)
    logB = np.zeros(NC * CBc * 128, np.float32)

    mA = grp == 0
    mB = ~mA
    wA = winv[mA]
    wB = winv[mB]
    flatA = (core_of[wA] * CAc + ofsA[pos_of[wA]]) * 128 + pos[mA]
    flatB = (core_of[wB] * CBc + ofsB[pos_of[wB]]) * 128 + pos[mB]
    idxA[flatA] = src_s[mA].astype(np.int16)
    idxB[flatB] = (src_s[mB] - OFF_B).astype(np.int16)
    dstlA[flatA] = (dst_s[mA] - wA * 128).astype(np.float32)
    dstlB[flatB] = (dst_s[mB] - wB * 128).astype(np.float32)
    logA[flatA] = log_s[mA]
    logB[flatB] = log_s[mB]

    def core_tiles(a, slots_tot):
        a = a.reshape(NC, slots_tot, 128)
        return [np.ascontiguousarray(a[k].T) for k in range(NC)]

    dstlA_cores = core_tiles(dstlA, CAc)
    dstlB_cores = core_tiles(dstlB, CBc)
    logA_cores = core_tiles(logA, CAc)
    logB_cores = core_tiles(logB, CBc)

    jj = np.arange(128, dtype=np.float32)

    def onehot_cores(dstl_cores, slots_tot):
        out = []
        for d in dstl_cores:
            o = (d[:, :, None] == jj[None, None, :]).astype(BF16)
            out.append(np.ascontiguousarray(
                o.reshape(128, slots_tot * 128)))
        return out

    o01A_cores = onehot_cores(dstlA_cores, CAc)
    o01B_cores = onehot_cores(dstlB_cores, CBc)

    def core_idx(a, slots_tot):
        a = a.reshape(NC, slots_tot * 128)
        out = []
        for k in range(NC):
            blk = a[k].reshape(-1, 16).T      # [16, L/16], i -> [i%16,i//16]
            out.append(np.ascontiguousarray(np.tile(blk, (8, 1))))
        return out

    idxA_cores = core_idx(idxA, CAc)
    idxB_cores = core_idx(idxB, CBc)

    nf = np.asarray(node_feats, np.float32)
    nf16 = nf.astype(BF16)
    nf_pad = np.zeros((WTOT * 128, F), np.float32)
    nf_pad[:V] = nf

    tab16 = np.ascontiguousarray(nf16)
    tabb16 = np.ascontiguousarray(nf16[OFF_B:])
    wproj16 = np.ascontiguousarray(np.asarray(W_proj, np.float32).T
                                   .astype(BF16))
    wih16 = np.ascontiguousarray(np.asarray(w_ih, np.float32).T.astype(BF16))
    whh16 = np.ascontiguousarray(np.asarray(w_hh, np.float32).T.astype(BF16))
    bih = np.asarray(b_ih, np.float32).reshape(384)
    bhh = np.asarray(b_hh, np.float32).reshape(384)
    bprojc = np.asarray(b_proj, np.float32).reshape(128, 1)
    brh = (0.5 * (bih[0:128] + bhh[0:128])).reshape(128, 1)
    bzh = (0.5 * (bih[128:256] + bhh[128:256])).reshape(128, 1)
    bnic = bih[256:384].reshape(128, 1)
    bnhh = (0.5 * bhh[256:384]).reshape(128, 1)
    ident16 = np.eye(128, dtype=BF16)
    identf = np.eye(128, dtype=np.float32)
    onesc16 = np.ones((128, 1), BF16)

    in_maps = []
    for k in range(NC):
        sl = nf_pad[k * NPC:(k + 1) * NPC]
        nfT = np.ascontiguousarray(sl.T)
        nfT16 = np.ascontiguousarray(sl.T.astype(BF16))
        in_maps.append({
            "idxa": idxA_cores[k], "idxb": idxB_cores[k],
            "o01a": o01A_cores[k], "o01b": o01B_cores[k],
            "loga": logA_cores[k], "logb": logB_cores[k],
            "tab16": tab16, "tabb16": tabb16,
            "nfT16": nfT16,
            "wproj16": wproj16, "wih16": wih16, "whh16": whh16,
            "bprojc": bprojc, "brh": brh, "bzh": bzh,
            "bnic": bnic, "bnhh": bnhh,
            "ident16": ident16, "identf": identf, "onesc16": onesc16,
        })
    return T_win, sA, sB, in_maps


def kernel(edge_logits, node_feats, W_proj, b_proj, w_ih, w_hh, b_ih, b_hh,
           src, dst):
    from concourse.bass_utils import run_bass_kernel_spmd

    T_win, sA, sB, in_maps = _prep(edge_logits, node_feats, W_proj, b_proj,
                                   w_ih, w_hh, b_ih, b_hh, src, dst)
    key = (T_win, sA, sB)
    if key not in _compiled:
        _compiled[key] = _build_nc(T_win, sA=sA, sB=sB)
    nc = _compiled[key]

    res = run_bass_kernel_spmd(nc, in_maps, list(range(NC)))
    full = np.concatenate(
        [np.ascontiguousarray(res.results[k]["out"]).T for k in range(NC)],
        axis=0)
    return np.ascontiguousarray(full[:V]).astype(np.float32)


# revision 48
# speedup vs baseline: 5.7611x; 2.1324x over previous
"""AttentiveGRU2 Trainium2 Bass kernel (v2).

Model (see reference):
  edge-softmax over incoming edges per dst node, attention-weighted
  gather of projected node features, segment-sum per dst, ELU, GRUCell.

Strategy (8 NeuronCores, SPMD, no collectives):
  * Host sorts edges by dst window (392 windows of 128 node ids; 49
    windows per core). Softmax folded through the segment sum:
    ctx_v = (sum_e ex_e nf[src_e]) / (sum_e ex_e); proj applied after.
  * Gather: node-feature table in bf16 (256B rows), hardware
    InstDMAGatherAnt across 4 SWDGE queues (the per-queue descriptor
    rate ~8ns/desc is the kernel bottleneck; 4 queues x bf16 measured
    ~4x faster than the fp32 single-queue baseline). int16 idx limit
    handled with two overlapping row views (A: src<32768, B: src-17232).
  * One-hot dst matrices are 0/1 bf16 built on host and streamed in via
    regular DMA (cheap sequential traffic) -- the only on-device
    elementwise edge work is O_s = O01 * ex (split DVE/Pool engines).
  * Edge matmuls per 128-edge slot tile (bf16, 1 cyc/row):
      psum[v,0:128] += O_s^T @ G_raw,  psum[v,128] += O_s^T @ ones.
  * Node phase in [channel, node] layout, weights stationary, batched
    over NB=4 windows: ctx scaled by 1/den -> bf16 -> PE transpose ->
    proj + ELU -> GRU gates. Sigmoid avoided via 0.5*tanh(x/2)+0.5 so
    every activation (Exp/Tanh/Relu) lives in one table: zero 1283ns
    act-table reloads. Biases folded into activation bias APs.
  * Output written [feat, node]; host transposes back.
"""

import numpy as np

V, E, F = 50000, 800000, 128
NC = 8
WPC = 49              # windows per core
NPC = WPC * 128       # 6272 node slots per core
WTOT = NC * WPC       # 392 windows total
WPB = 2               # windows per gather batch
NB = 4                # windows per node-phase group
S_SPLIT = 32768       # src < S -> table A
OFF_B = V - 32768     # 17232; table B rows [OFF_B, V)

_compiled = {}
_wassign = None   # [NC, WPC] window assignment from the last _prep


def _build_nc(T_win, sA=None, sB=None, skip_gather=False, skip_onehot=False,
              skip_mm=False, skip_node=False, repeat=1, one_act=False,
              nq_use=4, den_sep=True, tr_f32=False, den_seq=False,
              dump=None, ud1=False):
    import concourse.bass as bass
    import concourse.bacc as bacc
    import concourse.mybir as mybir
    import concourse.tile as tile

    f32 = mybir.dt.float32
    bf16 = mybir.dt.bfloat16
    i16 = mybir.dt.int16
    AF = mybir.ActivationFunctionType
    OP = mybir.AluOpType

    # sA/sB: per-position slot-count lists (scalars = uniform legacy)
    sAp = list(sA) if not isinstance(sA, int) else [sA] * WPC
    sBp = list(sB) if not isinstance(sB, int) else [sB] * WPC
    ofsA = np.concatenate([[0], np.cumsum(sAp)]).astype(int)
    ofsB = np.concatenate([[0], np.cumsum(sBp)]).astype(int)
    CA = int(ofsA[-1])      # A slot-tiles per core
    CB = int(ofsB[-1])
    LA = CA * 128           # A-gather idx count per core
    LB = CB * 128
    n_batches = (WPC + WPB - 1) // WPB
    mxA = max(int(ofsA[min(b * WPB + WPB, WPC)] - ofsA[b * WPB])
              for b in range(n_batches))
    mxB = max(int(ofsB[min(b * WPB + WPB, WPC)] - ofsB[b * WPB])
              for b in range(n_batches))

    nc = bacc.Bacc("TRN2", target_bir_lowering=False, debug=False,
                   num_devices=NC, num_swdge_queues=4)

    # ---- DRAM parameters ----
    idxa_d = nc.dram_tensor("idxa", [128, LA // 16], i16, kind="ExternalInput")
    idxb_d = nc.dram_tensor("idxb", [128, LB // 16], i16, kind="ExternalInput")
    # masked logits: logit value at the one-hot position, -100 elsewhere;
    # one Exp on the Activation engine turns a tile into the scaled one-hot
    lma_d = nc.dram_tensor("lma", [128, CA * 128], bf16,
                           kind="ExternalInput")
    lmb_d = nc.dram_tensor("lmb", [128, CB * 128], bf16,
                           kind="ExternalInput")
    tab16_d = nc.dram_tensor("tab16", [V, F], bf16, kind="ExternalInput")
    tabb16_d = nc.dram_tensor("tabb16", [32768, F], bf16,
                              kind="ExternalInput")
    nfT16_d = nc.dram_tensor("nfT16", [128, NPC], bf16, kind="ExternalInput")
    wproj16_d = nc.dram_tensor("wproj16", [128, 128], bf16,
                               kind="ExternalInput")
    wih16_d = nc.dram_tensor("wih16", [128, 384], bf16, kind="ExternalInput")
    whh16_d = nc.dram_tensor("whh16", [128, 384], bf16, kind="ExternalInput")
    bproj_d = nc.dram_tensor("bprojc", [128, 1], f32, kind="ExternalInput")
    brh_d = nc.dram_tensor("brh", [128, 1], f32, kind="ExternalInput")
    bzh_d = nc.dram_tensor("bzh", [128, 1], f32, kind="ExternalInput")
    bni_d = nc.dram_tensor("bnic", [128, 1], f32, kind="ExternalInput")
    bnhh_d = nc.dram_tensor("bnhh", [128, 1], f32, kind="ExternalInput")
    ident16_d = nc.dram_tensor("ident16", [128, 128], bf16,
                               kind="ExternalInput")
    identf_d = nc.dram_tensor("identf", [128, 128], f32,
                              kind="ExternalInput")
    onesc16_d = nc.dram_tensor("onesc16", [128, 1], bf16,
                               kind="ExternalInput")
    out_d = nc.dram_tensor("out", [128, NPC], f32, kind="ExternalOutput")
    if dump == "g":
        dump_d = nc.dram_tensor("dmp", [128, CA * 128], bf16,
                                kind="ExternalOutput")
    elif dump:
        dump_d = nc.dram_tensor("dmp", [NPC, 128], f32,
                                kind="ExternalOutput")

    tabA = tab16_d[0:32768, :]
    tabB = tabb16_d[:]

    def apx(base, dims):
        return bass.AP(base.tensor, base.offset,
                       [list(base.ap[0])] + dims)

    with tile.TileContext(nc) as tc:
        with (
            tc.tile_pool(name="const", bufs=1) as cpool,
            tc.tile_pool(name="gat", bufs=4) as gpool,
            tc.tile_pool(name="oh", bufs=4) as opool,
            tc.tile_pool(name="wrk", bufs=1) as wpool,
            tc.tile_pool(name="brdg", bufs=2) as bpool,
            tc.tile_pool(name="outp", bufs=2) as qpool,
            tc.tile_pool(name="pedge", bufs=1, space="PSUM") as pe_pool,
            tc.tile_pool(name="ptr", bufs=1, space="PSUM") as ptr_pool,
            tc.tile_pool(name="pnode", bufs=1, space="PSUM") as pn_pool,
        ):
            def load(pool, name, dram, shape, dtype=f32):
                t = pool.tile(shape, dtype, tag=name)
                nc.sync.dma_start(t[:], dram[:])
                return t

            ident_sb = load(cpool, "ident16", ident16_d, [128, 128], bf16)
            identf_sb = load(cpool, "identf", identf_d, [128, 128], f32)
            onesc_sb = load(cpool, "onesc16", onesc16_d, [128, 1], bf16)
            wproj_sb = load(cpool, "wproj16", wproj16_d, [128, 128], bf16)
            wih_sb = load(cpool, "wih16", wih16_d, [128, 384], bf16)
            whh_sb = load(cpool, "whh16", whh16_d, [128, 384], bf16)
            bproj_sb = load(cpool, "bprojc", bproj_d, [128, 1])
            brh_sb = load(cpool, "brh", brh_d, [128, 1])
            bzh_sb = load(cpool, "bzh", bzh_d, [128, 1])
            bni_sb = load(cpool, "bnic", bni_d, [128, 1])
            bnhh_sb = load(cpool, "bnhh", bnhh_d, [128, 1])
            idxa_sb = load(cpool, "idxa", idxa_d, [128, LA // 16], i16)
            idxb_sb = load(cpool, "idxb", idxb_d, [128, LB // 16], i16)
            nfT16_sb = load(cpool, "nfT16", nfT16_d, [128, NPC], bf16)

            GA_static = GB_static = None
            if skip_gather:
                GA_static = cpool.tile([128, mxA, 128], bf16, tag="GAs")
                nc.gpsimd.memset(GA_static[:], 0.0)
                GB_static = cpool.tile([128, mxB, 128], bf16, tag="GBs")
                nc.gpsimd.memset(GB_static[:], 0.0)
            OA_static = OB_static = None
            if skip_onehot:
                OA_static = cpool.tile([128, mxA, 128], bf16, tag="OAs")
                nc.gpsimd.memset(OA_static[:], 0.0)
                OB_static = cpool.tile([128, mxB, 128], bf16, tag="OBs")
                nc.gpsimd.memset(OB_static[:], 0.0)

            qload = [0] * nq_use   # greedy per-queue descriptor balancing
            for _rep in range(repeat):
              # node-group state: transpose psum + sbuf ctxT for NB windows
              for b in range(n_batches):
                w0 = b * WPB
                nw = min(WPB, WPC - w0)
                cA0, cB0 = int(ofsA[w0]), int(ofsB[w0])
                ntA = int(ofsA[w0 + nw]) - cA0
                ntB = int(ofsB[w0 + nw]) - cB0
                if skip_gather:
                    GA, GB = GA_static, GB_static
                else:
                    GA = gpool.tile([128, mxA, 128], bf16, tag="GA")
                    GB = gpool.tile([128, mxB, 128], bf16, tag="GB")
                    na = ntA * 128
                    qn = qload.index(min(qload))
                    qload[qn] += na
                    nc.gpsimd.dma_gather(
                        out_ap=GA[:, 0:ntA, :], in_ap=tabA,
                        idxs_ap=idxa_sb[:, (cA0 * 128) // 16:
                                        ((cA0 + ntA) * 128) // 16],
                        num_idxs=na, num_idxs_reg=na, elem_size=128,
                        single_packet=False, queue_num=qn,
                    )
                    nb_ = ntB * 128
                    qn = qload.index(min(qload))
                    qload[qn] += nb_
                    nc.gpsimd.dma_gather(
                        out_ap=GB[:, 0:ntB, :], in_ap=tabB,
                        idxs_ap=idxb_sb[:, (cB0 * 128) // 16:
                                        ((cB0 + ntB) * 128) // 16],
                        num_idxs=nb_, num_idxs_reg=nb_, elem_size=128,
                        single_packet=False, queue_num=qn,
                    )
                if dump == "g" and not skip_gather:
                    nc.sync.dma_start(
                        dump_d[:, cA0 * 128:(cA0 + ntA) * 128],
                        GA[:, 0:ntA, :])
                if skip_onehot:
                    OA, OB = OA_static, OB_static
                else:
                    OA = opool.tile([128, mxA, 128], bf16, tag="OA")
                    OB = opool.tile([128, mxB, 128], bf16, tag="OB")
                    nc.sync.dma_start(
                        OA[:, 0:ntA, :],
                        lma_d[:, cA0 * 128:(cA0 + ntA) * 128])
                    nc.sync.dma_start(
                        OB[:, 0:ntB, :],
                        lmb_d[:, cB0 * 128:(cB0 + ntB) * 128])
                    # O_s = exp(masked logits): the scaled one-hot in one
                    # Activation-engine op (exp(-100) == 0)
                    nc.scalar.activation(OA[:, 0:ntA, :], OA[:, 0:ntA, :],
                                         AF.Exp)
                    nc.scalar.activation(OB[:, 0:ntB, :], OB[:, 0:ntB, :],
                                         AF.Exp)

                for wl in range(nw):
                    w = w0 + wl
                    sAw, sBw = sAp[w], sBp[w]
                    SW = sAw + sBw
                    tA0 = int(ofsA[w]) - cA0    # window tile base in batch
                    tB0 = int(ofsB[w]) - cB0
                    psum_ud = pe_pool.tile([128, 132], f32, tag="psum_ud",
                                           bufs=1 if ud1 else 2)
                    if den_sep:
                        # den accumulates in its own PSUM bank: a second
                        # concurrently-open matmul group in the same 2KB
                        # zero region corrupts the first (hw start_tensor_
                        # calc marks the whole region pending-zero).
                        psum_dn = pe_pool.tile([128, 4], f32, tag="psum_dn",
                                               bufs=2)
                        den_ap = psum_dn[:, 0:1]
                    else:
                        den_ap = psum_ud[:, 128:129]
                    if not skip_mm:
                        def olh_grh(s_):
                            if s_ < sAw:
                                return (OA[:, tA0 + s_, :],
                                        GA[:, tA0 + s_, :])
                            return (OB[:, tB0 + (s_ - sAw), :],
                                    GB[:, tB0 + (s_ - sAw), :])
                        if den_seq:
                            for s_ in range(SW):
                                Olh, Grh = olh_grh(s_)
                                nc.tensor.matmul(
                                    psum_ud[:, 0:128], lhsT=Olh, rhs=Grh,
                                    start=(s_ == 0), stop=(s_ == SW - 1))
                            for s_ in range(SW):
                                Olh, _ = olh_grh(s_)
                                nc.tensor.matmul(
                                    den_ap, lhsT=Olh, rhs=onesc_sb[:],
                                    start=(s_ == 0), stop=(s_ == SW - 1))
                        else:
                            for s_ in range(SW):
                                Olh, Grh = olh_grh(s_)
                                nc.tensor.matmul(
                                    psum_ud[:, 0:128], lhsT=Olh, rhs=Grh,
                                    start=(s_ == 0), stop=(s_ == SW - 1))
                                nc.tensor.matmul(
                                    den_ap, lhsT=Olh,
                                    rhs=onesc_sb[:],
                                    start=(s_ == 0), stop=(s_ == SW - 1))

                    if skip_node:
                        continue

                    # ---- bridge: ctx = psum/den -> bf16 -> [feat, node] ----
                    g = w // NB
                    gl = w % NB
                    nwin = min(NB, WPC - g * NB)
                    trdt = f32 if tr_f32 else bf16
                    if gl == 0:
                        psum_tr = ptr_pool.tile([128, NB * 128], trdt,
                                                tag="psum_tr")
                        cur_tr = psum_tr
                    den = bpool.tile([128, 1], f32, tag="den")
                    nc.vector.tensor_scalar(
                        out=den[:], in0=den_ap, scalar1=1e-30,
                        scalar2=None, op0=OP.max)
                    rec = bpool.tile([128, 1], f32, tag="rec")
                    nc.vector.reciprocal(rec[:], den[:])
                    ctx16 = bpool.tile([128, 128], trdt, tag="ctx16")
                    nc.vector.tensor_scalar(
                        out=ctx16[:], in0=psum_ud[:, 0:128],
                        scalar1=rec[:, 0:1], scalar2=None, op0=OP.mult)
                    if dump == "ctx":
                        cdump = bpool.tile([128, 128], f32, tag="cdump")
                        nc.vector.tensor_scalar(
                            out=cdump[:], in0=psum_ud[:, 0:128],
                            scalar1=rec[:, 0:1], scalar2=None, op0=OP.mult)
                        nc.sync.dma_start(
                            dump_d[w * 128:(w + 1) * 128, :], cdump[:])
                    elif dump == "den":
                        cdump = bpool.tile([128, 128], f32, tag="cdump")
                        nc.vector.tensor_scalar(
                            out=cdump[:, 0:1], in0=den_ap,
                            scalar1=1.0, scalar2=None, op0=OP.mult)
                        nc.vector.tensor_copy(out=cdump[:, 1:128],
                                              in_=psum_ud[:, 1:128])
                        nc.sync.dma_start(
                            dump_d[w * 128:(w + 1) * 128, :], cdump[:])
                    nc.tensor.transpose(
                        cur_tr[:, gl * 128:(gl + 1) * 128], ctx16[:],
                        ident_sb[:] if not tr_f32 else identf_sb[:])

                    if gl != nwin - 1:
                        continue

                    # ---- node phase for group g: windows [g*NB, g*NB+nwin)
                    C = nwin * 128
                    c0 = g * NB * 128
                    ctxT = bpool.tile([128, NB * 128], bf16, tag="ctxT")
                    nc.vector.tensor_copy(out=ctxT[:, 0:C],
                                          in_=cur_tr[:, 0:C])

                    # proj + ELU -> ctxT2 (bf16)
                    psum_cT = pn_pool.tile([128, NB * 128], f32,
                                           tag="psum_cT")
                    nc.tensor.matmul(psum_cT[:, 0:C], lhsT=wproj_sb[:],
                                     rhs=ctxT[:, 0:C], start=True, stop=True)
                    eA = wpool.tile([128, NB * 128], f32, tag="eA")
                    nc.vector.tensor_scalar(
                        out=eA[:, 0:C], in0=psum_cT[:, 0:C],
                        scalar1=bproj_sb[:, 0:1], scalar2=0.0,
                        op0=OP.add, op1=OP.min)
                    nc.scalar.activation(eA[:, 0:C], eA[:, 0:C], AF.Exp)
                    eB = wpool.tile([128, NB * 128], f32, tag="eB")
                    nc.vector.tensor_scalar(
                        out=eB[:, 0:C], in0=psum_cT[:, 0:C],
                        scalar1=bproj_sb[:, 0:1], scalar2=0.0,
                        op0=OP.add, op1=OP.max)
                    # elu = (eA - 1) + eB, fused on DVE
                    ctxT2 = wpool.tile([128, NB * 128], bf16, tag="ctxT2")
                    nc.vector.scalar_tensor_tensor(
                        out=ctxT2[:, 0:C], in0=eA[:, 0:C], scalar=1.0,
                        in1=eB[:, 0:C], op0=OP.subtract, op1=OP.add)

                    # GRU gates, [gate, node] layout, weights stationary.
                    # r/n share a PSUM bank sequentially, h/z share another
                    # (group lifetimes don't overlap; WAR via tile reuse).
                    nf16c = nfT16_sb[:, c0:c0 + C]
                    psum_rn = pn_pool.tile([128, NB * 128], f32,
                                           tag="psum_rn")
                    nc.tensor.matmul(psum_rn[:, 0:C], lhsT=wih_sb[:, 0:128],
                                     rhs=ctxT2[:, 0:C], start=True,
                                     stop=False)
                    nc.tensor.matmul(psum_rn[:, 0:C], lhsT=whh_sb[:, 0:128],
                                     rhs=nf16c, start=False, stop=True)
                    psum_hz = pn_pool.tile([128, NB * 128], f32,
                                           tag="psum_hz")
                    nc.tensor.matmul(psum_hz[:, 0:C],
                                     lhsT=whh_sb[:, 256:384],
                                     rhs=nf16c, start=True, stop=True)

                    # r = sigmoid(s) = 0.5*tanh(0.5 s + 0.5 b_r) + 0.5
                    tr_ = wpool.tile([128, NB * 128], f32, tag="tr_")
                    nc.scalar.activation(tr_[:, 0:C], psum_rn[:, 0:C],
                                         AF.Tanh, bias=brh_sb[:, 0:1],
                                         scale=0.5)
                    # hnb2 = 0.5 h_n + 0.5 b_hn
                    hnb2 = wpool.tile([128, NB * 128], f32, tag="hnb2")
                    nc.vector.tensor_scalar(
                        out=hnb2[:, 0:C], in0=psum_hz[:, 0:C], scalar1=0.5,
                        scalar2=bnhh_sb[:, 0:1], op0=OP.mult, op1=OP.add)

                    # n-gate reuses the r bank; z reuses the h bank
                    nc.tensor.matmul(psum_rn[:, 0:C],
                                     lhsT=wih_sb[:, 256:384],
                                     rhs=ctxT2[:, 0:C], start=True, stop=True)
                    nc.tensor.matmul(psum_hz[:, 0:C],
                                     lhsT=wih_sb[:, 128:256],
                                     rhs=ctxT2[:, 0:C], start=True,
                                     stop=False)
                    nc.tensor.matmul(psum_hz[:, 0:C],
                                     lhsT=whh_sb[:, 128:256],
                                     rhs=nf16c, start=False, stop=True)

                    # n = tanh(i_n + b_in + (tr+1)*hnb2)
                    qq = wpool.tile([128, NB * 128], f32, tag="qq")
                    nc.vector.scalar_tensor_tensor(
                        out=qq[:, 0:C], in0=tr_[:, 0:C], scalar=1.0,
                        in1=hnb2[:, 0:C], op0=OP.add, op1=OP.mult)
                    nc.vector.tensor_tensor(out=qq[:, 0:C], in0=qq[:, 0:C],
                                            in1=psum_rn[:, 0:C], op=OP.add)
                    nn = wpool.tile([128, NB * 128], f32, tag="nn")
                    nc.scalar.activation(nn[:, 0:C], qq[:, 0:C], AF.Tanh,
                                         bias=bni_sb[:, 0:1])
                    tz_ = wpool.tile([128, NB * 128], f32, tag="tz_")
                    nc.scalar.activation(tz_[:, 0:C], psum_hz[:, 0:C],
                                         AF.Tanh, bias=bzh_sb[:, 0:1],
                                         scale=0.5)
                    # h = n + (0.5 tz + 0.5)*(nf - n)
                    dd = wpool.tile([128, NB * 128], f32, tag="dd")
                    nc.vector.tensor_tensor(out=dd[:, 0:C], in0=nf16c,
                                            in1=nn[:, 0:C], op=OP.subtract)
                    uu = wpool.tile([128, NB * 128], f32, tag="uu")
                    nc.vector.scalar_tensor_tensor(
                        out=uu[:, 0:C], in0=tz_[:, 0:C], scalar=1.0,
                        in1=dd[:, 0:C], op0=OP.add, op1=OP.mult)
                    nc.vector.scalar_tensor_tensor(
                        out=dd[:, 0:C], in0=uu[:, 0:C], scalar=0.5,
                        in1=nn[:, 0:C], op0=OP.mult, op1=OP.add)
                    outt = qpool.tile([128, NB * 128], f32, tag="outt")
                    nc.scalar.activation(outt[:, 0:C], dd[:, 0:C], AF.Relu)
                    nc.sync.dma_start(out_d[:, c0:c0 + C], outt[:, 0:C])

    nc.compile()
    return nc


def _prep(edge_logits, node_feats, W_proj, b_proj, w_ih, w_hh, b_ih, b_hh,
          src, dst):
    """Host-side sharding. Returns (T_win, sAp, sBp, in_maps).

    Windows are sorted by edge count and dealt round-robin to (core,
    position) so the 8 windows sharing a position have similar counts;
    slot counts are per-position (max over cores) instead of one global
    max -- cuts gather padding from ~19% to a few %.  _prep stashes the
    window assignment in module global _wassign for kernel() to invert.
    """
    global _wassign
    import ml_dtypes
    BF16 = ml_dtypes.bfloat16

    logits = np.asarray(edge_logits, np.float32).reshape(-1)
    src = np.asarray(src, np.int64)
    dst = np.asarray(dst, np.int64)

    is_b = (src >= S_SPLIT).astype(np.int64)
    win = dst // 128
    key = win * 2 + is_b
    order = np.argsort(key, kind="stable")
    key_s = key[order]
    src_s = src[order]
    dst_s = dst[order]
    log_s = logits[order]

    counts = np.bincount(key_s, minlength=WTOT * 2)
    cA = counts[0::2]
    cB = counts[1::2]

    # sorted round-robin window assignment: rank p -> (core p%NC, pos p//NC)
    worder = np.argsort(-(cA + cB), kind="stable")
    wassign = worder.reshape(WPC, NC).T          # [NC, WPC] window ids
    _wassign = wassign
    core_of = np.zeros(WTOT, np.int64)
    pos_of = np.zeros(WTOT, np.int64)
    core_of[worder] = np.arange(WTOT) % NC
    pos_of[worder] = np.arange(WTOT) // NC

    # per-position slot counts (max over the 8 cores at that position)
    cA_kp = cA[wassign]                          # [NC, WPC]
    cB_kp = cB[wassign]
    sAp = ((cA_kp.max(axis=0) + 127) // 128).astype(np.int64)
    sBp = ((cB_kp.max(axis=0) + 127) // 128).astype(np.int64)
    ofsA = np.concatenate([[0], np.cumsum(sAp)])
    ofsB = np.concatenate([[0], np.cumsum(sBp)])
    CAc = int(ofsA[-1])
    CBc = int(ofsB[-1])
    T_win = int(sAp.sum() + sBp.sum())
    sA_ret = tuple(int(x) for x in sAp)
    sB_ret = tuple(int(x) for x in sBp)

    starts = np.zeros(WTOT * 2, np.int64)
    starts[1:] = np.cumsum(counts)[:-1]
    pos = np.arange(E, dtype=np.int64) - starts[key_s]

    winv = key_s // 2
    grp = key_s % 2
    idxA = np.zeros(NC * CAc * 128, np.int16)
    idxB = np.zeros(NC * CBc * 128, np.int16)
    dstlA = np.full(NC * CAc * 128, -1.0, np.float32)
    dstlB = np.full(NC * CBc * 128, -1.0, np.float32)
    logA = np.zeros(NC * CAc * 128, np.float32)
    logB = np.zeros(NC * CBc * 128, np.float32)

    mA = grp == 0
    mB = ~mA
    wA = winv[mA]
    wB = winv[mB]
    flatA = (core_of[wA] * CAc + ofsA[pos_of[wA]]) * 128 + pos[mA]
    flatB = (core_of[wB] * CBc + ofsB[pos_of[wB]]) * 128 + pos[mB]
    idxA[flatA] = src_s[mA].astype(np.int16)
    idxB[flatB] = (src_s[mB] - OFF_B).astype(np.int16)
    dstlA[flatA] = (dst_s[mA] - wA * 128).astype(np.float32)
    dstlB[flatB] = (dst_s[mB] - wB * 128).astype(np.float32)
    logA[flatA] = log_s[mA]
    logB[flatB] = log_s[mB]

    def core_tiles(a, slots_tot):
        a = a.reshape(NC, slots_tot, 128)
        return [np.ascontiguousarray(a[k].T) for k in range(NC)]

    dstlA_cores = core_tiles(dstlA, CAc)
    dstlB_cores = core_tiles(dstlB, CBc)
    logA_cores = core_tiles(logA, CAc)
    logB_cores = core_tiles(logB, CBc)

    jj = np.arange(128, dtype=np.float32)

    def maskedlog_cores(dstl_cores, log_cores, slots_tot):
        out = []
        for d, lg in zip(dstl_cores, log_cores):
            o = np.where(d[:, :, None] == jj[None, None, :],
                         lg[:, :, None], np.float32(-100.0)).astype(BF16)
            out.append(np.ascontiguousarray(
                o.reshape(128, slots_tot * 128)))
        return out

    lmA_cores = maskedlog_cores(dstlA_cores, logA_cores, CAc)
    lmB_cores = maskedlog_cores(dstlB_cores, logB_cores, CBc)

    def core_idx(a, slots_tot):
        a = a.reshape(NC, slots_tot * 128)
        out = []
        for k in range(NC):
            blk = a[k].reshape(-1, 16).T      # [16, L/16], i -> [i%16,i//16]
            out.append(np.ascontiguousarray(np.tile(blk, (8, 1))))
        return out

    idxA_cores = core_idx(idxA, CAc)
    idxB_cores = core_idx(idxB, CBc)

    nf = np.asarray(node_feats, np.float32)
    nf16 = nf.astype(BF16)
    nf_pad = np.zeros((WTOT * 128, F), np.float32)
    nf_pad[:V] = nf

    tab16 = np.ascontiguousarray(nf16)
    tabb16 = np.ascontiguousarray(nf16[OFF_B:])
    wproj16 = np.ascontiguousarray(np.asarray(W_proj, np.float32).T
                                   .astype(BF16))
    wih16 = np.ascontiguousarray(np.asarray(w_ih, np.float32).T.astype(BF16))
    whh16 = np.ascontiguousarray(np.asarray(w_hh, np.float32).T.astype(BF16))
    bih = np.asarray(b_ih, np.float32).reshape(384)
    bhh = np.asarray(b_hh, np.float32).reshape(384)
    bprojc = np.asarray(b_proj, np.float32).reshape(128, 1)
    brh = (0.5 * (bih[0:128] + bhh[0:128])).reshape(128, 1)
    bzh = (0.5 * (bih[128:256] + bhh[128:256])).reshape(128, 1)
    bnic = bih[256:384].reshape(128, 1)
    bnhh = (0.5 * bhh[256:384]).reshape(128, 1)
    ident16 = np.eye(128, dtype=BF16)
    identf = np.eye(128, dtype=np.float32)
    onesc16 = np.ones((128, 1), BF16)

    nf_win = nf_pad.reshape(WTOT, 128, F)
    in_maps = []
    for k in range(NC):
        sl = nf_win[wassign[k]].reshape(NPC, F)   # core's windows, pos order
        nfT16 = np.ascontiguousarray(sl.T.astype(BF16))
        in_maps.append({
            "idxa": idxA_cores[k], "idxb": idxB_cores[k],
            "lma": lmA_cores[k], "lmb": lmB_cores[k],
            "tab16": tab16, "tabb16": tabb16,
            "nfT16": nfT16,
            "wproj16": wproj16, "wih16": wih16, "whh16": whh16,
            "bprojc": bprojc, "brh": brh, "bzh": bzh,
            "bnic": bnic, "bnhh": bnhh,
            "ident16": ident16, "identf": identf, "onesc16": onesc16,
        })
    return T_win, sA_ret, sB_ret, in_maps


def kernel(edge_logits, node_feats, W_proj, b_proj, w_ih, w_hh, b_ih, b_hh,
           src, dst):
    from concourse.bass_utils import run_bass_kernel_spmd

    T_win, sA, sB, in_maps = _prep(edge_logits, node_feats, W_proj, b_proj,
                                   w_ih, w_hh, b_ih, b_hh, src, dst)
    key = (T_win, sA, sB)
    if key not in _compiled:
        _compiled[key] = _build_nc(T_win, sA=sA, sB=sB)
    nc = _compiled[key]

    res = run_bass_kernel_spmd(nc, in_maps, list(range(NC)))
    full = np.zeros((WTOT * 128, F), np.float32)
    for k in range(NC):
        ok = np.ascontiguousarray(res.results[k]["out"]).T  # [NPC, F]
        full[(_wassign[k][:, None] * 128
              + np.arange(128)[None, :]).reshape(-1)] = ok
    return np.ascontiguousarray(full[:V]).astype(np.float32)


# revision 50
# speedup vs baseline: 13.0249x; 2.2608x over previous
"""AttentiveGRU2 Trainium2 Bass kernel (v2).

Model (see reference):
  edge-softmax over incoming edges per dst node, attention-weighted
  gather of projected node features, segment-sum per dst, ELU, GRUCell.

Strategy (8 NeuronCores, SPMD, no collectives):
  * Host sorts edges by dst window (392 windows of 128 node ids; 49
    windows per core). Softmax folded through the segment sum:
    ctx_v = (sum_e ex_e nf[src_e]) / (sum_e ex_e); proj applied after.
  * Gather: node-feature table in bf16 (256B rows), hardware
    InstDMAGatherAnt across 4 SWDGE queues (the per-queue descriptor
    rate ~8ns/desc is the kernel bottleneck; 4 queues x bf16 measured
    ~4x faster than the fp32 single-queue baseline). int16 idx limit
    handled with two overlapping row views (A: src<32768, B: src-17232).
  * One-hot dst matrices are 0/1 bf16 built on host and streamed in via
    regular DMA (cheap sequential traffic) -- the only on-device
    elementwise edge work is O_s = O01 * ex (split DVE/Pool engines).
  * Edge matmuls per 128-edge slot tile (bf16, 1 cyc/row):
      psum[v,0:128] += O_s^T @ G_raw,  psum[v,128] += O_s^T @ ones.
  * Node phase in [channel, node] layout, weights stationary, batched
    over NB=4 windows: ctx scaled by 1/den -> bf16 -> PE transpose ->
    proj + ELU -> GRU gates. Sigmoid avoided via 0.5*tanh(x/2)+0.5 so
    every activation (Exp/Tanh/Relu) lives in one table: zero 1283ns
    act-table reloads. Biases folded into activation bias APs.
  * Output written [feat, node]; host transposes back.
"""

import numpy as np

V, E, F = 50000, 800000, 128
NC = 8
WPC = 49              # windows per core
NPC = WPC * 128       # 6272 node slots per core
WTOT = NC * WPC       # 392 windows total
WPB = 2               # windows per gather batch
NB = 4                # windows per node-phase group
S_SPLIT = 32768       # src < S -> table A
OFF_B = V - 32768     # 17232; table B rows [OFF_B, V)

_compiled = {}
_wassign = None   # [NC, WPC] window assignment from the last _prep


def _build_nc(T_win, sA=None, sB=None, skip_gather=False, skip_onehot=False,
              skip_mm=False, skip_node=False, repeat=1, one_act=False,
              nq_use=4, den_sep=True, tr_f32=False, den_seq=False,
              dump=None, ud1=False, gbufs=4):
    import concourse.bass as bass
    import concourse.bacc as bacc
    import concourse.mybir as mybir
    import concourse.tile as tile

    f32 = mybir.dt.float32
    bf16 = mybir.dt.bfloat16
    i16 = mybir.dt.int16
    AF = mybir.ActivationFunctionType
    OP = mybir.AluOpType

    # sA/sB: per-position slot-count lists (scalars = uniform legacy)
    sAp = list(sA) if not isinstance(sA, int) else [sA] * WPC
    sBp = list(sB) if not isinstance(sB, int) else [sB] * WPC
    ofsA = np.concatenate([[0], np.cumsum(sAp)]).astype(int)
    ofsB = np.concatenate([[0], np.cumsum(sBp)]).astype(int)
    CA = int(ofsA[-1])      # A slot-tiles per core
    CB = int(ofsB[-1])
    LA = CA * 128           # A-gather idx count per core
    LB = CB * 128
    n_batches = (WPC + WPB - 1) // WPB
    mxA = max(int(ofsA[min(b * WPB + WPB, WPC)] - ofsA[b * WPB])
              for b in range(n_batches))
    mxB = max(int(ofsB[min(b * WPB + WPB, WPC)] - ofsB[b * WPB])
              for b in range(n_batches))

    nc = bacc.Bacc("TRN2", target_bir_lowering=False, debug=False,
                   num_devices=NC, num_swdge_queues=4)

    # ---- DRAM parameters ----
    idxa_d = nc.dram_tensor("idxa", [128, LA // 16], i16, kind="ExternalInput")
    idxb_d = nc.dram_tensor("idxb", [128, LB // 16], i16, kind="ExternalInput")
    # masked logits: logit value at the one-hot position, -100 elsewhere;
    # one Exp on the Activation engine turns a tile into the scaled one-hot
    lma_d = nc.dram_tensor("lma", [128, CA * 128], bf16,
                           kind="ExternalInput")
    lmb_d = nc.dram_tensor("lmb", [128, CB * 128], bf16,
                           kind="ExternalInput")
    tab16_d = nc.dram_tensor("tab16", [V, F], bf16, kind="ExternalInput")
    tabb16_d = nc.dram_tensor("tabb16", [32768, F], bf16,
                              kind="ExternalInput")
    nfT16_d = nc.dram_tensor("nfT16", [128, NPC], bf16, kind="ExternalInput")
    wproj16_d = nc.dram_tensor("wproj16", [128, 128], bf16,
                               kind="ExternalInput")
    wih16_d = nc.dram_tensor("wih16", [128, 384], bf16, kind="ExternalInput")
    whh16_d = nc.dram_tensor("whh16", [128, 384], bf16, kind="ExternalInput")
    bproj_d = nc.dram_tensor("bprojc", [128, 1], f32, kind="ExternalInput")
    brh_d = nc.dram_tensor("brh", [128, 1], f32, kind="ExternalInput")
    bzh_d = nc.dram_tensor("bzh", [128, 1], f32, kind="ExternalInput")
    bni_d = nc.dram_tensor("bnic", [128, 1], f32, kind="ExternalInput")
    bnhh_d = nc.dram_tensor("bnhh", [128, 1], f32, kind="ExternalInput")
    ident16_d = nc.dram_tensor("ident16", [128, 128], bf16,
                               kind="ExternalInput")
    identf_d = nc.dram_tensor("identf", [128, 128], f32,
                              kind="ExternalInput")
    onesc16_d = nc.dram_tensor("onesc16", [128, 1], bf16,
                               kind="ExternalInput")
    out_d = nc.dram_tensor("out", [128, NPC], f32, kind="ExternalOutput")
    if dump == "g":
        dump_d = nc.dram_tensor("dmp", [128, CA * 128], bf16,
                                kind="ExternalOutput")
    elif dump:
        dump_d = nc.dram_tensor("dmp", [NPC, 128], f32,
                                kind="ExternalOutput")

    tabA = tab16_d[0:32768, :]
    tabB = tabb16_d[:]

    def apx(base, dims):
        return bass.AP(base.tensor, base.offset,
                       [list(base.ap[0])] + dims)

    with tile.TileContext(nc) as tc:
        with (
            tc.tile_pool(name="const", bufs=1) as cpool,
            tc.tile_pool(name="gat", bufs=gbufs) as gpool,
            tc.tile_pool(name="oh", bufs=gbufs) as opool,
            tc.tile_pool(name="wrk", bufs=1) as wpool,
            tc.tile_pool(name="brdg", bufs=2) as bpool,
            tc.tile_pool(name="outp", bufs=2) as qpool,
            tc.tile_pool(name="pedge", bufs=1, space="PSUM") as pe_pool,
            tc.tile_pool(name="ptr", bufs=1, space="PSUM") as ptr_pool,
            tc.tile_pool(name="pnode", bufs=1, space="PSUM") as pn_pool,
        ):
            def load(pool, name, dram, shape, dtype=f32):
                t = pool.tile(shape, dtype, tag=name)
                nc.sync.dma_start(t[:], dram[:])
                return t

            ident_sb = load(cpool, "ident16", ident16_d, [128, 128], bf16)
            identf_sb = load(cpool, "identf", identf_d, [128, 128], f32)
            onesc_sb = load(cpool, "onesc16", onesc16_d, [128, 1], bf16)
            wproj_sb = load(cpool, "wproj16", wproj16_d, [128, 128], bf16)
            wih_sb = load(cpool, "wih16", wih16_d, [128, 384], bf16)
            whh_sb = load(cpool, "whh16", whh16_d, [128, 384], bf16)
            bproj_sb = load(cpool, "bprojc", bproj_d, [128, 1])
            brh_sb = load(cpool, "brh", brh_d, [128, 1])
            bzh_sb = load(cpool, "bzh", bzh_d, [128, 1])
            bni_sb = load(cpool, "bnic", bni_d, [128, 1])
            bnhh_sb = load(cpool, "bnhh", bnhh_d, [128, 1])
            idxa_sb = load(cpool, "idxa", idxa_d, [128, LA // 16], i16)
            idxb_sb = load(cpool, "idxb", idxb_d, [128, LB // 16], i16)
            nfT16_sb = load(cpool, "nfT16", nfT16_d, [128, NPC], bf16)

            GA_static = GB_static = None
            if skip_gather:
                GA_static = cpool.tile([128, mxA, 128], bf16, tag="GAs")
                nc.gpsimd.memset(GA_static[:], 0.0)
                GB_static = cpool.tile([128, mxB, 128], bf16, tag="GBs")
                nc.gpsimd.memset(GB_static[:], 0.0)
            OA_static = OB_static = None
            if skip_onehot:
                OA_static = cpool.tile([128, mxA, 128], bf16, tag="OAs")
                nc.gpsimd.memset(OA_static[:], 0.0)
                OB_static = cpool.tile([128, mxB, 128], bf16, tag="OBs")
                nc.gpsimd.memset(OB_static[:], 0.0)

            qload = [0] * nq_use   # greedy per-queue descriptor balancing
            for _rep in range(repeat):
              # node-group state: transpose psum + sbuf ctxT for NB windows
              for b in range(n_batches):
                w0 = b * WPB
                nw = min(WPB, WPC - w0)
                cA0, cB0 = int(ofsA[w0]), int(ofsB[w0])
                ntA = int(ofsA[w0 + nw]) - cA0
                ntB = int(ofsB[w0 + nw]) - cB0
                if skip_gather:
                    GA, GB = GA_static, GB_static
                else:
                    GA = gpool.tile([128, mxA, 128], bf16, tag="GA")
                    GB = gpool.tile([128, mxB, 128], bf16, tag="GB")
                    na = ntA * 128
                    qn = qload.index(min(qload))
                    qload[qn] += na
                    nc.gpsimd.dma_gather(
                        out_ap=GA[:, 0:ntA, :], in_ap=tabA,
                        idxs_ap=idxa_sb[:, (cA0 * 128) // 16:
                                        ((cA0 + ntA) * 128) // 16],
                        num_idxs=na, num_idxs_reg=na, elem_size=128,
                        single_packet=False, queue_num=qn,
                    )
                    nb_ = ntB * 128
                    qn = qload.index(min(qload))
                    qload[qn] += nb_
                    nc.gpsimd.dma_gather(
                        out_ap=GB[:, 0:ntB, :], in_ap=tabB,
                        idxs_ap=idxb_sb[:, (cB0 * 128) // 16:
                                        ((cB0 + ntB) * 128) // 16],
                        num_idxs=nb_, num_idxs_reg=nb_, elem_size=128,
                        single_packet=False, queue_num=qn,
                    )
                if dump == "g" and not skip_gather:
                    nc.sync.dma_start(
                        dump_d[:, cA0 * 128:(cA0 + ntA) * 128],
                        GA[:, 0:ntA, :])
                if skip_onehot:
                    OA, OB = OA_static, OB_static
                else:
                    OA = opool.tile([128, mxA, 128], bf16, tag="OA")
                    OB = opool.tile([128, mxB, 128], bf16, tag="OB")
                    nc.sync.dma_start(
                        OA[:, 0:ntA, :],
                        lma_d[:, cA0 * 128:(cA0 + ntA) * 128])
                    nc.sync.dma_start(
                        OB[:, 0:ntB, :],
                        lmb_d[:, cB0 * 128:(cB0 + ntB) * 128])
                    # O_s = exp(masked logits): the scaled one-hot in one
                    # Activation-engine op (exp(-100) == 0)
                    nc.scalar.activation(OA[:, 0:ntA, :], OA[:, 0:ntA, :],
                                         AF.Exp)
                    nc.scalar.activation(OB[:, 0:ntB, :], OB[:, 0:ntB, :],
                                         AF.Exp)

                for wl in range(nw):
                    w = w0 + wl
                    sAw, sBw = sAp[w], sBp[w]
                    SW = sAw + sBw
                    tA0 = int(ofsA[w]) - cA0    # window tile base in batch
                    tB0 = int(ofsB[w]) - cB0
                    psum_ud = pe_pool.tile([128, 132], f32, tag="psum_ud",
                                           bufs=1 if ud1 else 2)
                    if den_sep:
                        # den accumulates in its own PSUM bank: a second
                        # concurrently-open matmul group in the same 2KB
                        # zero region corrupts the first (hw start_tensor_
                        # calc marks the whole region pending-zero).
                        psum_dn = pe_pool.tile([128, 4], f32, tag="psum_dn",
                                               bufs=2)
                        den_ap = psum_dn[:, 0:1]
                    else:
                        den_ap = psum_ud[:, 128:129]
                    if not skip_mm:
                        def olh_grh(s_):
                            if s_ < sAw:
                                return (OA[:, tA0 + s_, :],
                                        GA[:, tA0 + s_, :])
                            return (OB[:, tB0 + (s_ - sAw), :],
                                    GB[:, tB0 + (s_ - sAw), :])
                        if den_seq:
                            for s_ in range(SW):
                                Olh, Grh = olh_grh(s_)
                                nc.tensor.matmul(
                                    psum_ud[:, 0:128], lhsT=Olh, rhs=Grh,
                                    start=(s_ == 0), stop=(s_ == SW - 1))
                            for s_ in range(SW):
                                Olh, _ = olh_grh(s_)
                                nc.tensor.matmul(
                                    den_ap, lhsT=Olh, rhs=onesc_sb[:],
                                    start=(s_ == 0), stop=(s_ == SW - 1))
                        else:
                            for s_ in range(SW):
                                Olh, Grh = olh_grh(s_)
                                nc.tensor.matmul(
                                    psum_ud[:, 0:128], lhsT=Olh, rhs=Grh,
                                    start=(s_ == 0), stop=(s_ == SW - 1))
                                nc.tensor.matmul(
                                    den_ap, lhsT=Olh,
                                    rhs=onesc_sb[:],
                                    start=(s_ == 0), stop=(s_ == SW - 1))

                    if skip_node:
                        continue

                    # ---- bridge: ctx = psum/den -> bf16 -> [feat, node] ----
                    g = w // NB
                    gl = w % NB
                    nwin = min(NB, WPC - g * NB)
                    trdt = f32 if tr_f32 else bf16
                    if gl == 0:
                        psum_tr = ptr_pool.tile([128, NB * 128], trdt,
                                                tag="psum_tr")
                        cur_tr = psum_tr
                    den = bpool.tile([128, 1], f32, tag="den")
                    nc.vector.tensor_scalar(
                        out=den[:], in0=den_ap, scalar1=1e-30,
                        scalar2=None, op0=OP.max)
                    rec = bpool.tile([128, 1], f32, tag="rec")
                    nc.vector.reciprocal(rec[:], den[:])
                    ctx16 = bpool.tile([128, 128], trdt, tag="ctx16")
                    nc.vector.tensor_scalar(
                        out=ctx16[:], in0=psum_ud[:, 0:128],
                        scalar1=rec[:, 0:1], scalar2=None, op0=OP.mult)
                    if dump == "ctx":
                        cdump = bpool.tile([128, 128], f32, tag="cdump")
                        nc.vector.tensor_scalar(
                            out=cdump[:], in0=psum_ud[:, 0:128],
                            scalar1=rec[:, 0:1], scalar2=None, op0=OP.mult)
                        nc.sync.dma_start(
                            dump_d[w * 128:(w + 1) * 128, :], cdump[:])
                    elif dump == "den":
                        cdump = bpool.tile([128, 128], f32, tag="cdump")
                        nc.vector.tensor_scalar(
                            out=cdump[:, 0:1], in0=den_ap,
                            scalar1=1.0, scalar2=None, op0=OP.mult)
                        nc.vector.tensor_copy(out=cdump[:, 1:128],
                                              in_=psum_ud[:, 1:128])
                        nc.sync.dma_start(
                            dump_d[w * 128:(w + 1) * 128, :], cdump[:])
                    nc.tensor.transpose(
                        cur_tr[:, gl * 128:(gl + 1) * 128], ctx16[:],
                        ident_sb[:] if not tr_f32 else identf_sb[:])

                    if gl != nwin - 1:
                        continue

                    # ---- node phase for group g: windows [g*NB, g*NB+nwin)
                    C = nwin * 128
                    c0 = g * NB * 128
                    ctxT = bpool.tile([128, NB * 128], bf16, tag="ctxT")
                    nc.vector.tensor_copy(out=ctxT[:, 0:C],
                                          in_=cur_tr[:, 0:C])

                    # proj + ELU -> ctxT2 (bf16)
                    psum_cT = pn_pool.tile([128, NB * 128], f32,
                                           tag="psum_cT")
                    nc.tensor.matmul(psum_cT[:, 0:C], lhsT=wproj_sb[:],
                                     rhs=ctxT[:, 0:C], start=True, stop=True)
                    eA = wpool.tile([128, NB * 128], f32, tag="eA")
                    nc.vector.tensor_scalar(
                        out=eA[:, 0:C], in0=psum_cT[:, 0:C],
                        scalar1=bproj_sb[:, 0:1], scalar2=0.0,
                        op0=OP.add, op1=OP.min)
                    nc.scalar.activation(eA[:, 0:C], eA[:, 0:C], AF.Exp)
                    eB = wpool.tile([128, NB * 128], f32, tag="eB")
                    nc.vector.tensor_scalar(
                        out=eB[:, 0:C], in0=psum_cT[:, 0:C],
                        scalar1=bproj_sb[:, 0:1], scalar2=0.0,
                        op0=OP.add, op1=OP.max)
                    # elu = (eA - 1) + eB, fused on DVE
                    ctxT2 = wpool.tile([128, NB * 128], bf16, tag="ctxT2")
                    nc.vector.scalar_tensor_tensor(
                        out=ctxT2[:, 0:C], in0=eA[:, 0:C], scalar=1.0,
                        in1=eB[:, 0:C], op0=OP.subtract, op1=OP.add)

                    # GRU gates, [gate, node] layout, weights stationary.
                    # r/n share a PSUM bank sequentially, h/z share another
                    # (group lifetimes don't overlap; WAR via tile reuse).
                    nf16c = nfT16_sb[:, c0:c0 + C]
                    psum_rn = pn_pool.tile([128, NB * 128], f32,
                                           tag="psum_rn")
                    nc.tensor.matmul(psum_rn[:, 0:C], lhsT=wih_sb[:, 0:128],
                                     rhs=ctxT2[:, 0:C], start=True,
                                     stop=False)
                    nc.tensor.matmul(psum_rn[:, 0:C], lhsT=whh_sb[:, 0:128],
                                     rhs=nf16c, start=False, stop=True)
                    psum_hz = pn_pool.tile([128, NB * 128], f32,
                                           tag="psum_hz")
                    nc.tensor.matmul(psum_hz[:, 0:C],
                                     lhsT=whh_sb[:, 256:384],
                                     rhs=nf16c, start=True, stop=True)

                    # r = sigmoid(s) = 0.5*tanh(0.5 s + 0.5 b_r) + 0.5
                    tr_ = wpool.tile([128, NB * 128], f32, tag="tr_")
                    nc.scalar.activation(tr_[:, 0:C], psum_rn[:, 0:C],
                                         AF.Tanh, bias=brh_sb[:, 0:1],
                                         scale=0.5)
                    # hnb2 = 0.5 h_n + 0.5 b_hn
                    hnb2 = wpool.tile([128, NB * 128], f32, tag="hnb2")
                    nc.vector.tensor_scalar(
                        out=hnb2[:, 0:C], in0=psum_hz[:, 0:C], scalar1=0.5,
                        scalar2=bnhh_sb[:, 0:1], op0=OP.mult, op1=OP.add)

                    # n-gate reuses the r bank; z reuses the h bank
                    nc.tensor.matmul(psum_rn[:, 0:C],
                                     lhsT=wih_sb[:, 256:384],
                                     rhs=ctxT2[:, 0:C], start=True, stop=True)
                    nc.tensor.matmul(psum_hz[:, 0:C],
                                     lhsT=wih_sb[:, 128:256],
                                     rhs=ctxT2[:, 0:C], start=True,
                                     stop=False)
                    nc.tensor.matmul(psum_hz[:, 0:C],
                                     lhsT=whh_sb[:, 128:256],
                                     rhs=nf16c, start=False, stop=True)

                    # n = tanh(i_n + b_in + (tr+1)*hnb2)
                    qq = wpool.tile([128, NB * 128], f32, tag="qq")
                    nc.vector.scalar_tensor_tensor(
                        out=qq[:, 0:C], in0=tr_[:, 0:C], scalar=1.0,
                        in1=hnb2[:, 0:C], op0=OP.add, op1=OP.mult)
                    nc.vector.tensor_tensor(out=qq[:, 0:C], in0=qq[:, 0:C],
                                            in1=psum_rn[:, 0:C], op=OP.add)
                    nn = wpool.tile([128, NB * 128], f32, tag="nn")
                    nc.scalar.activation(nn[:, 0:C], qq[:, 0:C], AF.Tanh,
                                         bias=bni_sb[:, 0:1])
                    tz_ = wpool.tile([128, NB * 128], f32, tag="tz_")
                    nc.scalar.activation(tz_[:, 0:C], psum_hz[:, 0:C],
                                         AF.Tanh, bias=bzh_sb[:, 0:1],
                                         scale=0.5)
                    # h = n + (0.5 tz + 0.5)*(nf - n)
                    dd = wpool.tile([128, NB * 128], f32, tag="dd")
                    nc.vector.tensor_tensor(out=dd[:, 0:C], in0=nf16c,
                                            in1=nn[:, 0:C], op=OP.subtract)
                    uu = wpool.tile([128, NB * 128], f32, tag="uu")
                    nc.vector.scalar_tensor_tensor(
                        out=uu[:, 0:C], in0=tz_[:, 0:C], scalar=1.0,
                        in1=dd[:, 0:C], op0=OP.add, op1=OP.mult)
                    nc.vector.scalar_tensor_tensor(
                        out=dd[:, 0:C], in0=uu[:, 0:C], scalar=0.5,
                        in1=nn[:, 0:C], op0=OP.mult, op1=OP.add)
                    outt = qpool.tile([128, NB * 128], f32, tag="outt")
                    nc.scalar.activation(outt[:, 0:C], dd[:, 0:C], AF.Relu)
                    nc.sync.dma_start(out_d[:, c0:c0 + C], outt[:, 0:C])

    nc.compile()
    return nc


def _prep(edge_logits, node_feats, W_proj, b_proj, w_ih, w_hh, b_ih, b_hh,
          src, dst):
    """Host-side sharding. Returns (T_win, sAp, sBp, in_maps).

    Windows are sorted by edge count and dealt round-robin to (core,
    position) so the 8 windows sharing a position have similar counts;
    slot counts are per-position (max over cores) instead of one global
    max -- cuts gather padding from ~19% to a few %.  _prep stashes the
    window assignment in module global _wassign for kernel() to invert.
    """
    global _wassign
    import ml_dtypes
    BF16 = ml_dtypes.bfloat16

    logits = np.asarray(edge_logits, np.float32).reshape(-1)
    src = np.asarray(src, np.int64)
    dst = np.asarray(dst, np.int64)

    is_b = (src >= S_SPLIT).astype(np.int64)
    win = dst // 128
    key = win * 2 + is_b
    order = np.argsort(key, kind="stable")
    key_s = key[order]
    src_s = src[order]
    dst_s = dst[order]
    log_s = logits[order]

    counts = np.bincount(key_s, minlength=WTOT * 2)
    cA = counts[0::2]
    cB = counts[1::2]

    # sorted round-robin window assignment: rank p -> (core p%NC, pos p//NC)
    worder = np.argsort(-(cA + cB), kind="stable")
    wassign = worder.reshape(WPC, NC).T          # [NC, WPC] window ids
    _wassign = wassign
    core_of = np.zeros(WTOT, np.int64)
    pos_of = np.zeros(WTOT, np.int64)
    core_of[worder] = np.arange(WTOT) % NC
    pos_of[worder] = np.arange(WTOT) // NC

    # per-position slot counts (max over the 8 cores at that position)
    cA_kp = cA[wassign]                          # [NC, WPC]
    cB_kp = cB[wassign]
    sAp = ((cA_kp.max(axis=0) + 127) // 128).astype(np.int64)
    sBp = ((cB_kp.max(axis=0) + 127) // 128).astype(np.int64)
    ofsA = np.concatenate([[0], np.cumsum(sAp)])
    ofsB = np.concatenate([[0], np.cumsum(sBp)])
    CAc = int(ofsA[-1])
    CBc = int(ofsB[-1])
    T_win = int(sAp.sum() + sBp.sum())
    sA_ret = tuple(int(x) for x in sAp)
    sB_ret = tuple(int(x) for x in sBp)

    starts = np.zeros(WTOT * 2, np.int64)
    starts[1:] = np.cumsum(counts)[:-1]
    pos = np.arange(E, dtype=np.int64) - starts[key_s]

    winv = key_s // 2
    grp = key_s % 2
    idxA = np.zeros(NC * CAc * 128, np.int16)
    idxB = np.zeros(NC * CBc * 128, np.int16)
    dstlA = np.full(NC * CAc * 128, -1.0, np.float32)
    dstlB = np.full(NC * CBc * 128, -1.0, np.float32)
    logA = np.zeros(NC * CAc * 128, np.float32)
    logB = np.zeros(NC * CBc * 128, np.float32)

    mA = grp == 0
    mB = ~mA
    wA = winv[mA]
    wB = winv[mB]
    flatA = (core_of[wA] * CAc + ofsA[pos_of[wA]]) * 128 + pos[mA]
    flatB = (core_of[wB] * CBc + ofsB[pos_of[wB]]) * 128 + pos[mB]
    idxA[flatA] = src_s[mA].astype(np.int16)
    idxB[flatB] = (src_s[mB] - OFF_B).astype(np.int16)
    dstlA[flatA] = (dst_s[mA] - wA * 128).astype(np.float32)
    dstlB[flatB] = (dst_s[mB] - wB * 128).astype(np.float32)
    logA[flatA] = log_s[mA]
    logB[flatB] = log_s[mB]

    def core_tiles(a, slots_tot):
        a = a.reshape(NC, slots_tot, 128)
        return [np.ascontiguousarray(a[k].T) for k in range(NC)]

    dstlA_cores = core_tiles(dstlA, CAc)
    dstlB_cores = core_tiles(dstlB, CBc)
    logA_cores = core_tiles(logA, CAc)
    logB_cores = core_tiles(logB, CBc)

    jj = np.arange(128, dtype=np.float32)

    def maskedlog_cores(dstl_cores, log_cores, slots_tot):
        out = []
        for d, lg in zip(dstl_cores, log_cores):
            o = np.where(d[:, :, None] == jj[None, None, :],
                         lg[:, :, None], np.float32(-100.0)).astype(BF16)
            out.append(np.ascontiguousarray(
                o.reshape(128, slots_tot * 128)))
        return out

    lmA_cores = maskedlog_cores(dstlA_cores, logA_cores, CAc)
    lmB_cores = maskedlog_cores(dstlB_cores, logB_cores, CBc)

    def core_idx(a, slots_tot):
        a = a.reshape(NC, slots_tot * 128)
        out = []
        for k in range(NC):
            blk = a[k].reshape(-1, 16).T      # [16, L/16], i -> [i%16,i//16]
            out.append(np.ascontiguousarray(np.tile(blk, (8, 1))))
        return out

    idxA_cores = core_idx(idxA, CAc)
    idxB_cores = core_idx(idxB, CBc)

    nf = np.asarray(node_feats, np.float32)
    nf16 = nf.astype(BF16)
    nf_pad = np.zeros((WTOT * 128, F), np.float32)
    nf_pad[:V] = nf

    tab16 = np.ascontiguousarray(nf16)
    tabb16 = np.ascontiguousarray(nf16[OFF_B:])
    wproj16 = np.ascontiguousarray(np.asarray(W_proj, np.float32).T
                                   .astype(BF16))
    wih16 = np.ascontiguousarray(np.asarray(w_ih, np.float32).T.astype(BF16))
    whh16 = np.ascontiguousarray(np.asarray(w_hh, np.float32).T.astype(BF16))
    bih = np.asarray(b_ih, np.float32).reshape(384)
    bhh = np.asarray(b_hh, np.float32).reshape(384)
    bprojc = np.asarray(b_proj, np.float32).reshape(128, 1)
    brh = (0.5 * (bih[0:128] + bhh[0:128])).reshape(128, 1)
    bzh = (0.5 * (bih[128:256] + bhh[128:256])).reshape(128, 1)
    bnic = bih[256:384].reshape(128, 1)
    bnhh = (0.5 * bhh[256:384]).reshape(128, 1)
    ident16 = np.eye(128, dtype=BF16)
    identf = np.eye(128, dtype=np.float32)
    onesc16 = np.ones((128, 1), BF16)

    nf_win = nf_pad.reshape(WTOT, 128, F)
    in_maps = []
    for k in range(NC):
        sl = nf_win[wassign[k]].reshape(NPC, F)   # core's windows, pos order
        nfT16 = np.ascontiguousarray(sl.T.astype(BF16))
        in_maps.append({
            "idxa": idxA_cores[k], "idxb": idxB_cores[k],
            "lma": lmA_cores[k], "lmb": lmB_cores[k],
            "tab16": tab16, "tabb16": tabb16,
            "nfT16": nfT16,
            "wproj16": wproj16, "wih16": wih16, "whh16": whh16,
            "bprojc": bprojc, "brh": brh, "bzh": bzh,
            "bnic": bnic, "bnhh": bnhh,
            "ident16": ident16, "identf": identf, "onesc16": onesc16,
        })
    return T_win, sA_ret, sB_ret, in_maps


def kernel(edge_logits, node_feats, W_proj, b_proj, w_ih, w_hh, b_ih, b_hh,
           src, dst):
    from concourse.bass_utils import run_bass_kernel_spmd

    T_win, sA, sB, in_maps = _prep(edge_logits, node_feats, W_proj, b_proj,
                                   w_ih, w_hh, b_ih, b_hh, src, dst)
    key = (T_win, sA, sB)
    if key not in _compiled:
        _compiled[key] = _build_nc(T_win, sA=sA, sB=sB)
    nc = _compiled[key]

    res = run_bass_kernel_spmd(nc, in_maps, list(range(NC)))
    full = np.zeros((WTOT * 128, F), np.float32)
    for k in range(NC):
        ok = np.ascontiguousarray(res.results[k]["out"]).T  # [NPC, F]
        full[(_wassign[k][:, None] * 128
              + np.arange(128)[None, :]).reshape(-1)] = ok
    return np.ascontiguousarray(full[:V]).astype(np.float32)


# revision 52
# speedup vs baseline: 22.1700x; 1.7021x over previous
"""AttentiveGRU2 Trainium2 Bass kernel (v2).

Model (see reference):
  edge-softmax over incoming edges per dst node, attention-weighted
  gather of projected node features, segment-sum per dst, ELU, GRUCell.

Strategy (8 NeuronCores, SPMD, no collectives):
  * Host sorts edges by dst window (392 windows of 128 node ids; 49
    windows per core). Softmax folded through the segment sum:
    ctx_v = (sum_e ex_e nf[src_e]) / (sum_e ex_e); proj applied after.
  * Gather: node-feature table in bf16 (256B rows), hardware
    InstDMAGatherAnt across 4 SWDGE queues (the per-queue descriptor
    rate ~8ns/desc is the kernel bottleneck; 4 queues x bf16 measured
    ~4x faster than the fp32 single-queue baseline). int16 idx limit
    handled with two overlapping row views (A: src<32768, B: src-17232).
  * One-hot dst matrices are 0/1 bf16 built on host and streamed in via
    regular DMA (cheap sequential traffic) -- the only on-device
    elementwise edge work is O_s = O01 * ex (split DVE/Pool engines).
  * Edge matmuls per 128-edge slot tile (bf16, 1 cyc/row):
      psum[v,0:128] += O_s^T @ G_raw,  psum[v,128] += O_s^T @ ones.
  * Node phase in [channel, node] layout, weights stationary, batched
    over NB=4 windows: ctx scaled by 1/den -> bf16 -> PE transpose ->
    proj + ELU -> GRU gates. Sigmoid avoided via 0.5*tanh(x/2)+0.5 so
    every activation (Exp/Tanh/Relu) lives in one table: zero 1283ns
    act-table reloads. Biases folded into activation bias APs.
  * Output written [feat, node]; host transposes back.
"""

import numpy as np

V, E, F = 50000, 800000, 128
NC = 8
WPC = 49              # windows per core
NPC = WPC * 128       # 6272 node slots per core
WTOT = NC * WPC       # 392 windows total
WPB = 2               # windows per gather batch
NB = 4                # windows per node-phase group
S_SPLIT = 32768       # src < S -> table A
OFF_B = V - 32768     # 17232; table B rows [OFF_B, V)

_compiled = {}
_wassign = None   # [NC, WPC] window assignment from the last _prep


def _build_nc(T_win, sA=None, sB=None, skip_gather=False, skip_onehot=False,
              skip_mm=False, skip_node=False, repeat=1, one_act=False,
              nq_use=4, den_sep=True, tr_f32=False, den_seq=False,
              dump=None, ud1=False, gbufs=6):
    import concourse.bass as bass
    import concourse.bacc as bacc
    import concourse.mybir as mybir
    import concourse.tile as tile

    f32 = mybir.dt.float32
    bf16 = mybir.dt.bfloat16
    i16 = mybir.dt.int16
    AF = mybir.ActivationFunctionType
    OP = mybir.AluOpType

    # sA/sB: per-position slot-count lists (scalars = uniform legacy)
    sAp = list(sA) if not isinstance(sA, int) else [sA] * WPC
    sBp = list(sB) if not isinstance(sB, int) else [sB] * WPC
    ofsA = np.concatenate([[0], np.cumsum(sAp)]).astype(int)
    ofsB = np.concatenate([[0], np.cumsum(sBp)]).astype(int)
    CA = int(ofsA[-1])      # A slot-tiles per core
    CB = int(ofsB[-1])
    LA = CA * 128           # A-gather idx count per core
    LB = CB * 128
    n_batches = (WPC + WPB - 1) // WPB
    mxA = max(int(ofsA[min(b * WPB + WPB, WPC)] - ofsA[b * WPB])
              for b in range(n_batches))
    mxB = max(int(ofsB[min(b * WPB + WPB, WPC)] - ofsB[b * WPB])
              for b in range(n_batches))

    nc = bacc.Bacc("TRN2", target_bir_lowering=False, debug=False,
                   num_devices=NC, num_swdge_queues=4)

    # ---- DRAM parameters ----
    idxa_d = nc.dram_tensor("idxa", [128, LA // 16], i16, kind="ExternalInput")
    idxb_d = nc.dram_tensor("idxb", [128, LB // 16], i16, kind="ExternalInput")
    # masked logits: logit value at the one-hot position, -100 elsewhere;
    # one Exp on the Activation engine turns a tile into the scaled one-hot
    lma_d = nc.dram_tensor("lma", [128, CA * 128], bf16,
                           kind="ExternalInput")
    lmb_d = nc.dram_tensor("lmb", [128, CB * 128], bf16,
                           kind="ExternalInput")
    tab16_d = nc.dram_tensor("tab16", [V, F], bf16, kind="ExternalInput")
    tabb16_d = nc.dram_tensor("tabb16", [32768, F], bf16,
                              kind="ExternalInput")
    nfT16_d = nc.dram_tensor("nfT16", [128, NPC], bf16, kind="ExternalInput")
    wproj16_d = nc.dram_tensor("wproj16", [128, 128], bf16,
                               kind="ExternalInput")
    wih16_d = nc.dram_tensor("wih16", [128, 384], bf16, kind="ExternalInput")
    whh16_d = nc.dram_tensor("whh16", [128, 384], bf16, kind="ExternalInput")
    bproj_d = nc.dram_tensor("bprojc", [128, 1], f32, kind="ExternalInput")
    brh_d = nc.dram_tensor("brh", [128, 1], f32, kind="ExternalInput")
    bzh_d = nc.dram_tensor("bzh", [128, 1], f32, kind="ExternalInput")
    bni_d = nc.dram_tensor("bnic", [128, 1], f32, kind="ExternalInput")
    bnhh_d = nc.dram_tensor("bnhh", [128, 1], f32, kind="ExternalInput")
    ident16_d = nc.dram_tensor("ident16", [128, 128], bf16,
                               kind="ExternalInput")
    identf_d = nc.dram_tensor("identf", [128, 128], f32,
                              kind="ExternalInput")
    onesc16_d = nc.dram_tensor("onesc16", [128, 1], bf16,
                               kind="ExternalInput")
    out_d = nc.dram_tensor("out", [128, NPC], f32, kind="ExternalOutput")
    if dump == "g":
        dump_d = nc.dram_tensor("dmp", [128, CA * 128], bf16,
                                kind="ExternalOutput")
    elif dump:
        dump_d = nc.dram_tensor("dmp", [NPC, 128], f32,
                                kind="ExternalOutput")

    tabA = tab16_d[0:32768, :]
    tabB = tabb16_d[:]

    def apx(base, dims):
        return bass.AP(base.tensor, base.offset,
                       [list(base.ap[0])] + dims)

    with tile.TileContext(nc) as tc:
        with (
            tc.tile_pool(name="const", bufs=1) as cpool,
            tc.tile_pool(name="gat", bufs=gbufs) as gpool,
            tc.tile_pool(name="oh", bufs=gbufs) as opool,
            tc.tile_pool(name="wrk", bufs=2) as wpool,
            tc.tile_pool(name="brdg", bufs=2) as bpool,
            tc.tile_pool(name="outp", bufs=2) as qpool,
            tc.tile_pool(name="pedge", bufs=1, space="PSUM") as pe_pool,
            tc.tile_pool(name="ptr", bufs=1, space="PSUM") as ptr_pool,
            tc.tile_pool(name="pnode", bufs=1, space="PSUM") as pn_pool,
        ):
            def load(pool, name, dram, shape, dtype=f32):
                t = pool.tile(shape, dtype, tag=name)
                nc.sync.dma_start(t[:], dram[:])
                return t

            ident_sb = load(cpool, "ident16", ident16_d, [128, 128], bf16)
            identf_sb = load(cpool, "identf", identf_d, [128, 128], f32)
            onesc_sb = load(cpool, "onesc16", onesc16_d, [128, 1], bf16)
            wproj_sb = load(cpool, "wproj16", wproj16_d, [128, 128], bf16)
            wih_sb = load(cpool, "wih16", wih16_d, [128, 384], bf16)
            whh_sb = load(cpool, "whh16", whh16_d, [128, 384], bf16)
            bproj_sb = load(cpool, "bprojc", bproj_d, [128, 1])
            brh_sb = load(cpool, "brh", brh_d, [128, 1])
            bzh_sb = load(cpool, "bzh", bzh_d, [128, 1])
            bni_sb = load(cpool, "bnic", bni_d, [128, 1])
            bnhh_sb = load(cpool, "bnhh", bnhh_d, [128, 1])
            idxa_sb = load(cpool, "idxa", idxa_d, [128, LA // 16], i16)
            idxb_sb = load(cpool, "idxb", idxb_d, [128, LB // 16], i16)
            nfT16_sb = load(cpool, "nfT16", nfT16_d, [128, NPC], bf16)

            GA_static = GB_static = None
            if skip_gather:
                GA_static = cpool.tile([128, mxA, 128], bf16, tag="GAs")
                nc.gpsimd.memset(GA_static[:], 0.0)
                GB_static = cpool.tile([128, mxB, 128], bf16, tag="GBs")
                nc.gpsimd.memset(GB_static[:], 0.0)
            OA_static = OB_static = None
            if skip_onehot:
                OA_static = cpool.tile([128, mxA, 128], bf16, tag="OAs")
                nc.gpsimd.memset(OA_static[:], 0.0)
                OB_static = cpool.tile([128, mxB, 128], bf16, tag="OBs")
                nc.gpsimd.memset(OB_static[:], 0.0)

            qload = [0] * nq_use   # greedy per-queue descriptor balancing
            for _rep in range(repeat):
              # node-group state: transpose psum + sbuf ctxT for NB windows
              for b in range(n_batches):
                w0 = b * WPB
                nw = min(WPB, WPC - w0)
                cA0, cB0 = int(ofsA[w0]), int(ofsB[w0])
                ntA = int(ofsA[w0 + nw]) - cA0
                ntB = int(ofsB[w0 + nw]) - cB0
                if skip_gather:
                    GA, GB = GA_static, GB_static
                else:
                    GA = gpool.tile([128, mxA, 128], bf16, tag="GA")
                    GB = gpool.tile([128, mxB, 128], bf16, tag="GB")
                    na = ntA * 128
                    qn = qload.index(min(qload))
                    qload[qn] += na
                    nc.gpsimd.dma_gather(
                        out_ap=GA[:, 0:ntA, :], in_ap=tabA,
                        idxs_ap=idxa_sb[:, (cA0 * 128) // 16:
                                        ((cA0 + ntA) * 128) // 16],
                        num_idxs=na, num_idxs_reg=na, elem_size=128,
                        single_packet=False, queue_num=qn,
                    )
                    nb_ = ntB * 128
                    qn = qload.index(min(qload))
                    qload[qn] += nb_
                    nc.gpsimd.dma_gather(
                        out_ap=GB[:, 0:ntB, :], in_ap=tabB,
                        idxs_ap=idxb_sb[:, (cB0 * 128) // 16:
                                        ((cB0 + ntB) * 128) // 16],
                        num_idxs=nb_, num_idxs_reg=nb_, elem_size=128,
                        single_packet=False, queue_num=qn,
                    )
                if dump == "g" and not skip_gather:
                    nc.sync.dma_start(
                        dump_d[:, cA0 * 128:(cA0 + ntA) * 128],
                        GA[:, 0:ntA, :])
                if skip_onehot:
                    OA, OB = OA_static, OB_static
                else:
                    OA = opool.tile([128, mxA, 128], bf16, tag="OA")
                    OB = opool.tile([128, mxB, 128], bf16, tag="OB")
                    nc.sync.dma_start(
                        OA[:, 0:ntA, :],
                        lma_d[:, cA0 * 128:(cA0 + ntA) * 128])
                    nc.sync.dma_start(
                        OB[:, 0:ntB, :],
                        lmb_d[:, cB0 * 128:(cB0 + ntB) * 128])
                    # O_s = exp(masked logits): the scaled one-hot in one
                    # Activation-engine op (exp(-100) == 0)
                    nc.scalar.activation(OA[:, 0:ntA, :], OA[:, 0:ntA, :],
                                         AF.Exp)
                    nc.scalar.activation(OB[:, 0:ntB, :], OB[:, 0:ntB, :],
                                         AF.Exp)

                for wl in range(nw):
                    w = w0 + wl
                    sAw, sBw = sAp[w], sBp[w]
                    SW = sAw + sBw
                    tA0 = int(ofsA[w]) - cA0    # window tile base in batch
                    tB0 = int(ofsB[w]) - cB0
                    psum_ud = pe_pool.tile([128, 132], f32, tag="psum_ud",
                                           bufs=1 if ud1 else 2)
                    if den_sep:
                        # den accumulates in its own PSUM bank: a second
                        # concurrently-open matmul group in the same 2KB
                        # zero region corrupts the first (hw start_tensor_
                        # calc marks the whole region pending-zero).
                        psum_dn = pe_pool.tile([128, 4], f32, tag="psum_dn",
                                               bufs=2)
                        den_ap = psum_dn[:, 0:1]
                    else:
                        den_ap = psum_ud[:, 128:129]
                    if not skip_mm:
                        def olh_grh(s_):
                            if s_ < sAw:
                                return (OA[:, tA0 + s_, :],
                                        GA[:, tA0 + s_, :])
                            return (OB[:, tB0 + (s_ - sAw), :],
                                    GB[:, tB0 + (s_ - sAw), :])
                        if den_seq:
                            for s_ in range(SW):
                                Olh, Grh = olh_grh(s_)
                                nc.tensor.matmul(
                                    psum_ud[:, 0:128], lhsT=Olh, rhs=Grh,
                                    start=(s_ == 0), stop=(s_ == SW - 1))
                            for s_ in range(SW):
                                Olh, _ = olh_grh(s_)
                                nc.tensor.matmul(
                                    den_ap, lhsT=Olh, rhs=onesc_sb[:],
                                    start=(s_ == 0), stop=(s_ == SW - 1))
                        else:
                            for s_ in range(SW):
                                Olh, Grh = olh_grh(s_)
                                nc.tensor.matmul(
                                    psum_ud[:, 0:128], lhsT=Olh, rhs=Grh,
                                    start=(s_ == 0), stop=(s_ == SW - 1))
                                nc.tensor.matmul(
                                    den_ap, lhsT=Olh,
                                    rhs=onesc_sb[:],
                                    start=(s_ == 0), stop=(s_ == SW - 1))

                    if skip_node:
                        continue

                    # ---- bridge: ctx = psum/den -> bf16 -> [feat, node] ----
                    g = w // NB
                    gl = w % NB
                    nwin = min(NB, WPC - g * NB)
                    trdt = f32 if tr_f32 else bf16
                    if gl == 0:
                        psum_tr = ptr_pool.tile([128, NB * 128], trdt,
                                                tag="psum_tr")
                        cur_tr = psum_tr
                    den = bpool.tile([128, 1], f32, tag="den")
                    nc.vector.tensor_scalar(
                        out=den[:], in0=den_ap, scalar1=1e-30,
                        scalar2=None, op0=OP.max)
                    rec = bpool.tile([128, 1], f32, tag="rec")
                    nc.vector.reciprocal(rec[:], den[:])
                    ctx16 = bpool.tile([128, 128], trdt, tag="ctx16")
                    nc.vector.tensor_scalar(
                        out=ctx16[:], in0=psum_ud[:, 0:128],
                        scalar1=rec[:, 0:1], scalar2=None, op0=OP.mult)
                    if dump == "ctx":
                        cdump = bpool.tile([128, 128], f32, tag="cdump")
                        nc.vector.tensor_scalar(
                            out=cdump[:], in0=psum_ud[:, 0:128],
                            scalar1=rec[:, 0:1], scalar2=None, op0=OP.mult)
                        nc.sync.dma_start(
                            dump_d[w * 128:(w + 1) * 128, :], cdump[:])
                    elif dump == "den":
                        cdump = bpool.tile([128, 128], f32, tag="cdump")
                        nc.vector.tensor_scalar(
                            out=cdump[:, 0:1], in0=den_ap,
                            scalar1=1.0, scalar2=None, op0=OP.mult)
                        nc.vector.tensor_copy(out=cdump[:, 1:128],
                                              in_=psum_ud[:, 1:128])
                        nc.sync.dma_start(
                            dump_d[w * 128:(w + 1) * 128, :], cdump[:])
                    nc.tensor.transpose(
                        cur_tr[:, gl * 128:(gl + 1) * 128], ctx16[:],
                        ident_sb[:] if not tr_f32 else identf_sb[:])

                    if gl != nwin - 1:
                        continue

                    # ---- node phase for group g: windows [g*NB, g*NB+nwin)
                    C = nwin * 128
                    c0 = g * NB * 128
                    ctxT = bpool.tile([128, NB * 128], bf16, tag="ctxT")
                    nc.vector.tensor_copy(out=ctxT[:, 0:C],
                                          in_=cur_tr[:, 0:C])

                    # proj + ELU -> ctxT2 (bf16)
                    psum_cT = pn_pool.tile([128, NB * 128], f32,
                                           tag="psum_cT")
                    nc.tensor.matmul(psum_cT[:, 0:C], lhsT=wproj_sb[:],
                                     rhs=ctxT[:, 0:C], start=True, stop=True)
                    eA = wpool.tile([128, NB * 128], f32, tag="eA")
                    nc.vector.tensor_scalar(
                        out=eA[:, 0:C], in0=psum_cT[:, 0:C],
                        scalar1=bproj_sb[:, 0:1], scalar2=0.0,
                        op0=OP.add, op1=OP.min)
                    nc.scalar.activation(eA[:, 0:C], eA[:, 0:C], AF.Exp)
                    eB = wpool.tile([128, NB * 128], f32, tag="eB")
                    nc.vector.tensor_scalar(
                        out=eB[:, 0:C], in0=psum_cT[:, 0:C],
                        scalar1=bproj_sb[:, 0:1], scalar2=0.0,
                        op0=OP.add, op1=OP.max)
                    # elu = (eA - 1) + eB, fused on DVE
                    ctxT2 = wpool.tile([128, NB * 128], bf16, tag="ctxT2")
                    nc.vector.scalar_tensor_tensor(
                        out=ctxT2[:, 0:C], in0=eA[:, 0:C], scalar=1.0,
                        in1=eB[:, 0:C], op0=OP.subtract, op1=OP.add)

                    # GRU gates, [gate, node] layout, weights stationary.
                    # r/n share a PSUM bank sequentially, h/z share another
                    # (group lifetimes don't overlap; WAR via tile reuse).
                    nf16c = nfT16_sb[:, c0:c0 + C]
                    psum_rn = pn_pool.tile([128, NB * 128], f32,
                                           tag="psum_rn")
                    nc.tensor.matmul(psum_rn[:, 0:C], lhsT=wih_sb[:, 0:128],
                                     rhs=ctxT2[:, 0:C], start=True,
                                     stop=False)
                    nc.tensor.matmul(psum_rn[:, 0:C], lhsT=whh_sb[:, 0:128],
                                     rhs=nf16c, start=False, stop=True)
                    psum_hz = pn_pool.tile([128, NB * 128], f32,
                                           tag="psum_hz")
                    nc.tensor.matmul(psum_hz[:, 0:C],
                                     lhsT=whh_sb[:, 256:384],
                                     rhs=nf16c, start=True, stop=True)

                    # r = sigmoid(s) = 0.5*tanh(0.5 s + 0.5 b_r) + 0.5
                    tr_ = wpool.tile([128, NB * 128], f32, tag="tr_")
                    nc.scalar.activation(tr_[:, 0:C], psum_rn[:, 0:C],
                                         AF.Tanh, bias=brh_sb[:, 0:1],
                                         scale=0.5)
                    # hnb2 = 0.5 h_n + 0.5 b_hn
                    hnb2 = wpool.tile([128, NB * 128], f32, tag="hnb2")
                    nc.vector.tensor_scalar(
                        out=hnb2[:, 0:C], in0=psum_hz[:, 0:C], scalar1=0.5,
                        scalar2=bnhh_sb[:, 0:1], op0=OP.mult, op1=OP.add)

                    # n-gate reuses the r bank; z reuses the h bank
                    nc.tensor.matmul(psum_rn[:, 0:C],
                                     lhsT=wih_sb[:, 256:384],
                                     rhs=ctxT2[:, 0:C], start=True, stop=True)
                    nc.tensor.matmul(psum_hz[:, 0:C],
                                     lhsT=wih_sb[:, 128:256],
                                     rhs=ctxT2[:, 0:C], start=True,
                                     stop=False)
                    nc.tensor.matmul(psum_hz[:, 0:C],
                                     lhsT=whh_sb[:, 128:256],
                                     rhs=nf16c, start=False, stop=True)

                    # n = tanh(i_n + b_in + (tr+1)*hnb2)
                    qq = wpool.tile([128, NB * 128], f32, tag="qq")
                    nc.vector.scalar_tensor_tensor(
                        out=qq[:, 0:C], in0=tr_[:, 0:C], scalar=1.0,
                        in1=hnb2[:, 0:C], op0=OP.add, op1=OP.mult)
                    nc.vector.tensor_tensor(out=qq[:, 0:C], in0=qq[:, 0:C],
                                            in1=psum_rn[:, 0:C], op=OP.add)
                    nn = wpool.tile([128, NB * 128], f32, tag="nn")
                    nc.scalar.activation(nn[:, 0:C], qq[:, 0:C], AF.Tanh,
                                         bias=bni_sb[:, 0:1])
                    tz_ = wpool.tile([128, NB * 128], f32, tag="tz_")
                    nc.scalar.activation(tz_[:, 0:C], psum_hz[:, 0:C],
                                         AF.Tanh, bias=bzh_sb[:, 0:1],
                                         scale=0.5)
                    # h = n + (0.5 tz + 0.5)*(nf - n)
                    dd = wpool.tile([128, NB * 128], f32, tag="dd")
                    nc.vector.tensor_tensor(out=dd[:, 0:C], in0=nf16c,
                                            in1=nn[:, 0:C], op=OP.subtract)
                    uu = wpool.tile([128, NB * 128], f32, tag="uu")
                    nc.vector.scalar_tensor_tensor(
                        out=uu[:, 0:C], in0=tz_[:, 0:C], scalar=1.0,
                        in1=dd[:, 0:C], op0=OP.add, op1=OP.mult)
                    nc.vector.scalar_tensor_tensor(
                        out=dd[:, 0:C], in0=uu[:, 0:C], scalar=0.5,
                        in1=nn[:, 0:C], op0=OP.mult, op1=OP.add)
                    outt = qpool.tile([128, NB * 128], f32, tag="outt")
                    nc.scalar.activation(outt[:, 0:C], dd[:, 0:C], AF.Relu)
                    nc.sync.dma_start(out_d[:, c0:c0 + C], outt[:, 0:C])

    nc.compile()
    return nc


def _prep(edge_logits, node_feats, W_proj, b_proj, w_ih, w_hh, b_ih, b_hh,
          src, dst):
    """Host-side sharding. Returns (T_win, sAp, sBp, in_maps).

    Windows are sorted by edge count and dealt round-robin to (core,
    position) so the 8 windows sharing a position have similar counts;
    slot counts are per-position (max over cores) instead of one global
    max -- cuts gather padding from ~19% to a few %.  _prep stashes the
    window assignment in module global _wassign for kernel() to invert.
    """
    global _wassign
    import ml_dtypes
    BF16 = ml_dtypes.bfloat16

    logits = np.asarray(edge_logits, np.float32).reshape(-1)
    src = np.asarray(src, np.int64)
    dst = np.asarray(dst, np.int64)

    is_b = (src >= S_SPLIT).astype(np.int64)
    win = dst // 128
    key = win * 2 + is_b
    order = np.argsort(key, kind="stable")
    key_s = key[order]
    src_s = src[order]
    dst_s = dst[order]
    log_s = logits[order]

    counts = np.bincount(key_s, minlength=WTOT * 2)
    cA = counts[0::2]
    cB = counts[1::2]

    # sorted round-robin window assignment: rank p -> (core p%NC, pos p//NC)
    worder = np.argsort(-(cA + cB), kind="stable")
    wassign = worder.reshape(WPC, NC).T          # [NC, WPC] window ids
    _wassign = wassign
    core_of = np.zeros(WTOT, np.int64)
    pos_of = np.zeros(WTOT, np.int64)
    core_of[worder] = np.arange(WTOT) % NC
    pos_of[worder] = np.arange(WTOT) // NC

    # per-position slot counts (max over the 8 cores at that position)
    cA_kp = cA[wassign]                          # [NC, WPC]
    cB_kp = cB[wassign]
    sAp = ((cA_kp.max(axis=0) + 127) // 128).astype(np.int64)
    sBp = ((cB_kp.max(axis=0) + 127) // 128).astype(np.int64)
    ofsA = np.concatenate([[0], np.cumsum(sAp)])
    ofsB = np.concatenate([[0], np.cumsum(sBp)])
    CAc = int(ofsA[-1])
    CBc = int(ofsB[-1])
    T_win = int(sAp.sum() + sBp.sum())
    sA_ret = tuple(int(x) for x in sAp)
    sB_ret = tuple(int(x) for x in sBp)

    starts = np.zeros(WTOT * 2, np.int64)
    starts[1:] = np.cumsum(counts)[:-1]
    pos = np.arange(E, dtype=np.int64) - starts[key_s]

    winv = key_s // 2
    grp = key_s % 2
    idxA = np.zeros(NC * CAc * 128, np.int16)
    idxB = np.zeros(NC * CBc * 128, np.int16)
    dstlA = np.full(NC * CAc * 128, -1.0, np.float32)
    dstlB = np.full(NC * CBc * 128, -1.0, np.float32)
    logA = np.zeros(NC * CAc * 128, np.float32)
    logB = np.zeros(NC * CBc * 128, np.float32)

    mA = grp == 0
    mB = ~mA
    wA = winv[mA]
    wB = winv[mB]
    flatA = (core_of[wA] * CAc + ofsA[pos_of[wA]]) * 128 + pos[mA]
    flatB = (core_of[wB] * CBc + ofsB[pos_of[wB]]) * 128 + pos[mB]
    idxA[flatA] = src_s[mA].astype(np.int16)
    idxB[flatB] = (src_s[mB] - OFF_B).astype(np.int16)
    dstlA[flatA] = (dst_s[mA] - wA * 128).astype(np.float32)
    dstlB[flatB] = (dst_s[mB] - wB * 128).astype(np.float32)
    logA[flatA] = log_s[mA]
    logB[flatB] = log_s[mB]

    def core_tiles(a, slots_tot):
        a = a.reshape(NC, slots_tot, 128)
        return [np.ascontiguousarray(a[k].T) for k in range(NC)]

    dstlA_cores = core_tiles(dstlA, CAc)
    dstlB_cores = core_tiles(dstlB, CBc)
    logA_cores = core_tiles(logA, CAc)
    logB_cores = core_tiles(logB, CBc)

    jj = np.arange(128, dtype=np.float32)

    def maskedlog_cores(dstl_cores, log_cores, slots_tot):
        out = []
        for d, lg in zip(dstl_cores, log_cores):
            o = np.where(d[:, :, None] == jj[None, None, :],
                         lg[:, :, None], np.float32(-100.0)).astype(BF16)
            out.append(np.ascontiguousarray(
                o.reshape(128, slots_tot * 128)))
        return out

    lmA_cores = maskedlog_cores(dstlA_cores, logA_cores, CAc)
    lmB_cores = maskedlog_cores(dstlB_cores, logB_cores, CBc)

    def core_idx(a, slots_tot):
        a = a.reshape(NC, slots_tot * 128)
        out = []
        for k in range(NC):
            blk = a[k].reshape(-1, 16).T      # [16, L/16], i -> [i%16,i//16]
            out.append(np.ascontiguousarray(np.tile(blk, (8, 1))))
        return out

    idxA_cores = core_idx(idxA, CAc)
    idxB_cores = core_idx(idxB, CBc)

    nf = np.asarray(node_feats, np.float32)
    nf16 = nf.astype(BF16)
    nf_pad = np.zeros((WTOT * 128, F), np.float32)
    nf_pad[:V] = nf

    tab16 = np.ascontiguousarray(nf16)
    tabb16 = np.ascontiguousarray(nf16[OFF_B:])
    wproj16 = np.ascontiguousarray(np.asarray(W_proj, np.float32).T
                                   .astype(BF16))
    wih16 = np.ascontiguousarray(np.asarray(w_ih, np.float32).T.astype(BF16))
    whh16 = np.ascontiguousarray(np.asarray(w_hh, np.float32).T.astype(BF16))
    bih = np.asarray(b_ih, np.float32).reshape(384)
    bhh = np.asarray(b_hh, np.float32).reshape(384)
    bprojc = np.asarray(b_proj, np.float32).reshape(128, 1)
    brh = (0.5 * (bih[0:128] + bhh[0:128])).reshape(128, 1)
    bzh = (0.5 * (bih[128:256] + bhh[128:256])).reshape(128, 1)
    bnic = bih[256:384].reshape(128, 1)
    bnhh = (0.5 * bhh[256:384]).reshape(128, 1)
    ident16 = np.eye(128, dtype=BF16)
    identf = np.eye(128, dtype=np.float32)
    onesc16 = np.ones((128, 1), BF16)

    nf_win = nf_pad.reshape(WTOT, 128, F)
    in_maps = []
    for k in range(NC):
        sl = nf_win[wassign[k]].reshape(NPC, F)   # core's windows, pos order
        nfT16 = np.ascontiguousarray(sl.T.astype(BF16))
        in_maps.append({
            "idxa": idxA_cores[k], "idxb": idxB_cores[k],
            "lma": lmA_cores[k], "lmb": lmB_cores[k],
            "tab16": tab16, "tabb16": tabb16,
            "nfT16": nfT16,
            "wproj16": wproj16, "wih16": wih16, "whh16": whh16,
            "bprojc": bprojc, "brh": brh, "bzh": bzh,
            "bnic": bnic, "bnhh": bnhh,
            "ident16": ident16, "identf": identf, "onesc16": onesc16,
        })
    return T_win, sA_ret, sB_ret, in_maps


def kernel(edge_logits, node_feats, W_proj, b_proj, w_ih, w_hh, b_ih, b_hh,
           src, dst):
    from concourse.bass_utils import run_bass_kernel_spmd

    T_win, sA, sB, in_maps = _prep(edge_logits, node_feats, W_proj, b_proj,
                                   w_ih, w_hh, b_ih, b_hh, src, dst)
    key = (T_win, sA, sB)
    if key not in _compiled:
        _compiled[key] = _build_nc(T_win, sA=sA, sB=sB)
    nc = _compiled[key]

    res = run_bass_kernel_spmd(nc, in_maps, list(range(NC)))
    full = np.zeros((WTOT * 128, F), np.float32)
    for k in range(NC):
        ok = np.ascontiguousarray(res.results[k]["out"]).T  # [NPC, F]
        full[(_wassign[k][:, None] * 128
              + np.arange(128)[None, :]).reshape(-1)] = ok
    return np.ascontiguousarray(full[:V]).astype(np.float32)
